# revision 35
# baseline (speedup 1.0000x reference)
"""EnhancedGAT Trainium2 Bass kernel (8 NeuronCores, SPMD).

Strategy:
  - Edges are sorted by destination node on the host; core k owns dst nodes
    [k*N/8, (k+1)*N/8) and every edge targeting them. Per-core edge lists are
    bucketed into 64-node bins and padded to 128-edge chunks with a per-bin
    chunk count shared across cores (SPMD uniformity). Dummy (padding) edges
    carry dst-offset 64, which falls outside the 64-wide one-hot used by the
    scatter matmuls, so they contribute exactly nothing.
  - Each GAT layer:
      node phase: every core computes a table row [h(128) | b(4)] for its own
        nodes, where b = per-head <h, att_s + att_d> comes directly out of the
        h matmul via 4 extra weight columns W @ A. Rows live in a [NPC, 256]
        bf16 DRAM table (512B stride for the gather); an AllGather replicates
        it to every core.
      edge phase: per 4096-edge superstep one dma_gather pulls the rows for
        the edges' sources; attention coefficients alpha = b[src] (+ edge
        term) are leaky-relu'd and exp'd in place, messages h*ex are scattered
        into per-bin PSUM accumulators via one-hot matmuls. Softmax is
        unnormalized (max-subtraction skipped; alphas are O(0.3)); the divide
        happens per node at the group epilogue, where self-loop contributions
        are added. As soon as a window-group's epilogue finishes, the NEXT
        layer's node phase for those windows runs (transpose + matmul + table
        write), hiding the layer boundary behind the remaining gathers.
  - Layer 1 additionally computes, per edge, the folded edge-attention terms
    for layers 2-4 (eterm = ea @ V + be, with the padding mask folded in as a
    fifth all-ones/zeros EAT row) plus the per-edge mask into an [C,10] SBUF
    cache, and accumulates per-node mean edge-feature terms and in-degrees
    (extra scatter-matmul columns) used by the self-loops of layers 2-4.
  - Final graph mean-pool via one-hot matmuls into a [33, G] accumulator,
    AllReduce across cores, tiny dense readout replicated on every core.
"""
import sys
import numpy as np

sys.path.insert(0, "/opt/trn_rl_repo")

HID = 32
NCORES = 8
P = 128
BIN = 64
SS = 32          # chunks per superstep
CHUNK = 128
ROW = 256        # table row elements (bf16) for layers 1-3 (512B stride)
ROW4 = 128       # layer-4 table row elements


# ----------------------------------------------------------------- host prep
def host_prep(inputs):
    x = np.asarray(inputs["x"], np.float32)
    ei = np.asarray(inputs["edge_index"]).astype(np.int64)
    ea = np.asarray(inputs["edge_attr"], np.float32)
    batch = np.asarray(inputs["batch"]).astype(np.int64)
    desc = np.asarray(inputs["descriptors"], np.float32)

    N = x.shape[0]
    E = ei.shape[1]
    Gn = desc.shape[0]
    NPC = N // NCORES
    NW = -(-NPC // P)
    NBINS = -(-NPC // BIN)

    src_all, dst_all = ei[0], ei[1]
    order = np.argsort(dst_all, kind="stable")
    src_s, dst_s = src_all[order], dst_all[order]
    ea_s = ea[order]
    core_of = dst_s // NPC
    local = dst_s - core_of * NPC
    bin_of = local // BIN

    cnt = np.zeros((NCORES, NBINS), np.int64)
    np.add.at(cnt, (core_of, bin_of), 1)
    cpb = np.max(-(-cnt // CHUNK), axis=0)          # chunks per bin (shared)
    cpb = np.maximum(cpb, 1)                        # every bin gets a chunk
    C_total = int(cpb.sum())
    off = np.zeros(NBINS, np.int64)
    off[1:] = np.cumsum(cpb)[:-1]
    EP = C_total * CHUNK                            # padded edges per core

    per_core = []
    for k in range(NCORES):
        srck = np.zeros(EP, np.int64)
        dstrk = np.full(EP, float(BIN), np.float32)  # dummies -> dead one-hot
        maskk = np.zeros(EP, np.float32)
        eak = np.zeros((EP, 4), np.float32)
        sel = core_of == k
        bins_k = bin_of[sel]
        start = np.searchsorted(bins_k, np.arange(NBINS))
        pos = np.arange(bins_k.size) - start[bins_k]
        slot = off[bins_k] * CHUNK + pos
        srck[slot] = src_s[sel]
        dstrk[slot] = (local[sel] - bins_k * BIN).astype(np.float32)
        maskk[slot] = 1.0
        eak[slot] = ea_s[sel]

        # device layouts: edge e = c*128 + p
        src16 = np.tile(srck.reshape(-1, 16).T.astype(np.int16), (8, 1))
        dstr_d = dstrk.reshape(C_total, P).T.copy()
        import ml_dtypes
        ea5 = np.concatenate([eak.T, maskk[None, :]], axis=0).astype(ml_dtypes.bfloat16)

        xk = x[k * NPC:(k + 1) * NPC]
        xT = np.zeros((8, NW * P), np.float32)
        xT[:, :NPC] = xk.T
        bk = np.full(NW * P, Gn + 5, np.float32)
        bk[:NPC] = batch[k * NPC:(k + 1) * NPC].astype(np.float32)
        batch_d = bk.reshape(NW, P).T.copy()

        per_core.append(dict(SRC16=src16, DSTR=dstr_d, EAT=ea5,
                             XT=xT, BATCH=batch_d))

    # ---- weight folding
    w = {k: np.asarray(v, np.float32) for k, v in inputs.items()
         if k not in ("x", "edge_index", "edge_attr", "batch", "descriptors")}

    def vfold(We, ae, heads):
        Vp = (We.reshape(w["We_enc"].shape[1], heads, HID) * ae[None]).sum(-1)
        return w["We_enc"] @ Vp, w["be_enc"] @ Vp      # [4,heads],[heads]

    V2, bv2 = vfold(w["We2"], w["ae2"], 4)
    V3, bv3 = vfold(w["We3"], w["ae3"], 4)
    V4, bv4 = vfold(w["We4"], w["ae4"], 1)
    # [5,10]: rows = 4 edge-attr dims + mask; cols = 9 eterms + cnt
    W5x10 = np.zeros((5, 10), np.float32)
    W5x10[0:4, 0:9] = np.concatenate([V2, V3, V4], axis=1)
    W5x10[4, 0:9] = np.concatenate([bv2, bv3, bv4])
    W5x10[4, 9] = 1.0

    def padr(v, n):
        o = np.zeros(n, np.float32)
        o[: v.size] = v
        return o

    # channel-major reorder of the 128-wide (4 heads x 32 ch) dimension:
    # new position c*4+a holds old a*32+c. Keeps per-head broadcasts
    # innermost-packed on DVE (2x mode).
    cm = (np.arange(128) % 4) * 32 + np.arange(128) // 4

    def wext(W, att_s, att_d, heads):
        # append per-head b-columns: b_a = h . (att_s+att_d)_a
        att = (att_s + att_d).reshape(-1)  # [heads*HID] head-major
        if heads == 4:
            attc = att[cm]                 # channel-major to match W cols
            A = np.zeros((128, 4), np.float32)
            A[np.arange(128), np.arange(128) % 4] = attc
        else:
            A = att[:, None]               # [32,1]
        return np.concatenate([W, W @ A], axis=1)

    W1e = wext(w["W1"][:, cm], w["as1"], w["ad1"], 4)            # [8,132]
    W2e = wext(w["W2"][cm][:, cm], w["as2"], w["ad2"], 4)        # [128,132]
    W3e = wext(w["W3"][cm][:, cm], w["as3"], w["ad3"], 4)
    W4e = wext(w["W4"][cm], w["as4"], w["ad4"], 1)               # [128,33]

    bout = np.stack([padr(w["b1"][cm], 128), padr(w["b2"][cm], 128),
                     padr(w["b3"][cm], 128), padr(w["b4"], 128)])

    import ml_dtypes
    T0h = (x @ W1e).astype(np.float32)
    pk0 = np.zeros((N, 256), np.uint8)
    pk0[:, 0:8] = T0h[:, 128:132].astype(ml_dtypes.bfloat16).view(np.uint8)
    pk0[:, 8:136] = T0h[:, 0:128].astype(ml_dtypes.float8_e4m3).view(np.uint8)
    TG0 = pk0.view(ml_dtypes.bfloat16)

    gcnt = np.bincount(batch, minlength=Gn).astype(np.float32)
    cntr = (1.0 / np.maximum(gcnt, 1.0))[None, :]           # [1, Gn]
    shared = dict(
        W1=W1e, WL2=W2e, WL3=W3e, WL4=W4e, TG0=TG0,
        W5X10=W5x10, BOUT=bout, CNTR=cntr,
        WD=w["Wd"], BD=w["bd"][:, None], WLIN=w["Wl"], DESCT=desc.T.copy(),
    )
    bl = float(np.asarray(w["bl"]).reshape(-1)[0])

    dims = dict(N=N, E=E, Gn=Gn, NPC=NPC, NW=NW, NBINS=NBINS,
                C=C_total, cpb=cpb, off=off, bl=bl)
    return dims, shared, per_core


# ------------------------------------------------------------- program build
def build_program(dims, shared):
    import concourse.bass as bass
    import concourse.mybir as mybir
    import concourse.tile as tile
    import concourse.bacc as bacc
    from concourse.masks import make_identity
    from contextlib import ExitStack

    F32 = mybir.dt.float32
    FP8 = mybir.dt.float8e4
    BF16 = mybir.dt.bfloat16
    I32 = mybir.dt.int32
    I16 = mybir.dt.int16
    AF = mybir.ActivationFunctionType
    ALU = mybir.AluOpType
    AX = mybir.AxisListType

    N, Gn, NPC, NW, NBINS, C = (dims[k] for k in ("N", "Gn", "NPC", "NW", "NBINS", "C"))
    cpb, off, bl = dims["cpb"], dims["off"], dims["bl"]
    NSS = C // SS
    # layer params: h width, heads, rhs width, gather row elems
    # PK tables pack rows as [b bf16 x4 | h fp8 x128] (136B) in a 256B stride;
    # HX = leading h-columns multiplied on DVE straight from fp8 (1x mode), the
    # rest is cast to bf16 on Act first so the DVE part runs in 2x mode.
    HXD = dims.get("hx", {0: 96, 1: 48, 2: 48})
    PKL = dims.get("pk_layers", (0, 1, 2))
    LP = [dict(HW=128, AW=4, RW=142, EL=128 if 0 in PKL else ROW,
               GEL=68 if 0 in PKL else 132, PK=0 in PKL, HX=HXD[0]),
          dict(HW=128, AW=4, RW=132, EL=128, GEL=68, PK=1 in PKL, HX=HXD[1]),
          dict(HW=128, AW=4, RW=132, EL=128, GEL=68, PK=2 in PKL, HX=HXD[2]),
          dict(HW=32, AW=1, RW=33, EL=ROW4, GEL=34, PK=False, HX=32)]
    for l_ in (1, 2):
        if not LP[l_]["PK"]:
            LP[l_].update(EL=ROW, GEL=ROW)

    nc = bacc.Bacc(num_swdge_queues=2)
    SIM1 = dims.get("sim1", False)

    # ---- params
    pr = {}
    for nm, shp, dt in [("SRC16", [P, C * 8], I16), ("DSTR", [P, C], F32),
                        ("EAT", [5, C * CHUNK], BF16), ("XT", [8, NW * P], F32),
                        ("BATCH", [P, NW], F32), ("W1", [8, 132], F32),
                        ("WL2", [128, 132], F32), ("WL3", [128, 132], F32),
                        ("WL4", [128, 33], F32), ("W5X10", [5, 10], F32),
                        ("BOUT", [4, 128], F32),
                        ("WD", [48, 32], F32), ("BD", [32, 1], F32),
                        ("WLIN", [64, 1], F32), ("DESCT", [48, Gn], F32),
                        ("CNTR", [1, Gn], F32), ("TG0", [N, 128], BF16)]:
        pr[nm] = nc.declare_dram_parameter(nm, shp, dt, isOutput=False)
    out_p = nc.declare_dram_parameter("out", [1, Gn], F32, isOutput=True)
    pr_TG0_ph = pr["TG0"]

    # ---- internal DRAM
    T_loc = [None] + [nc.dram_tensor(f"T_loc{l}", [NPC, LP[l]["EL"]], BF16)
                      for l in range(1, 4)]
    T_glob = [pr_TG0_ph] + [nc.dram_tensor(f"T_glob{l}", [N, LP[l]["EL"]], BF16,
                                           addr_space="Shared")
                            for l in range(1, 4)]
    ar_in = nc.dram_tensor("ar_in", [32, Gn], F32)
    ar_out = nc.dram_tensor("ar_out", [32, Gn], F32, addr_space="Shared")

    # bin/window bookkeeping (compile-time)
    bin_of_chunk = []
    for b in range(NBINS):
        bin_of_chunk += [b] * int(cpb[b])
    BPW = P // BIN  # bins per window
    win_of_bin = [b // BPW for b in range(NBINS)]
    last_chunk_of_bin = {}
    first_chunk_of_bin = {}
    for c_i, b in enumerate(bin_of_chunk):
        last_chunk_of_bin[b] = c_i
        first_chunk_of_bin.setdefault(b, c_i)

    with tile.TileContext(nc) as tc, ExitStack() as ctx:
        cp = ctx.enter_context(tc.tile_pool(name="const", bufs=1))
        wp = ctx.enter_context(tc.tile_pool(name="work", bufs=2))
        vp = ctx.enter_context(tc.tile_pool(name="win", bufs=2))
        pp = ctx.enter_context(tc.tile_pool(name="psum", bufs=2, space="PSUM"))
        bp = ctx.enter_context(tc.tile_pool(name="binp", bufs=2, space="PSUM"))

        sync, gps, vec, act, pe = nc.sync, nc.gpsimd, nc.vector, nc.scalar, nc.tensor

        def dma_gather_short(out_ap, in_ap, idxs_ap, num_idxs, elem_size,
                             elem_step, queue_num):
            from concourse.bass import exact_div
            eng = gps
            _in_ap = eng.lower_ap_dma(in_ap, for_custom_bir_dma=True)
            _idxs_ap = eng.lower_ap(idxs_ap)
            _out_ap = eng.lower_ap(out_ap)
            stride_bytes_256 = exact_div(elem_step * 2, 256)
            return eng.add_instruction(
                mybir.InstDMAGatherAnt(
                    name=eng.bass.get_next_instruction_name(),
                    ins=[*_in_ap, _idxs_ap,
                         eng.lower_val_access(eng.to_reg(num_idxs))],
                    outs=[_out_ap],
                    transpose=False, num_idxs=num_idxs, elem_size=elem_size,
                    stride_bytes_256=stride_bytes_256, gen_mode=0,
                    single_packet=False, queue_num=queue_num,
                    sbuf_tokens_per_rank=0, sbuf_free_dim_per_rank=0,
                    sbuf_free_dim_pad_per_rank=0, sbuf_byte_offset=0))
        ZTPB = dims.get("ztpb", 1)

        # ---- resident tiles
        src16 = cp.tile([P, C * 8], I16)
        sync.dma_start(out=src16[:], in_=pr["SRC16"][:, :])
        dstr = cp.tile([P, C], BF16)
        gps.dma_start(out=dstr[:], in_=pr["DSTR"][:, :])   # f32 -> bf16 cast
        batcht = cp.tile([P, NW], F32)
        sync.dma_start(out=batcht[:], in_=pr["BATCH"][:, :])
        xT_sb = cp.tile([8, NW * P], BF16)
        gps.dma_start(out=xT_sb[:], in_=pr["XT"][:, :])

        iota_i = cp.tile([P, BIN], I32)
        gps.iota(iota_i[:], pattern=[[1, BIN]], base=0, channel_multiplier=0)
        iotab = cp.tile([P, BIN], BF16)
        vec.tensor_copy(iotab[:], iota_i[:])
        iotag_i = cp.tile([P, Gn], I32)
        gps.iota(iotag_i[:], pattern=[[1, Gn]], base=0, channel_multiplier=0)
        iotagf = cp.tile([P, Gn], F32)
        vec.tensor_copy(iotagf[:], iotag_i[:])
        identf = cp.tile([P, P], F32)
        make_identity(nc, identf[:])

        w1_sb = cp.tile([8, 132], BF16)
        gps.dma_start(out=w1_sb[:], in_=pr["W1"][:, :])
        wl_sb = [None,
                 cp.tile([128, 132], BF16, name="wl2", tag="wl2"),
                 cp.tile([128, 132], BF16, name="wl3", tag="wl3"),
                 cp.tile([128, 33], BF16, name="wl4", tag="wl4")]
        gps.dma_start(out=wl_sb[1][:], in_=pr["WL2"][:, :])   # gpsimd casts f32->bf16
        gps.dma_start(out=wl_sb[2][:], in_=pr["WL3"][:, :])
        gps.dma_start(out=wl_sb[3][:], in_=pr["WL4"][:, :])
        w5x10 = cp.tile([5, 10], BF16)
        gps.dma_start(out=w5x10[:], in_=pr["W5X10"][:, :])
        bout_t = []
        for l in range(4):
            t3 = cp.tile([P, 128], F32, tag=f"bout{l}")
            sync.dma_start(out=t3[:], in_=pr["BOUT"][l:l + 1, :].to_broadcast([P, 128]))
            bout_t.append(t3)

        etc = cp.tile([P, C, 10], BF16)      # eterm9 | cnt  per edge
        pt_all = cp.tile([P, C, BIN], BF16)  # one-hot dst rows per edge
        loop_sb = cp.tile([P, NW, 10], F32)
        gsp = ctx.enter_context(tc.tile_pool(name="gsp", bufs=1, space="PSUM"))
        eap = ctx.enter_context(tc.tile_pool(name="eap", bufs=1))
        gsum_ps = None
        n_pool_mm = [0]

        # ---- readout head start: descriptor branch is input-independent
        comb = cp.tile([64, Gn], F32)
        wd_sb = cp.tile([48, 32], F32)
        sync.dma_start(out=wd_sb[:], in_=pr["WD"][:, :])
        desct_sb = cp.tile([48, Gn], F32)
        sync.dma_start(out=desct_sb[:], in_=pr["DESCT"][:, :])
        bd_sb = cp.tile([32, 1], F32)
        sync.dma_start(out=bd_sb[:], in_=pr["BD"][:, :])
        dps = pp.tile([32, Gn], F32, tag="hps", bufs=4 - ZTPB)
        pe.matmul(out=dps[:], lhsT=wd_sb[:], rhs=desct_sb[:], start=True, stop=True)
        act.activation(out=comb[32:64, :], in_=dps[:], func=AF.Relu, bias=bd_sb[:])
        wlin_sb = cp.tile([64, 1], F32)
        sync.dma_start(out=wlin_sb[:], in_=pr["WLIN"][:, :])
        cntrb = cp.tile([32, Gn], F32)
        sync.dma_start(out=cntrb[:], in_=pr["CNTR"][0:1, :].to_broadcast([32, Gn]))

        WG = dims.get("wg", 5)  # max windows per epilogue group
        # non-uniform groups: taper toward the end so the serial layer-boundary
        # tail (last epilogue -> node phase -> AllGather) shrinks
        grp_bounds = []
        w0_ = 0
        while NW - w0_ > 10:
            grp_bounds.append((w0_, WG))
            w0_ += WG
        for t_ in dims.get("taper", (4, 3, 2, 1)):
            if NW - w0_ > t_:
                grp_bounds.append((w0_, t_))
                w0_ += t_
        if NW > w0_:
            grp_bounds.append((w0_, NW - w0_))
        NG = len(grp_bounds)
        grp_of_win = {}
        for gi, (gw0, gsz_) in enumerate(grp_bounds):
            for w_ in range(gw0, gw0 + gsz_):
                grp_of_win[w_] = gi
        last_chunk_of_grp = {}
        for b in range(NBINS):
            g_ = grp_of_win[win_of_bin[b]]
            last_chunk_of_grp[g_] = max(last_chunk_of_grp.get(g_, -1),
                                        last_chunk_of_bin[b])

        # T_sb tables: [P, NW, 132] (h | b); layer l+1's is built during
        # layer l's edge phase, group by group.
        def node_phase_group(l, g_, T_next, z_src):
            """Build T_next rows for group g_ of layer l (0-based), write T_loc."""
            w0, gsz = grp_bounds[g_]
            HWn = LP[l]["HW"]
            BW = HWn + LP[l]["AW"]  # table row width
            for w_ in range(w0, w0 + gsz):
                if l == 0:
                    hps = pp.tile([P, 132], F32, tag="hps", bufs=4 - ZTPB)
                    pe.matmul(out=hps[:, 0:BW], lhsT=xT_sb[:, w_ * P:(w_ + 1) * P],
                              rhs=w1_sb[:], start=True, stop=True)
                else:
                    ztp = pp.tile([P, P], F32, tag="ztp", bufs=ZTPB)
                    pe.transpose(out=ztp[:], in_=z_src[:, w_ - w0, 0:128],
                                 identity=identf[:])
                    zt_sb = wp.tile([P, P], BF16, tag="ztsb")
                    act.copy(out=zt_sb[:], in_=ztp[:])
                    hps = pp.tile([P, 132], F32, tag="hps", bufs=4 - ZTPB)
                    pe.matmul(out=hps[:, 0:BW], lhsT=zt_sb[:], rhs=wl_sb[l][:],
                              start=True, stop=True)
                act.copy(out=T_next[:, w_, 0:BW], in_=hps[:, 0:BW])
                if l > 0 and LP[l]["PK"]:
                    act.copy(out=Tpk[:, w_, 0:4], in_=hps[:, 128:132])
                    act.copy(out=Tpk[:, w_, 4:68].bitcast(FP8), in_=hps[:, 0:128])
            if l == 0:
                return  # layer-1 table ships as the TG0 param; SBUF copy only
            stage, SW = (Tpk, 68) if LP[l]["PK"] else (T_next, BW)
            # batched table write: full windows in one DMA, ragged tail apart
            wfull = gsz - (1 if (w0 + gsz) * P > NPC else 0)
            if wfull > 0:
                sync.dma_start(
                    out=T_loc[l][w0 * P:(w0 + wfull) * P, 0:SW]
                        .rearrange("(w p) e -> p w e", p=P),
                    in_=stage[:, w0:w0 + wfull, 0:SW])
            if wfull < gsz:
                w_ = w0 + wfull
                nr = NPC - w_ * P
                sync.dma_start(out=T_loc[l][w_ * P:w_ * P + nr, 0:SW],
                               in_=stage[0:nr, w_, 0:SW])

        PT_AHEAD = dims.get("pt_ahead", 64)
        # prebuild the one-hot cache for the first chunks while the layer-0
        # node phase occupies PE/Act
        for g in range(0, PT_AHEAD, 8):
            vec.tensor_tensor(
                out=pt_all[:, g:g + 8, :],
                in0=dstr[:, g:g + 8].unsqueeze(2).to_broadcast([P, 8, BIN]),
                in1=iotab[:].unsqueeze(1).to_broadcast([P, 8, BIN]),
                op=ALU.is_equal)

        # ---- layer 0 node phase (all groups up front)
        T_sb = wp.tile([P, NW, 132], BF16, tag="tsb")
        for g_ in range(NG):
            node_phase_group(0, g_, T_sb, None)

        for l in range(4):
            HW, AW, RW, EL, GEL, PK, HX = (
                LP[l][k] for k in ("HW", "AW", "RW", "EL", "GEL", "PK", "HX"))
            BW = HW + AW

            T_next = None
            if l < 3:
                T_next = wp.tile([P, NW, 132], BF16, name="tnext", tag="tsb")
                if LP[l + 1]["PK"]:
                    Tpk = wp.tile([P, NW, 68], BF16, name="tpk", tag="tpk", bufs=1)

            grp_tiles = {}
            grp_done = set()

            def open_group(g_):
                t = vp.tile([P, WG, 142], F32, name="wingrp", tag="wingrp")
                grp_tiles[g_] = t
                return t

            def epilogue_group(g_):
                w0, gsz = grp_bounds[g_]
                wg = grp_tiles[g_]
                scr = wp.tile([P, WG, 12], F32, name="scr", tag="scr")
                # self-loop alpha (= b_own [+ eterm means]) -> exp
                if l > 0:
                    sl = [None, (0, 4), (4, 8), (8, 9)][l]
                    vec.tensor_tensor(out=scr[:, 0:gsz, 0:AW],
                                      in0=T_sb[:, w0:w0 + gsz, HW:HW + AW],
                                      in1=loop_sb[:, w0:w0 + gsz, sl[0]:sl[1]],
                                      op=ALU.add)
                else:
                    act.copy(out=scr[:, 0:gsz, 0:AW],
                             in_=T_sb[:, w0:w0 + gsz, HW:HW + AW])
                vec.tensor_scalar_mul(out=scr[:, 0:gsz, 4:4 + AW],
                                      in0=scr[:, 0:gsz, 0:AW], scalar1=0.2)
                vec.tensor_tensor(out=scr[:, 0:gsz, 0:AW], in0=scr[:, 0:gsz, 0:AW],
                                  in1=scr[:, 0:gsz, 4:4 + AW], op=ALU.max)
                act.activation(out=scr[:, 0:gsz, 0:AW], in_=scr[:, 0:gsz, 0:AW],
                               func=AF.Exp)
                # num += h_own * ex_loop
                nt = wp.tile([P, WG, 128], BF16, name="nt", tag="nt")
                vec.tensor_tensor(
                    out=nt[:, 0:gsz, 0:HW].rearrange("p g (c a) -> p g c a", a=AW),
                    in0=T_sb[:, w0:w0 + gsz, 0:HW].rearrange("p g (c a) -> p g c a", a=AW),
                    in1=scr[:, 0:gsz, 0:AW].unsqueeze(2)
                        .to_broadcast([P, gsz, HW // AW, AW]),
                    op=ALU.mult)
                vec.tensor_tensor(out=wg[:, 0:gsz, 0:HW], in0=wg[:, 0:gsz, 0:HW],
                                  in1=nt[:, 0:gsz, 0:HW], op=ALU.add)
                # den -> reciprocal
                vec.tensor_tensor(out=scr[:, 0:gsz, 4:4 + AW],
                                  in0=wg[:, 0:gsz, HW:HW + AW],
                                  in1=scr[:, 0:gsz, 0:AW], op=ALU.add)
                vec.tensor_scalar_add(out=scr[:, 0:gsz, 4:4 + AW],
                                      in0=scr[:, 0:gsz, 4:4 + AW], scalar1=1e-16)
                vec.reciprocal(out=scr[:, 0:gsz, 4:4 + AW], in_=scr[:, 0:gsz, 4:4 + AW])
                if l == 0:
                    vec.tensor_scalar_max(out=scr[:, 0:gsz, 8:9],
                                          in0=wg[:, 0:gsz, 141:142], scalar1=1.0)
                    vec.reciprocal(out=scr[:, 0:gsz, 8:9], in_=scr[:, 0:gsz, 8:9])
                    vec.tensor_tensor(
                        out=loop_sb[:, w0:w0 + gsz, 0:9], in0=wg[:, 0:gsz, 132:141],
                        in1=scr[:, 0:gsz, 8:9].to_broadcast([P, gsz, 9]), op=ALU.mult)
                # z = num * recip(den) + bias [+ relu]
                vec.tensor_tensor(
                    out=wg[:, 0:gsz, 0:HW].rearrange("p g (c a) -> p g c a", a=AW),
                    in0=wg[:, 0:gsz, 0:HW].rearrange("p g (c a) -> p g c a", a=AW),
                    in1=scr[:, 0:gsz, 4:4 + AW].unsqueeze(2)
                        .to_broadcast([P, gsz, HW // AW, AW]),
                    op=ALU.mult)
                vec.tensor_tensor(
                    out=wg[:, 0:gsz, 0:HW], in0=wg[:, 0:gsz, 0:HW],
                    in1=bout_t[l][:, 0:HW].unsqueeze(1).to_broadcast([P, gsz, HW]),
                    op=ALU.add)
                if l < 3:
                    act.activation(out=wg[:, 0:gsz, 0:128], in_=wg[:, 0:gsz, 0:128],
                                   func=AF.Relu)
                    node_phase_group(l + 1, g_, T_next, wg)
                else:
                    nonlocal gsum_ps
                    pool_sb = wp.tile([P, WG, 32], BF16, name="pool_sb", tag="poolsb")
                    act.copy(out=pool_sb[:, 0:gsz, 0:32], in_=wg[:, 0:gsz, 0:32])
                    bt = wp.tile([P, WG, Gn], BF16, name="bt", tag="bt", bufs=1)
                    vec.tensor_tensor(
                        out=bt[:, 0:gsz, :],
                        in0=batcht[:, w0:w0 + gsz].unsqueeze(2).to_broadcast([P, gsz, Gn]),
                        in1=iotagf[:].unsqueeze(1).to_broadcast([P, gsz, Gn]),
                        op=ALU.is_equal)
                    if gsum_ps is None:
                        gsum_ps = gsp.tile([32, Gn], F32, name="gsum_ps")
                    for j_ in range(gsz):
                        n_pool_mm[0] += 1
                        pe.matmul(out=gsum_ps[:], lhsT=pool_sb[:, j_, :],
                                  rhs=bt[:, j_, :],
                                  start=(n_pool_mm[0] == 1),
                                  stop=(n_pool_mm[0] == NW))
                grp_done.add(g_)

            cur_bin_tile = {}
            ss_plan = []
            rem_ = C
            while rem_ > 0:
                n_ = min(SS, rem_)
                ss_plan.append(n_)
                rem_ -= n_
            for t_ in dims.get("ss_tail", (8,)):
                if ss_plan[-1] > t_:
                    ss_plan[-1] -= t_
                    ss_plan.append(t_)
            s0 = 0
            GW = GEL if PK else max(GEL, RW)
            for ss, NCH in enumerate(ss_plan):
                Gt = wp.tile([P, SS, GW], BF16, tag="gt" if GW > 68 else "gtp", bufs=4)
                if GEL == EL:
                    gps.dma_gather(
                        out_ap=Gt[:, 0:NCH, 0:GEL], in_ap=T_glob[l][:, :],
                        idxs_ap=src16[:, s0 * 8:(s0 + NCH) * 8],
                        num_idxs=NCH * CHUNK, num_idxs_reg=NCH * CHUNK,
                        elem_size=EL, single_packet=False, queue_num=ss % 2)
                else:
                    dma_gather_short(
                        out_ap=Gt[:, 0:NCH, 0:GEL], in_ap=T_glob[l][:, 0:GEL],
                        idxs_ap=src16[:, s0 * 8:(s0 + NCH) * 8],
                        num_idxs=NCH * CHUNK, elem_size=GEL, elem_step=EL,
                        queue_num=ss % 2)
                if l == 0:
                    # edge-term + mask precompute (feeds rhs cols 132:142 +
                    # later layers' alpha); mask folded into EAT row 5.
                    eaT_sl = eap.tile([5, SS * CHUNK], BF16, name="easl", tag="eat")
                    sync.dma_start(
                        out=eaT_sl[:, 0:NCH * CHUNK],
                        in_=pr["EAT"][:, s0 * CHUNK:(s0 + NCH) * CHUNK])
                    for q0 in range(0, NCH, 16):
                        qn = min(16, NCH - q0)
                        etp = pp.tile([P, 160], F32, tag="etp", bufs=1)
                        for j in range(qn):
                            ci = q0 + j
                            pe.matmul(out=etp[:, j * 10:(j + 1) * 10],
                                      lhsT=eaT_sl[:, ci * CHUNK:(ci + 1) * CHUNK],
                                      rhs=w5x10[:], start=True, stop=True)
                        act.copy(out=etc[:, s0 + q0:s0 + q0 + qn, :]
                                 .rearrange("p a b -> p (a b)"),
                                 in_=etp[:, 0:qn * 10])
                    # staircase one-hots built once, reused by all layers;
                    # built PT_AHEAD chunks ahead so the DVE cost sits in the
                    # pipeline's slack instead of its critical phase
                    pb0 = PT_AHEAD + s0
                    pb1 = min(pb0 + NCH, C)
                    for g in range(pb0, pb1, 8):
                        gn = min(8, pb1 - g)
                        vec.tensor_tensor(
                            out=pt_all[:, g:g + gn, :],
                            in0=dstr[:, g:g + gn].unsqueeze(2).to_broadcast([P, gn, BIN]),
                            in1=iotab[:].unsqueeze(1).to_broadcast([P, gn, BIN]),
                            op=ALU.is_equal)
                # alpha = b[src] (+ eterm) -> leaky relu -> exp
                AT = wp.tile([P, SS, 8], BF16, tag="at", bufs=2)
                if PK:
                    SCT = wp.tile([P, SS, 142], BF16, tag="rhs", bufs=3)
                    BS = 0            # b slot in the packed gathered row
                else:
                    SCT = Gt
                    BS = HW
                if l > 0:
                    sl = [None, (0, 4), (4, 8), (8, 9)][l]
                    vec.tensor_tensor(out=AT[:, 0:NCH, 0:AW],
                                      in0=Gt[:, 0:NCH, BS:BS + AW],
                                      in1=etc[:, s0:s0 + NCH, sl[0]:sl[1]],
                                      op=ALU.add)
                    vec.tensor_scalar_mul(out=AT[:, 0:NCH, AW:2 * AW],
                                          in0=AT[:, 0:NCH, 0:AW], scalar1=0.2)
                    vec.tensor_tensor(out=AT[:, 0:NCH, 0:AW], in0=AT[:, 0:NCH, 0:AW],
                                      in1=AT[:, 0:NCH, AW:2 * AW], op=ALU.max)
                else:
                    vec.tensor_scalar_mul(out=AT[:, 0:NCH, AW:2 * AW],
                                          in0=Gt[:, 0:NCH, BS:BS + AW], scalar1=0.2)
                    vec.tensor_tensor(out=AT[:, 0:NCH, 0:AW],
                                      in0=Gt[:, 0:NCH, BS:BS + AW],
                                      in1=AT[:, 0:NCH, AW:2 * AW], op=ALU.max)
                act.activation(out=SCT[:, 0:NCH, HW:HW + AW], in_=AT[:, 0:NCH, 0:AW],
                               func=AF.Exp)
                if PK:
                    # h x ex: leading HX columns straight from fp8 on DVE (1x);
                    # the rest cast to bf16 on Act, then multiplied in 2x mode
                    vec.tensor_tensor(
                        out=SCT[:, 0:NCH, 0:HX].rearrange("p s (c a) -> p s c a", a=AW),
                        in0=Gt[:, 0:NCH, 4:4 + HX // 2].bitcast(FP8)
                            .rearrange("p s (c a) -> p s c a", a=AW),
                        in1=SCT[:, 0:NCH, HW:HW + AW].unsqueeze(2)
                            .to_broadcast([P, NCH, HX // AW, AW]),
                        op=ALU.mult)
                    if HX < HW:
                        act.copy(out=SCT[:, 0:NCH, HX:HW],
                                 in_=Gt[:, 0:NCH, 4 + HX // 2:4 + HW // 2].bitcast(FP8))
                        vec.tensor_tensor(
                            out=SCT[:, 0:NCH, HX:HW].rearrange("p s (c a) -> p s c a", a=AW),
                            in0=SCT[:, 0:NCH, HX:HW].rearrange("p s (c a) -> p s c a", a=AW),
                            in1=SCT[:, 0:NCH, HW:HW + AW].unsqueeze(2)
                                .to_broadcast([P, NCH, (HW - HX) // AW, AW]),
                            op=ALU.mult)
                else:
                    vec.tensor_tensor(
                        out=SCT[:, 0:NCH, 0:HW].rearrange("p s (c a) -> p s c a", a=AW),
                        in0=SCT[:, 0:NCH, 0:HW].rearrange("p s (c a) -> p s c a", a=AW),
                        in1=SCT[:, 0:NCH, HW:HW + AW].unsqueeze(2)
                            .to_broadcast([P, NCH, HW // AW, AW]),
                        op=ALU.mult)
                if l == 0:
                    # append eterm9|cnt as rhs cols 132:142
                    act.copy(out=SCT[:, 0:NCH, 132:142],
                             in_=etc[:, s0:s0 + NCH, :])
                # scatter matmuls
                for c_i in range(NCH):
                    gc = s0 + c_i
                    b = bin_of_chunk[gc]
                    w_ = win_of_bin[b]
                    g_ = grp_of_win[w_]
                    if g_ not in grp_tiles:
                        open_group(g_)
                    if gc == first_chunk_of_bin[b]:
                        cur_bin_tile[b] = bp.tile([BIN, 142], F32, name="binacc",
                                                  tag="binacc")
                    pe.matmul(out=cur_bin_tile[b][:, 0:RW],
                              lhsT=pt_all[:, gc, :], rhs=SCT[:, c_i, 0:RW],
                              start=(gc == first_chunk_of_bin[b]),
                              stop=(gc == last_chunk_of_bin[b]))
                    if gc == last_chunk_of_bin[b]:
                        j = b % BPW
                        wrel = w_ - grp_bounds[g_][0]
                        act.copy(out=grp_tiles[g_][BIN * j:BIN * (j + 1), wrel, 0:RW],
                                 in_=cur_bin_tile[b][:, 0:RW])
                        del cur_bin_tile[b]
                    if gc == last_chunk_of_grp.get(g_, None):
                        epilogue_group(g_)
                s0 += NCH
            # groups never triggered (e.g. all-empty windows)
            for g_ in range(NG):
                if g_ not in grp_done:
                    if g_ not in grp_tiles:
                        open_group(g_)
                    epilogue_group(g_)
            if l < 3:
                if SIM1:
                    sync.dma_start(out=T_glob[l + 1][0:NPC, :], in_=T_loc[l + 1][:, :])
                else:
                    gps.collective_compute(
                        "AllGather", ALU.bypass, replica_groups=[list(range(NCORES))],
                        ins=[T_loc[l + 1][:, :]], outs=[T_glob[l + 1][:, :]])
                T_sb = T_next

        # ============ readout
        gsum_sb = cp.tile([32, Gn], F32)
        act.copy(out=gsum_sb[:], in_=gsum_ps[:])
        sync.dma_start(out=ar_in[:], in_=gsum_sb[:])
        if SIM1:
            sync.dma_start(out=ar_out[:], in_=ar_in[:])
        else:
            gps.collective_compute("AllReduce", ALU.add,
                                   replica_groups=[list(range(NCORES))],
                                   ins=[ar_in[:]], outs=[ar_out[:]])
        gs = cp.tile([32, Gn], F32)
        sync.dma_start(out=gs[:], in_=ar_out[:])
        vec.tensor_tensor(out=comb[0:32, :], in0=gs[:, :], in1=cntrb[:],
                          op=ALU.mult)
        blt = cp.tile([1, 1], F32)
        vec.memset(blt[:], bl)
        fin = pp.tile([1, Gn], F32, tag="hps", bufs=4 - ZTPB)
        pe.matmul(out=fin[:], lhsT=wlin_sb[:], rhs=comb[:], start=True, stop=True)
        res_sb = cp.tile([1, Gn], F32)
        act.activation(out=res_sb[:], in_=fin[:], func=AF.Sigmoid, bias=blt[:])
        sync.dma_start(out=out_p[:, :], in_=res_sb[:])

    nc.finalize()
    return nc


# ------------------------------------------------------------------ entry
def _run(inputs, trace=False, debug=False):
    dims, shared, per_core = host_prep(inputs)
    nc = build_program(dims, shared)
    in_maps = [{**shared, **pc} for pc in per_core]
    from concourse.bass_utils import run_bass_kernel_spmd
    return run_bass_kernel_spmd(nc, in_maps, list(range(NCORES)), trace=trace)


def kernel(**inputs):
    res = _run(inputs)
    return res.results[0]["out"].reshape(-1).astype(np.float32)


# revision 38
# speedup vs baseline: 1.0150x; 1.0150x over previous
"""EnhancedGAT Trainium2 Bass kernel (8 NeuronCores, SPMD).

Strategy:
  - Edges are sorted by destination node on the host; core k owns dst nodes
    [k*N/8, (k+1)*N/8) and every edge targeting them. Per-core edge lists are
    bucketed into 64-node bins and padded to 128-edge chunks with a per-bin
    chunk count shared across cores (SPMD uniformity). Dummy (padding) edges
    carry dst-offset 64, which falls outside the 64-wide one-hot used by the
    scatter matmuls, so they contribute exactly nothing.
  - Each GAT layer:
      node phase: every core computes a table row [h(128) | b(4)] for its own
        nodes, where b = per-head <h, att_s + att_d> comes directly out of the
        h matmul via 4 extra weight columns W @ A. Rows live in a [NPC, 256]
        bf16 DRAM table (512B stride for the gather); an AllGather replicates
        it to every core.
      edge phase: per 4096-edge superstep one dma_gather pulls the rows for
        the edges' sources; attention coefficients alpha = b[src] (+ edge
        term) are leaky-relu'd and exp'd in place, messages h*ex are scattered
        into per-bin PSUM accumulators via one-hot matmuls. Softmax is
        unnormalized (max-subtraction skipped; alphas are O(0.3)); the divide
        happens per node at the group epilogue, where self-loop contributions
        are added. As soon as a window-group's epilogue finishes, the NEXT
        layer's node phase for those windows runs (transpose + matmul + table
        write), hiding the layer boundary behind the remaining gathers.
  - Layer 1 additionally computes, per edge, the folded edge-attention terms
    for layers 2-4 (eterm = ea @ V + be, with the padding mask folded in as a
    fifth all-ones/zeros EAT row) plus the per-edge mask into an [C,10] SBUF
    cache, and accumulates per-node mean edge-feature terms and in-degrees
    (extra scatter-matmul columns) used by the self-loops of layers 2-4.
  - Final graph mean-pool via one-hot matmuls into a [33, G] accumulator,
    AllReduce across cores, tiny dense readout replicated on every core.
"""
import sys
import numpy as np

sys.path.insert(0, "/opt/trn_rl_repo")

HID = 32
NCORES = 8
P = 128
BIN = 64
SS = 32          # chunks per superstep
CHUNK = 128
ROW = 256        # table row elements (bf16) for layers 1-3 (512B stride)
ROW4 = 128       # layer-4 table row elements


# ----------------------------------------------------------------- host prep
def host_prep(inputs):
    x = np.asarray(inputs["x"], np.float32)
    ei = np.asarray(inputs["edge_index"]).astype(np.int64)
    ea = np.asarray(inputs["edge_attr"], np.float32)
    batch = np.asarray(inputs["batch"]).astype(np.int64)
    desc = np.asarray(inputs["descriptors"], np.float32)

    N = x.shape[0]
    E = ei.shape[1]
    Gn = desc.shape[0]
    NPC = N // NCORES
    NW = -(-NPC // P)
    NBINS = -(-NPC // BIN)

    src_all, dst_all = ei[0], ei[1]
    order = np.argsort(dst_all, kind="stable")
    src_s, dst_s = src_all[order], dst_all[order]
    ea_s = ea[order]
    core_of = dst_s // NPC
    local = dst_s - core_of * NPC
    bin_of = local // BIN

    cnt = np.zeros((NCORES, NBINS), np.int64)
    np.add.at(cnt, (core_of, bin_of), 1)
    cpb = np.max(-(-cnt // CHUNK), axis=0)          # chunks per bin (shared)
    cpb = np.maximum(cpb, 1)                        # every bin gets a chunk
    C_total = int(cpb.sum())
    off = np.zeros(NBINS, np.int64)
    off[1:] = np.cumsum(cpb)[:-1]
    EP = C_total * CHUNK                            # padded edges per core

    per_core = []
    for k in range(NCORES):
        srck = np.zeros(EP, np.int64)
        dstrk = np.full(EP, float(BIN), np.float32)  # dummies -> dead one-hot
        maskk = np.zeros(EP, np.float32)
        eak = np.zeros((EP, 4), np.float32)
        sel = core_of == k
        bins_k = bin_of[sel]
        start = np.searchsorted(bins_k, np.arange(NBINS))
        pos = np.arange(bins_k.size) - start[bins_k]
        slot = off[bins_k] * CHUNK + pos
        srck[slot] = src_s[sel]
        dstrk[slot] = (local[sel] - bins_k * BIN).astype(np.float32)
        maskk[slot] = 1.0
        eak[slot] = ea_s[sel]

        # device layouts: edge e = c*128 + p
        src16 = np.tile(srck.reshape(-1, 16).T.astype(np.int16), (8, 1))
        dstr_d = dstrk.reshape(C_total, P).T.copy()
        import ml_dtypes
        ea5 = np.concatenate([eak.T, maskk[None, :]], axis=0).astype(ml_dtypes.bfloat16)

        xk = x[k * NPC:(k + 1) * NPC]
        xT = np.zeros((8, NW * P), np.float32)
        xT[:, :NPC] = xk.T
        bk = np.full(NW * P, Gn + 5, np.float32)
        bk[:NPC] = batch[k * NPC:(k + 1) * NPC].astype(np.float32)
        batch_d = bk.reshape(NW, P).T.copy()

        per_core.append(dict(SRC16=src16, DSTR=dstr_d, EAT=ea5,
                             XT=xT, BATCH=batch_d))

    # ---- weight folding
    w = {k: np.asarray(v, np.float32) for k, v in inputs.items()
         if k not in ("x", "edge_index", "edge_attr", "batch", "descriptors")}

    def vfold(We, ae, heads):
        Vp = (We.reshape(w["We_enc"].shape[1], heads, HID) * ae[None]).sum(-1)
        return w["We_enc"] @ Vp, w["be_enc"] @ Vp      # [4,heads],[heads]

    V2, bv2 = vfold(w["We2"], w["ae2"], 4)
    V3, bv3 = vfold(w["We3"], w["ae3"], 4)
    V4, bv4 = vfold(w["We4"], w["ae4"], 1)
    # [5,10]: rows = 4 edge-attr dims + mask; cols = 9 eterms + cnt
    W5x10 = np.zeros((5, 10), np.float32)
    W5x10[0:4, 0:9] = np.concatenate([V2, V3, V4], axis=1)
    W5x10[4, 0:9] = np.concatenate([bv2, bv3, bv4])
    W5x10[4, 9] = 1.0

    def padr(v, n):
        o = np.zeros(n, np.float32)
        o[: v.size] = v
        return o

    # channel-major reorder of the 128-wide (4 heads x 32 ch) dimension:
    # new position c*4+a holds old a*32+c. Keeps per-head broadcasts
    # innermost-packed on DVE (2x mode).
    cm = (np.arange(128) % 4) * 32 + np.arange(128) // 4

    def wext(W, att_s, att_d, heads):
        # append per-head b-columns: b_a = h . (att_s+att_d)_a
        att = (att_s + att_d).reshape(-1)  # [heads*HID] head-major
        if heads == 4:
            attc = att[cm]                 # channel-major to match W cols
            A = np.zeros((128, 4), np.float32)
            A[np.arange(128), np.arange(128) % 4] = attc
        else:
            A = att[:, None]               # [32,1]
        return np.concatenate([W, W @ A], axis=1)

    W1e = wext(w["W1"][:, cm], w["as1"], w["ad1"], 4)            # [8,132]
    W2e = wext(w["W2"][cm][:, cm], w["as2"], w["ad2"], 4)        # [128,132]
    W3e = wext(w["W3"][cm][:, cm], w["as3"], w["ad3"], 4)
    W4e = wext(w["W4"][cm], w["as4"], w["ad4"], 1)               # [128,33]

    bout = np.stack([padr(w["b1"][cm], 128), padr(w["b2"][cm], 128),
                     padr(w["b3"][cm], 128), padr(w["b4"], 128)])

    import ml_dtypes
    T0h = (x @ W1e).astype(np.float32)
    pk0 = np.zeros((N, 256), np.uint8)
    pk0[:, 0:8] = T0h[:, 128:132].astype(ml_dtypes.bfloat16).view(np.uint8)
    pk0[:, 8:136] = T0h[:, 0:128].astype(ml_dtypes.float8_e4m3).view(np.uint8)
    TG0 = pk0.view(ml_dtypes.bfloat16)

    gcnt = np.bincount(batch, minlength=Gn).astype(np.float32)
    cntr = (1.0 / np.maximum(gcnt, 1.0))[None, :]           # [1, Gn]
    shared = dict(
        W1=W1e, WL2=W2e, WL3=W3e, WL4=W4e, TG0=TG0,
        W5X10=W5x10, BOUT=bout, CNTR=cntr,
        WD=w["Wd"], BD=w["bd"][:, None], WLIN=w["Wl"], DESCT=desc.T.copy(),
    )
    bl = float(np.asarray(w["bl"]).reshape(-1)[0])

    dims = dict(N=N, E=E, Gn=Gn, NPC=NPC, NW=NW, NBINS=NBINS,
                C=C_total, cpb=cpb, off=off, bl=bl)
    return dims, shared, per_core


# ------------------------------------------------------------- program build
def build_program(dims, shared):
    import concourse.bass as bass
    import concourse.mybir as mybir
    import concourse.tile as tile
    import concourse.bacc as bacc
    from concourse.masks import make_identity
    from contextlib import ExitStack

    F32 = mybir.dt.float32
    FP8 = mybir.dt.float8e4
    BF16 = mybir.dt.bfloat16
    I32 = mybir.dt.int32
    I16 = mybir.dt.int16
    AF = mybir.ActivationFunctionType
    ALU = mybir.AluOpType
    AX = mybir.AxisListType

    N, Gn, NPC, NW, NBINS, C = (dims[k] for k in ("N", "Gn", "NPC", "NW", "NBINS", "C"))
    cpb, off, bl = dims["cpb"], dims["off"], dims["bl"]
    NSS = C // SS
    # layer params: h width, heads, rhs width, gather row elems
    # PK tables pack rows as [b bf16 x4 | h fp8 x128] (136B) in a 256B stride;
    # HX = leading h-columns multiplied on DVE straight from fp8 (1x mode), the
    # rest is cast to bf16 on Act first so the DVE part runs in 2x mode.
    HXD = dims.get("hx", {0: 96, 1: 48, 2: 48})
    PKL = dims.get("pk_layers", (0, 1, 2))
    LP = [dict(HW=128, AW=4, RW=142, EL=128 if 0 in PKL else ROW,
               GEL=68 if 0 in PKL else 132, PK=0 in PKL, HX=HXD[0]),
          dict(HW=128, AW=4, RW=132, EL=128, GEL=68, PK=1 in PKL, HX=HXD[1]),
          dict(HW=128, AW=4, RW=132, EL=128, GEL=68, PK=2 in PKL, HX=HXD[2]),
          dict(HW=32, AW=1, RW=33, EL=ROW4, GEL=34, PK=False, HX=32)]
    for l_ in (1, 2):
        if not LP[l_]["PK"]:
            LP[l_].update(EL=ROW, GEL=ROW)

    nc = bacc.Bacc(num_swdge_queues=2)
    SIM1 = dims.get("sim1", False)

    # ---- params
    pr = {}
    for nm, shp, dt in [("SRC16", [P, C * 8], I16), ("DSTR", [P, C], F32),
                        ("EAT", [5, C * CHUNK], BF16), ("XT", [8, NW * P], F32),
                        ("BATCH", [P, NW], F32), ("W1", [8, 132], F32),
                        ("WL2", [128, 132], F32), ("WL3", [128, 132], F32),
                        ("WL4", [128, 33], F32), ("W5X10", [5, 10], F32),
                        ("BOUT", [4, 128], F32),
                        ("WD", [48, 32], F32), ("BD", [32, 1], F32),
                        ("WLIN", [64, 1], F32), ("DESCT", [48, Gn], F32),
                        ("CNTR", [1, Gn], F32), ("TG0", [N, 128], BF16)]:
        pr[nm] = nc.declare_dram_parameter(nm, shp, dt, isOutput=False)
    out_p = nc.declare_dram_parameter("out", [1, Gn], F32, isOutput=True)
    pr_TG0_ph = pr["TG0"]

    # ---- internal DRAM
    T_loc = [None] + [nc.dram_tensor(f"T_loc{l}", [NPC, LP[l]["EL"]], BF16)
                      for l in range(1, 4)]
    T_glob = [pr_TG0_ph] + [nc.dram_tensor(f"T_glob{l}", [N, LP[l]["EL"]], BF16,
                                           addr_space="Shared")
                            for l in range(1, 4)]
    ar_in = nc.dram_tensor("ar_in", [32, Gn], F32)
    ar_out = nc.dram_tensor("ar_out", [32, Gn], F32, addr_space="Shared")

    # bin/window bookkeeping (compile-time)
    bin_of_chunk = []
    for b in range(NBINS):
        bin_of_chunk += [b] * int(cpb[b])
    BPW = P // BIN  # bins per window
    win_of_bin = [b // BPW for b in range(NBINS)]
    last_chunk_of_bin = {}
    first_chunk_of_bin = {}
    for c_i, b in enumerate(bin_of_chunk):
        last_chunk_of_bin[b] = c_i
        first_chunk_of_bin.setdefault(b, c_i)

    with tile.TileContext(nc) as tc, ExitStack() as ctx:
        cp = ctx.enter_context(tc.tile_pool(name="const", bufs=1))
        wp = ctx.enter_context(tc.tile_pool(name="work", bufs=2))
        vp = ctx.enter_context(tc.tile_pool(name="win", bufs=2))
        pp = ctx.enter_context(tc.tile_pool(name="psum", bufs=2, space="PSUM"))
        bp = ctx.enter_context(tc.tile_pool(name="binp", bufs=2, space="PSUM"))

        sync, gps, vec, act, pe = nc.sync, nc.gpsimd, nc.vector, nc.scalar, nc.tensor

        def dma_gather_short(out_ap, in_ap, idxs_ap, num_idxs, elem_size,
                             elem_step, queue_num):
            from concourse.bass import exact_div
            eng = gps
            _in_ap = eng.lower_ap_dma(in_ap, for_custom_bir_dma=True)
            _idxs_ap = eng.lower_ap(idxs_ap)
            _out_ap = eng.lower_ap(out_ap)
            stride_bytes_256 = exact_div(elem_step * 2, 256)
            return eng.add_instruction(
                mybir.InstDMAGatherAnt(
                    name=eng.bass.get_next_instruction_name(),
                    ins=[*_in_ap, _idxs_ap,
                         eng.lower_val_access(eng.to_reg(num_idxs))],
                    outs=[_out_ap],
                    transpose=False, num_idxs=num_idxs, elem_size=elem_size,
                    stride_bytes_256=stride_bytes_256, gen_mode=0,
                    single_packet=False, queue_num=queue_num,
                    sbuf_tokens_per_rank=0, sbuf_free_dim_per_rank=0,
                    sbuf_free_dim_pad_per_rank=0, sbuf_byte_offset=0))
        ZTPB = dims.get("ztpb", 1)

        # ---- resident tiles
        src16 = cp.tile([P, C * 8], I16)
        sync.dma_start(out=src16[:], in_=pr["SRC16"][:, :])
        dstr = cp.tile([P, C], BF16)
        gps.dma_start(out=dstr[:], in_=pr["DSTR"][:, :])   # f32 -> bf16 cast
        batcht = cp.tile([P, NW], F32)
        sync.dma_start(out=batcht[:], in_=pr["BATCH"][:, :])
        xT_sb = cp.tile([8, NW * P], BF16)
        gps.dma_start(out=xT_sb[:], in_=pr["XT"][:, :])

        iota_i = cp.tile([P, BIN], I32)
        gps.iota(iota_i[:], pattern=[[1, BIN]], base=0, channel_multiplier=0)
        iotab = cp.tile([P, BIN], BF16)
        vec.tensor_copy(iotab[:], iota_i[:])
        iotag_i = cp.tile([P, Gn], I32)
        gps.iota(iotag_i[:], pattern=[[1, Gn]], base=0, channel_multiplier=0)
        iotagf = cp.tile([P, Gn], F32)
        vec.tensor_copy(iotagf[:], iotag_i[:])
        identf = cp.tile([P, P], F32)
        make_identity(nc, identf[:])

        w1_sb = cp.tile([8, 132], BF16)
        gps.dma_start(out=w1_sb[:], in_=pr["W1"][:, :])
        wl_sb = [None,
                 cp.tile([128, 132], BF16, name="wl2", tag="wl2"),
                 cp.tile([128, 132], BF16, name="wl3", tag="wl3"),
                 cp.tile([128, 33], BF16, name="wl4", tag="wl4")]
        gps.dma_start(out=wl_sb[1][:], in_=pr["WL2"][:, :])   # gpsimd casts f32->bf16
        gps.dma_start(out=wl_sb[2][:], in_=pr["WL3"][:, :])
        gps.dma_start(out=wl_sb[3][:], in_=pr["WL4"][:, :])
        w5x10 = cp.tile([5, 10], BF16)
        gps.dma_start(out=w5x10[:], in_=pr["W5X10"][:, :])
        bout_t = []
        for l in range(4):
            t3 = cp.tile([P, 128], F32, tag=f"bout{l}")
            sync.dma_start(out=t3[:], in_=pr["BOUT"][l:l + 1, :].to_broadcast([P, 128]))
            bout_t.append(t3)

        etc = cp.tile([P, C, 10], BF16)      # eterm9 | cnt  per edge
        pt_all = cp.tile([P, C, BIN], BF16)  # one-hot dst rows per edge
        loop_sb = cp.tile([P, NW, 10], F32)
        gsp = ctx.enter_context(tc.tile_pool(name="gsp", bufs=1, space="PSUM"))
        eap = ctx.enter_context(tc.tile_pool(name="eap", bufs=1))
        gsum_ps = None
        n_pool_mm = [0]

        # ---- readout head start: descriptor branch is input-independent
        comb = cp.tile([64, Gn], F32)
        wd_sb = cp.tile([48, 32], F32)
        sync.dma_start(out=wd_sb[:], in_=pr["WD"][:, :])
        desct_sb = cp.tile([48, Gn], F32)
        sync.dma_start(out=desct_sb[:], in_=pr["DESCT"][:, :])
        bd_sb = cp.tile([32, 1], F32)
        sync.dma_start(out=bd_sb[:], in_=pr["BD"][:, :])
        dps = pp.tile([32, Gn], F32, tag="hps", bufs=4 - ZTPB)
        pe.matmul(out=dps[:], lhsT=wd_sb[:], rhs=desct_sb[:], start=True, stop=True)
        act.activation(out=comb[32:64, :], in_=dps[:], func=AF.Relu, bias=bd_sb[:])
        wlin_sb = cp.tile([64, 1], F32)
        sync.dma_start(out=wlin_sb[:], in_=pr["WLIN"][:, :])
        cntrb = cp.tile([32, Gn], F32)
        sync.dma_start(out=cntrb[:], in_=pr["CNTR"][0:1, :].to_broadcast([32, Gn]))

        WG = dims.get("wg", 5)  # max windows per epilogue group
        # non-uniform groups: taper toward the end so the serial layer-boundary
        # tail (last epilogue -> node phase -> AllGather) shrinks
        grp_bounds = []
        w0_ = 0
        while NW - w0_ > 10:
            grp_bounds.append((w0_, WG))
            w0_ += WG
        for t_ in dims.get("taper", (4, 3, 2, 1)):
            if NW - w0_ > t_:
                grp_bounds.append((w0_, t_))
                w0_ += t_
        if NW > w0_:
            grp_bounds.append((w0_, NW - w0_))
        NG = len(grp_bounds)
        grp_of_win = {}
        for gi, (gw0, gsz_) in enumerate(grp_bounds):
            for w_ in range(gw0, gw0 + gsz_):
                grp_of_win[w_] = gi
        last_chunk_of_grp = {}
        for b in range(NBINS):
            g_ = grp_of_win[win_of_bin[b]]
            last_chunk_of_grp[g_] = max(last_chunk_of_grp.get(g_, -1),
                                        last_chunk_of_bin[b])

        # T_sb tables: [P, NW, 132] (h | b); layer l+1's is built during
        # layer l's edge phase, group by group.
        def node_phase_group(l, g_, T_next, z_src):
            """Build T_next rows for group g_ of layer l (0-based), write T_loc."""
            w0, gsz = grp_bounds[g_]
            HWn = LP[l]["HW"]
            BW = HWn + LP[l]["AW"]  # table row width
            for w_ in range(w0, w0 + gsz):
                if l == 0:
                    hps = pp.tile([P, 132], F32, tag="hps", bufs=4 - ZTPB)
                    pe.matmul(out=hps[:, 0:BW], lhsT=xT_sb[:, w_ * P:(w_ + 1) * P],
                              rhs=w1_sb[:], start=True, stop=True)
                else:
                    ztp = pp.tile([P, P], F32, tag="ztp", bufs=ZTPB)
                    pe.transpose(out=ztp[:], in_=z_src[:, w_ - w0, 0:128],
                                 identity=identf[:])
                    zt_sb = wp.tile([P, P], BF16, tag="ztsb")
                    act.copy(out=zt_sb[:], in_=ztp[:])
                    hps = pp.tile([P, 132], F32, tag="hps", bufs=4 - ZTPB)
                    pe.matmul(out=hps[:, 0:BW], lhsT=zt_sb[:], rhs=wl_sb[l][:],
                              start=True, stop=True)
                act.copy(out=T_next[:, w_, 0:BW], in_=hps[:, 0:BW])
                if l > 0 and LP[l]["PK"]:
                    act.copy(out=Tpk[:, w_, 0:4], in_=hps[:, 128:132])
                    act.copy(out=Tpk[:, w_, 4:68].bitcast(FP8), in_=hps[:, 0:128])
            if l == 0:
                return  # layer-1 table ships as the TG0 param; SBUF copy only
            stage, SW = (Tpk, 68) if LP[l]["PK"] else (T_next, BW)
            # batched table write: full windows in one DMA, ragged tail apart
            wfull = gsz - (1 if (w0 + gsz) * P > NPC else 0)
            if wfull > 0:
                sync.dma_start(
                    out=T_loc[l][w0 * P:(w0 + wfull) * P, 0:SW]
                        .rearrange("(w p) e -> p w e", p=P),
                    in_=stage[:, w0:w0 + wfull, 0:SW])
            if wfull < gsz:
                w_ = w0 + wfull
                nr = NPC - w_ * P
                sync.dma_start(out=T_loc[l][w_ * P:w_ * P + nr, 0:SW],
                               in_=stage[0:nr, w_, 0:SW])

        PT_AHEAD = dims.get("pt_ahead", 64)
        # prebuild the one-hot cache for the first chunks while the layer-0
        # node phase occupies PE/Act
        for g in range(0, PT_AHEAD, 8):
            vec.tensor_tensor(
                out=pt_all[:, g:g + 8, :],
                in0=dstr[:, g:g + 8].unsqueeze(2).to_broadcast([P, 8, BIN]),
                in1=iotab[:].unsqueeze(1).to_broadcast([P, 8, BIN]),
                op=ALU.is_equal)

        # ---- layer 0 node phase (all groups up front)
        T_sb = wp.tile([P, NW, 132], BF16, tag="tsb")
        for g_ in range(NG):
            node_phase_group(0, g_, T_sb, None)

        for l in range(4):
            HW, AW, RW, EL, GEL, PK, HX = (
                LP[l][k] for k in ("HW", "AW", "RW", "EL", "GEL", "PK", "HX"))
            BW = HW + AW

            T_next = None
            if l < 3:
                T_next = wp.tile([P, NW, 132], BF16, name="tnext", tag="tsb")
                if LP[l + 1]["PK"]:
                    Tpk = wp.tile([P, NW, 68], BF16, name="tpk", tag="tpk", bufs=1)

            grp_tiles = {}
            grp_done = set()

            def open_group(g_):
                t = vp.tile([P, WG, 142], F32, name="wingrp", tag="wingrp")
                grp_tiles[g_] = t
                return t

            def epilogue_group(g_):
                w0, gsz = grp_bounds[g_]
                wg = grp_tiles[g_]
                scr = wp.tile([P, WG, 12], F32, name="scr", tag="scr")
                # self-loop alpha (= b_own [+ eterm means]) -> exp
                if l > 0:
                    sl = [None, (0, 4), (4, 8), (8, 9)][l]
                    vec.tensor_tensor(out=scr[:, 0:gsz, 0:AW],
                                      in0=T_sb[:, w0:w0 + gsz, HW:HW + AW],
                                      in1=loop_sb[:, w0:w0 + gsz, sl[0]:sl[1]],
                                      op=ALU.add)
                else:
                    act.copy(out=scr[:, 0:gsz, 0:AW],
                             in_=T_sb[:, w0:w0 + gsz, HW:HW + AW])
                vec.tensor_scalar_mul(out=scr[:, 0:gsz, 4:4 + AW],
                                      in0=scr[:, 0:gsz, 0:AW], scalar1=0.2)
                vec.tensor_tensor(out=scr[:, 0:gsz, 0:AW], in0=scr[:, 0:gsz, 0:AW],
                                  in1=scr[:, 0:gsz, 4:4 + AW], op=ALU.max)
                act.activation(out=scr[:, 0:gsz, 0:AW], in_=scr[:, 0:gsz, 0:AW],
                               func=AF.Exp)
                # num += h_own * ex_loop
                nt = wp.tile([P, WG, 128], BF16, name="nt", tag="nt")
                vec.tensor_tensor(
                    out=nt[:, 0:gsz, 0:HW].rearrange("p g (c a) -> p g c a", a=AW),
                    in0=T_sb[:, w0:w0 + gsz, 0:HW].rearrange("p g (c a) -> p g c a", a=AW),
                    in1=scr[:, 0:gsz, 0:AW].unsqueeze(2)
                        .to_broadcast([P, gsz, HW // AW, AW]),
                    op=ALU.mult)
                vec.tensor_tensor(out=wg[:, 0:gsz, 0:HW], in0=wg[:, 0:gsz, 0:HW],
                                  in1=nt[:, 0:gsz, 0:HW], op=ALU.add)
                # den -> reciprocal
                vec.tensor_tensor(out=scr[:, 0:gsz, 4:4 + AW],
                                  in0=wg[:, 0:gsz, HW:HW + AW],
                                  in1=scr[:, 0:gsz, 0:AW], op=ALU.add)
                vec.tensor_scalar_add(out=scr[:, 0:gsz, 4:4 + AW],
                                      in0=scr[:, 0:gsz, 4:4 + AW], scalar1=1e-16)
                vec.reciprocal(out=scr[:, 0:gsz, 4:4 + AW], in_=scr[:, 0:gsz, 4:4 + AW])
                if l == 0:
                    vec.tensor_scalar_max(out=scr[:, 0:gsz, 8:9],
                                          in0=wg[:, 0:gsz, 141:142], scalar1=1.0)
                    vec.reciprocal(out=scr[:, 0:gsz, 8:9], in_=scr[:, 0:gsz, 8:9])
                    vec.tensor_tensor(
                        out=loop_sb[:, w0:w0 + gsz, 0:9], in0=wg[:, 0:gsz, 132:141],
                        in1=scr[:, 0:gsz, 8:9].to_broadcast([P, gsz, 9]), op=ALU.mult)
                # z = num * recip(den) + bias [+ relu]
                vec.tensor_tensor(
                    out=wg[:, 0:gsz, 0:HW].rearrange("p g (c a) -> p g c a", a=AW),
                    in0=wg[:, 0:gsz, 0:HW].rearrange("p g (c a) -> p g c a", a=AW),
                    in1=scr[:, 0:gsz, 4:4 + AW].unsqueeze(2)
                        .to_broadcast([P, gsz, HW // AW, AW]),
                    op=ALU.mult)
                vec.tensor_tensor(
                    out=wg[:, 0:gsz, 0:HW], in0=wg[:, 0:gsz, 0:HW],
                    in1=bout_t[l][:, 0:HW].unsqueeze(1).to_broadcast([P, gsz, HW]),
                    op=ALU.add)
                if l < 3:
                    act.activation(out=wg[:, 0:gsz, 0:128], in_=wg[:, 0:gsz, 0:128],
                                   func=AF.Relu)
                    node_phase_group(l + 1, g_, T_next, wg)
                else:
                    nonlocal gsum_ps
                    pool_sb = wp.tile([P, WG, 32], BF16, name="pool_sb", tag="poolsb")
                    act.copy(out=pool_sb[:, 0:gsz, 0:32], in_=wg[:, 0:gsz, 0:32])
                    bt = wp.tile([P, WG, Gn], BF16, name="bt", tag="bt", bufs=1)
                    vec.tensor_tensor(
                        out=bt[:, 0:gsz, :],
                        in0=batcht[:, w0:w0 + gsz].unsqueeze(2).to_broadcast([P, gsz, Gn]),
                        in1=iotagf[:].unsqueeze(1).to_broadcast([P, gsz, Gn]),
                        op=ALU.is_equal)
                    if gsum_ps is None:
                        gsum_ps = gsp.tile([32, Gn], F32, name="gsum_ps")
                    for j_ in range(gsz):
                        n_pool_mm[0] += 1
                        pe.matmul(out=gsum_ps[:], lhsT=pool_sb[:, j_, :],
                                  rhs=bt[:, j_, :],
                                  start=(n_pool_mm[0] == 1),
                                  stop=(n_pool_mm[0] == NW))
                grp_done.add(g_)

            cur_bin_tile = {}
            SSL = dims.get("ss4", 64) if l == 3 else SS
            ss_plan = []
            rem_ = C
            while rem_ > 0:
                n_ = min(SSL, rem_)
                ss_plan.append(n_)
                rem_ -= n_
            for t_ in dims.get("ss_tail", (8,)):
                if ss_plan[-1] > t_:
                    ss_plan[-1] -= t_
                    ss_plan.append(t_)
            s0 = 0
            GW = GEL if PK else max(GEL, RW)
            for ss, NCH in enumerate(ss_plan):
                Gt = wp.tile([P, SSL, GW], BF16, tag="gt" if GW > 68 else "gtp", bufs=4)
                if GEL == EL:
                    gps.dma_gather(
                        out_ap=Gt[:, 0:NCH, 0:GEL], in_ap=T_glob[l][:, :],
                        idxs_ap=src16[:, s0 * 8:(s0 + NCH) * 8],
                        num_idxs=NCH * CHUNK, num_idxs_reg=NCH * CHUNK,
                        elem_size=EL, single_packet=False, queue_num=ss % 2)
                else:
                    dma_gather_short(
                        out_ap=Gt[:, 0:NCH, 0:GEL], in_ap=T_glob[l][:, 0:GEL],
                        idxs_ap=src16[:, s0 * 8:(s0 + NCH) * 8],
                        num_idxs=NCH * CHUNK, elem_size=GEL, elem_step=EL,
                        queue_num=ss % 2)
                if l == 0:
                    # edge-term + mask precompute (feeds rhs cols 132:142 +
                    # later layers' alpha); mask folded into EAT row 5.
                    eaT_sl = eap.tile([5, SS * CHUNK], BF16, name="easl", tag="eat")
                    sync.dma_start(
                        out=eaT_sl[:, 0:NCH * CHUNK],
                        in_=pr["EAT"][:, s0 * CHUNK:(s0 + NCH) * CHUNK])
                    for q0 in range(0, NCH, 16):
                        qn = min(16, NCH - q0)
                        etp = pp.tile([P, 160], F32, tag="etp", bufs=1)
                        for j in range(qn):
                            ci = q0 + j
                            pe.matmul(out=etp[:, j * 10:(j + 1) * 10],
                                      lhsT=eaT_sl[:, ci * CHUNK:(ci + 1) * CHUNK],
                                      rhs=w5x10[:], start=True, stop=True)
                        act.copy(out=etc[:, s0 + q0:s0 + q0 + qn, :]
                                 .rearrange("p a b -> p (a b)"),
                                 in_=etp[:, 0:qn * 10])
                    # staircase one-hots built once, reused by all layers;
                    # built PT_AHEAD chunks ahead so the DVE cost sits in the
                    # pipeline's slack instead of its critical phase
                    pb0 = PT_AHEAD + s0
                    pb1 = min(pb0 + NCH, C)
                    for g in range(pb0, pb1, 8):
                        gn = min(8, pb1 - g)
                        vec.tensor_tensor(
                            out=pt_all[:, g:g + gn, :],
                            in0=dstr[:, g:g + gn].unsqueeze(2).to_broadcast([P, gn, BIN]),
                            in1=iotab[:].unsqueeze(1).to_broadcast([P, gn, BIN]),
                            op=ALU.is_equal)
                # alpha = b[src] (+ eterm) -> leaky relu -> exp
                AT = wp.tile([P, SSL, 8], BF16, tag="at", bufs=2)
                if PK:
                    SCT = wp.tile([P, SS, 142], BF16, tag="rhs", bufs=3)
                    BS = 0            # b slot in the packed gathered row
                else:
                    SCT = Gt
                    BS = HW
                if l > 0:
                    sl = [None, (0, 4), (4, 8), (8, 9)][l]
                    vec.tensor_tensor(out=AT[:, 0:NCH, 0:AW],
                                      in0=Gt[:, 0:NCH, BS:BS + AW],
                                      in1=etc[:, s0:s0 + NCH, sl[0]:sl[1]],
                                      op=ALU.add)
                    vec.tensor_scalar_mul(out=AT[:, 0:NCH, AW:2 * AW],
                                          in0=AT[:, 0:NCH, 0:AW], scalar1=0.2)
                    vec.tensor_tensor(out=AT[:, 0:NCH, 0:AW], in0=AT[:, 0:NCH, 0:AW],
                                      in1=AT[:, 0:NCH, AW:2 * AW], op=ALU.max)
                else:
                    vec.tensor_scalar_mul(out=AT[:, 0:NCH, AW:2 * AW],
                                          in0=Gt[:, 0:NCH, BS:BS + AW], scalar1=0.2)
                    vec.tensor_tensor(out=AT[:, 0:NCH, 0:AW],
                                      in0=Gt[:, 0:NCH, BS:BS + AW],
                                      in1=AT[:, 0:NCH, AW:2 * AW], op=ALU.max)
                act.activation(out=SCT[:, 0:NCH, HW:HW + AW], in_=AT[:, 0:NCH, 0:AW],
                               func=AF.Exp)
                if PK:
                    # h x ex: leading HX columns straight from fp8 on DVE (1x);
                    # the rest cast to bf16 on Act, then multiplied in 2x mode
                    vec.tensor_tensor(
                        out=SCT[:, 0:NCH, 0:HX].rearrange("p s (c a) -> p s c a", a=AW),
                        in0=Gt[:, 0:NCH, 4:4 + HX // 2].bitcast(FP8)
                            .rearrange("p s (c a) -> p s c a", a=AW),
                        in1=SCT[:, 0:NCH, HW:HW + AW].unsqueeze(2)
                            .to_broadcast([P, NCH, HX // AW, AW]),
                        op=ALU.mult)
                    if HX < HW:
                        act.copy(out=SCT[:, 0:NCH, HX:HW],
                                 in_=Gt[:, 0:NCH, 4 + HX // 2:4 + HW // 2].bitcast(FP8))
                        PHX = dims.get("phx", 16) if l in (1, 2) else 0
                        DH = HW - PHX
                        vec.tensor_tensor(
                            out=SCT[:, 0:NCH, HX:DH].rearrange("p s (c a) -> p s c a", a=AW),
                            in0=SCT[:, 0:NCH, HX:DH].rearrange("p s (c a) -> p s c a", a=AW),
                            in1=SCT[:, 0:NCH, HW:HW + AW].unsqueeze(2)
                                .to_broadcast([P, NCH, (DH - HX) // AW, AW]),
                            op=ALU.mult)
                        if PHX:
                            gps.tensor_tensor(
                                out=SCT[:, 0:NCH, DH:HW].rearrange("p s (c a) -> p s c a", a=AW),
                                in0=SCT[:, 0:NCH, DH:HW].rearrange("p s (c a) -> p s c a", a=AW),
                                in1=SCT[:, 0:NCH, HW:HW + AW].unsqueeze(2)
                                    .to_broadcast([P, NCH, PHX // AW, AW]),
                                op=ALU.mult)
                else:
                    vec.tensor_tensor(
                        out=SCT[:, 0:NCH, 0:HW].rearrange("p s (c a) -> p s c a", a=AW),
                        in0=SCT[:, 0:NCH, 0:HW].rearrange("p s (c a) -> p s c a", a=AW),
                        in1=SCT[:, 0:NCH, HW:HW + AW].unsqueeze(2)
                            .to_broadcast([P, NCH, HW // AW, AW]),
                        op=ALU.mult)
                if l == 0:
                    # append eterm9|cnt as rhs cols 132:142
                    act.copy(out=SCT[:, 0:NCH, 132:142],
                             in_=etc[:, s0:s0 + NCH, :])
                # scatter matmuls
                for c_i in range(NCH):
                    gc = s0 + c_i
                    b = bin_of_chunk[gc]
                    w_ = win_of_bin[b]
                    g_ = grp_of_win[w_]
                    if g_ not in grp_tiles:
                        open_group(g_)
                    if gc == first_chunk_of_bin[b]:
                        cur_bin_tile[b] = bp.tile([BIN, 142], F32, name="binacc",
                                                  tag="binacc")
                    pe.matmul(out=cur_bin_tile[b][:, 0:RW],
                              lhsT=pt_all[:, gc, :], rhs=SCT[:, c_i, 0:RW],
                              start=(gc == first_chunk_of_bin[b]),
                              stop=(gc == last_chunk_of_bin[b]))
                    if gc == last_chunk_of_bin[b]:
                        j = b % BPW
                        wrel = w_ - grp_bounds[g_][0]
                        act.copy(out=grp_tiles[g_][BIN * j:BIN * (j + 1), wrel, 0:RW],
                                 in_=cur_bin_tile[b][:, 0:RW])
                        del cur_bin_tile[b]
                    if gc == last_chunk_of_grp.get(g_, None):
                        epilogue_group(g_)
                s0 += NCH
            # groups never triggered (e.g. all-empty windows)
            for g_ in range(NG):
                if g_ not in grp_done:
                    if g_ not in grp_tiles:
                        open_group(g_)
                    epilogue_group(g_)
            if l < 3:
                if SIM1:
                    sync.dma_start(out=T_glob[l + 1][0:NPC, :], in_=T_loc[l + 1][:, :])
                else:
                    gps.collective_compute(
                        "AllGather", ALU.bypass, replica_groups=[list(range(NCORES))],
                        ins=[T_loc[l + 1][:, :]], outs=[T_glob[l + 1][:, :]])
                T_sb = T_next

        # ============ readout
        gsum_sb = cp.tile([32, Gn], F32)
        act.copy(out=gsum_sb[:], in_=gsum_ps[:])
        sync.dma_start(out=ar_in[:], in_=gsum_sb[:])
        if SIM1:
            sync.dma_start(out=ar_out[:], in_=ar_in[:])
        else:
            gps.collective_compute("AllReduce", ALU.add,
                                   replica_groups=[list(range(NCORES))],
                                   ins=[ar_in[:]], outs=[ar_out[:]])
        gs = cp.tile([32, Gn], F32)
        sync.dma_start(out=gs[:], in_=ar_out[:])
        vec.tensor_tensor(out=comb[0:32, :], in0=gs[:, :], in1=cntrb[:],
                          op=ALU.mult)
        blt = cp.tile([1, 1], F32)
        vec.memset(blt[:], bl)
        fin = pp.tile([1, Gn], F32, tag="hps", bufs=4 - ZTPB)
        pe.matmul(out=fin[:], lhsT=wlin_sb[:], rhs=comb[:], start=True, stop=True)
        res_sb = cp.tile([1, Gn], F32)
        act.activation(out=res_sb[:], in_=fin[:], func=AF.Sigmoid, bias=blt[:])
        sync.dma_start(out=out_p[:, :], in_=res_sb[:])

    nc.finalize()
    return nc


# ------------------------------------------------------------------ entry
def _run(inputs, trace=False, debug=False):
    dims, shared, per_core = host_prep(inputs)
    nc = build_program(dims, shared)
    in_maps = [{**shared, **pc} for pc in per_core]
    from concourse.bass_utils import run_bass_kernel_spmd
    return run_bass_kernel_spmd(nc, in_maps, list(range(NCORES)), trace=trace)


def kernel(**inputs):
    res = _run(inputs)
    return res.results[0]["out"].reshape(-1).astype(np.float32)


# revision 39
# speedup vs baseline: 1.0263x; 1.0111x over previous
"""EnhancedGAT Trainium2 Bass kernel (8 NeuronCores, SPMD).

Strategy:
  - Edges are sorted by destination node on the host; core k owns dst nodes
    [k*N/8, (k+1)*N/8) and every edge targeting them. Per-core edge lists are
    bucketed into 64-node bins and padded to 128-edge chunks with a per-bin
    chunk count shared across cores (SPMD uniformity). Dummy (padding) edges
    carry dst-offset 64, which falls outside the 64-wide one-hot used by the
    scatter matmuls, so they contribute exactly nothing.
  - Each GAT layer:
      node phase: every core computes a table row [h(128) | b(4)] for its own
        nodes, where b = per-head <h, att_s + att_d> comes directly out of the
        h matmul via 4 extra weight columns W @ A. Rows live in a [NPC, 256]
        bf16 DRAM table (512B stride for the gather); an AllGather replicates
        it to every core.
      edge phase: per 4096-edge superstep one dma_gather pulls the rows for
        the edges' sources; attention coefficients alpha = b[src] (+ edge
        term) are leaky-relu'd and exp'd in place, messages h*ex are scattered
        into per-bin PSUM accumulators via one-hot matmuls. Softmax is
        unnormalized (max-subtraction skipped; alphas are O(0.3)); the divide
        happens per node at the group epilogue, where self-loop contributions
        are added. As soon as a window-group's epilogue finishes, the NEXT
        layer's node phase for those windows runs (transpose + matmul + table
        write), hiding the layer boundary behind the remaining gathers.
  - Layer 1 additionally computes, per edge, the folded edge-attention terms
    for layers 2-4 (eterm = ea @ V + be, with the padding mask folded in as a
    fifth all-ones/zeros EAT row) plus the per-edge mask into an [C,10] SBUF
    cache, and accumulates per-node mean edge-feature terms and in-degrees
    (extra scatter-matmul columns) used by the self-loops of layers 2-4.
  - Final graph mean-pool via one-hot matmuls into a [33, G] accumulator,
    AllReduce across cores, tiny dense readout replicated on every core.
"""
import sys
import numpy as np

sys.path.insert(0, "/opt/trn_rl_repo")

HID = 32
NCORES = 8
P = 128
BIN = 64
SS = 32          # chunks per superstep
CHUNK = 128
ROW = 256        # table row elements (bf16) for layers 1-3 (512B stride)
ROW4 = 128       # layer-4 table row elements


# ----------------------------------------------------------------- host prep
def host_prep(inputs):
    x = np.asarray(inputs["x"], np.float32)
    ei = np.asarray(inputs["edge_index"]).astype(np.int64)
    ea = np.asarray(inputs["edge_attr"], np.float32)
    batch = np.asarray(inputs["batch"]).astype(np.int64)
    desc = np.asarray(inputs["descriptors"], np.float32)

    N = x.shape[0]
    E = ei.shape[1]
    Gn = desc.shape[0]
    NPC = N // NCORES
    NW = -(-NPC // P)
    NBINS = -(-NPC // BIN)

    src_all, dst_all = ei[0], ei[1]
    order = np.argsort(dst_all, kind="stable")
    src_s, dst_s = src_all[order], dst_all[order]
    ea_s = ea[order]
    core_of = dst_s // NPC
    local = dst_s - core_of * NPC
    bin_of = local // BIN

    cnt = np.zeros((NCORES, NBINS), np.int64)
    np.add.at(cnt, (core_of, bin_of), 1)
    cpb = np.max(-(-cnt // CHUNK), axis=0)          # chunks per bin (shared)
    cpb = np.maximum(cpb, 1)                        # every bin gets a chunk
    C_total = int(cpb.sum())
    off = np.zeros(NBINS, np.int64)
    off[1:] = np.cumsum(cpb)[:-1]
    EP = C_total * CHUNK                            # padded edges per core

    per_core = []
    for k in range(NCORES):
        srck = np.zeros(EP, np.int64)
        dstrk = np.full(EP, float(BIN), np.float32)  # dummies -> dead one-hot
        maskk = np.zeros(EP, np.float32)
        eak = np.zeros((EP, 4), np.float32)
        sel = core_of == k
        bins_k = bin_of[sel]
        start = np.searchsorted(bins_k, np.arange(NBINS))
        pos = np.arange(bins_k.size) - start[bins_k]
        slot = off[bins_k] * CHUNK + pos
        srck[slot] = src_s[sel]
        dstrk[slot] = (local[sel] - bins_k * BIN).astype(np.float32)
        maskk[slot] = 1.0
        eak[slot] = ea_s[sel]

        # device layouts: edge e = c*128 + p
        src16 = np.tile(srck.reshape(-1, 16).T.astype(np.int16), (8, 1))
        dstr_d = dstrk.reshape(C_total, P).T.copy()
        import ml_dtypes
        ea5 = np.concatenate([eak.T, maskk[None, :]], axis=0).astype(ml_dtypes.bfloat16)

        xk = x[k * NPC:(k + 1) * NPC]
        xT = np.zeros((8, NW * P), np.float32)
        xT[:, :NPC] = xk.T
        bk = np.full(NW * P, Gn + 5, np.float32)
        bk[:NPC] = batch[k * NPC:(k + 1) * NPC].astype(np.float32)
        batch_d = bk.reshape(NW, P).T.copy()

        per_core.append(dict(SRC16=src16, DSTR=dstr_d, EAT=ea5,
                             XT=xT, BATCH=batch_d))

    # ---- weight folding
    w = {k: np.asarray(v, np.float32) for k, v in inputs.items()
         if k not in ("x", "edge_index", "edge_attr", "batch", "descriptors")}

    def vfold(We, ae, heads):
        Vp = (We.reshape(w["We_enc"].shape[1], heads, HID) * ae[None]).sum(-1)
        return w["We_enc"] @ Vp, w["be_enc"] @ Vp      # [4,heads],[heads]

    V2, bv2 = vfold(w["We2"], w["ae2"], 4)
    V3, bv3 = vfold(w["We3"], w["ae3"], 4)
    V4, bv4 = vfold(w["We4"], w["ae4"], 1)
    # [5,10]: rows = 4 edge-attr dims + mask; cols = 9 eterms + cnt
    W5x10 = np.zeros((5, 10), np.float32)
    W5x10[0:4, 0:9] = np.concatenate([V2, V3, V4], axis=1)
    W5x10[4, 0:9] = np.concatenate([bv2, bv3, bv4])
    W5x10[4, 9] = 1.0

    def padr(v, n):
        o = np.zeros(n, np.float32)
        o[: v.size] = v
        return o

    # channel-major reorder of the 128-wide (4 heads x 32 ch) dimension:
    # new position c*4+a holds old a*32+c. Keeps per-head broadcasts
    # innermost-packed on DVE (2x mode).
    cm = (np.arange(128) % 4) * 32 + np.arange(128) // 4

    def wext(W, att_s, att_d, heads):
        # append per-head b-columns: b_a = h . (att_s+att_d)_a
        att = (att_s + att_d).reshape(-1)  # [heads*HID] head-major
        if heads == 4:
            attc = att[cm]                 # channel-major to match W cols
            A = np.zeros((128, 4), np.float32)
            A[np.arange(128), np.arange(128) % 4] = attc
        else:
            A = att[:, None]               # [32,1]
        return np.concatenate([W, W @ A], axis=1)

    W1e = wext(w["W1"][:, cm], w["as1"], w["ad1"], 4)            # [8,132]
    W2e = wext(w["W2"][cm][:, cm], w["as2"], w["ad2"], 4)        # [128,132]
    W3e = wext(w["W3"][cm][:, cm], w["as3"], w["ad3"], 4)
    W4e = wext(w["W4"][cm], w["as4"], w["ad4"], 1)               # [128,33]

    bout = np.stack([padr(w["b1"][cm], 128), padr(w["b2"][cm], 128),
                     padr(w["b3"][cm], 128), padr(w["b4"], 128)])

    import ml_dtypes
    T0h = (x @ W1e).astype(np.float32)
    pk0 = np.zeros((N, 256), np.uint8)
    pk0[:, 0:8] = T0h[:, 128:132].astype(ml_dtypes.bfloat16).view(np.uint8)
    pk0[:, 8:136] = T0h[:, 0:128].astype(ml_dtypes.float8_e4m3).view(np.uint8)
    TG0 = pk0.view(ml_dtypes.bfloat16)

    gcnt = np.bincount(batch, minlength=Gn).astype(np.float32)
    cntr = (1.0 / np.maximum(gcnt, 1.0))[None, :]           # [1, Gn]
    shared = dict(
        W1=W1e, WL2=W2e, WL3=W3e, WL4=W4e, TG0=TG0,
        W5X10=W5x10, BOUT=bout, CNTR=cntr,
        WD=w["Wd"], BD=w["bd"][:, None], WLIN=w["Wl"], DESCT=desc.T.copy(),
    )
    bl = float(np.asarray(w["bl"]).reshape(-1)[0])

    dims = dict(N=N, E=E, Gn=Gn, NPC=NPC, NW=NW, NBINS=NBINS,
                C=C_total, cpb=cpb, off=off, bl=bl)
    return dims, shared, per_core


# ------------------------------------------------------------- program build
def build_program(dims, shared):
    import concourse.bass as bass
    import concourse.mybir as mybir
    import concourse.tile as tile
    import concourse.bacc as bacc
    from concourse.masks import make_identity
    from contextlib import ExitStack

    F32 = mybir.dt.float32
    FP8 = mybir.dt.float8e4
    BF16 = mybir.dt.bfloat16
    I32 = mybir.dt.int32
    I16 = mybir.dt.int16
    AF = mybir.ActivationFunctionType
    ALU = mybir.AluOpType
    AX = mybir.AxisListType

    N, Gn, NPC, NW, NBINS, C = (dims[k] for k in ("N", "Gn", "NPC", "NW", "NBINS", "C"))
    cpb, off, bl = dims["cpb"], dims["off"], dims["bl"]
    NSS = C // SS
    # layer params: h width, heads, rhs width, gather row elems
    # PK tables pack rows as [b bf16 x4 | h fp8 x128] (136B) in a 256B stride;
    # HX = leading h-columns multiplied on DVE straight from fp8 (1x mode), the
    # rest is cast to bf16 on Act first so the DVE part runs in 2x mode.
    HXD = dims.get("hx", {0: 96, 1: 48, 2: 48})
    PKL = dims.get("pk_layers", (0, 1, 2))
    LP = [dict(HW=128, AW=4, RW=142, EL=128 if 0 in PKL else ROW,
               GEL=68 if 0 in PKL else 132, PK=0 in PKL, HX=HXD[0]),
          dict(HW=128, AW=4, RW=132, EL=128, GEL=68, PK=1 in PKL, HX=HXD[1]),
          dict(HW=128, AW=4, RW=132, EL=128, GEL=68, PK=2 in PKL, HX=HXD[2]),
          dict(HW=32, AW=1, RW=33, EL=ROW4, GEL=34, PK=False, HX=32)]
    for l_ in (1, 2):
        if not LP[l_]["PK"]:
            LP[l_].update(EL=ROW, GEL=ROW)

    nc = bacc.Bacc(num_swdge_queues=2)
    SIM1 = dims.get("sim1", False)

    # ---- params
    pr = {}
    for nm, shp, dt in [("SRC16", [P, C * 8], I16), ("DSTR", [P, C], F32),
                        ("EAT", [5, C * CHUNK], BF16), ("XT", [8, NW * P], F32),
                        ("BATCH", [P, NW], F32), ("W1", [8, 132], F32),
                        ("WL2", [128, 132], F32), ("WL3", [128, 132], F32),
                        ("WL4", [128, 33], F32), ("W5X10", [5, 10], F32),
                        ("BOUT", [4, 128], F32),
                        ("WD", [48, 32], F32), ("BD", [32, 1], F32),
                        ("WLIN", [64, 1], F32), ("DESCT", [48, Gn], F32),
                        ("CNTR", [1, Gn], F32), ("TG0", [N, 128], BF16)]:
        pr[nm] = nc.declare_dram_parameter(nm, shp, dt, isOutput=False)
    out_p = nc.declare_dram_parameter("out", [1, Gn], F32, isOutput=True)
    pr_TG0_ph = pr["TG0"]

    # ---- internal DRAM
    T_loc = [None] + [nc.dram_tensor(f"T_loc{l}", [NPC, LP[l]["EL"]], BF16)
                      for l in range(1, 4)]
    T_glob = [pr_TG0_ph] + [nc.dram_tensor(f"T_glob{l}", [N, LP[l]["EL"]], BF16,
                                           addr_space="Shared")
                            for l in range(1, 4)]
    ar_in = nc.dram_tensor("ar_in", [32, Gn], F32)
    ar_out = nc.dram_tensor("ar_out", [32, Gn], F32, addr_space="Shared")

    # bin/window bookkeeping (compile-time)
    bin_of_chunk = []
    for b in range(NBINS):
        bin_of_chunk += [b] * int(cpb[b])
    BPW = P // BIN  # bins per window
    win_of_bin = [b // BPW for b in range(NBINS)]
    last_chunk_of_bin = {}
    first_chunk_of_bin = {}
    for c_i, b in enumerate(bin_of_chunk):
        last_chunk_of_bin[b] = c_i
        first_chunk_of_bin.setdefault(b, c_i)

    with tile.TileContext(nc) as tc, ExitStack() as ctx:
        cp = ctx.enter_context(tc.tile_pool(name="const", bufs=1))
        wp = ctx.enter_context(tc.tile_pool(name="work", bufs=2))
        vp = ctx.enter_context(tc.tile_pool(name="win", bufs=2))
        pp = ctx.enter_context(tc.tile_pool(name="psum", bufs=2, space="PSUM"))
        bp = ctx.enter_context(tc.tile_pool(name="binp", bufs=2, space="PSUM"))

        sync, gps, vec, act, pe = nc.sync, nc.gpsimd, nc.vector, nc.scalar, nc.tensor

        def dma_gather_short(out_ap, in_ap, idxs_ap, num_idxs, elem_size,
                             elem_step, queue_num):
            from concourse.bass import exact_div
            eng = gps
            _in_ap = eng.lower_ap_dma(in_ap, for_custom_bir_dma=True)
            _idxs_ap = eng.lower_ap(idxs_ap)
            _out_ap = eng.lower_ap(out_ap)
            stride_bytes_256 = exact_div(elem_step * 2, 256)
            return eng.add_instruction(
                mybir.InstDMAGatherAnt(
                    name=eng.bass.get_next_instruction_name(),
                    ins=[*_in_ap, _idxs_ap,
                         eng.lower_val_access(eng.to_reg(num_idxs))],
                    outs=[_out_ap],
                    transpose=False, num_idxs=num_idxs, elem_size=elem_size,
                    stride_bytes_256=stride_bytes_256, gen_mode=0,
                    single_packet=False, queue_num=queue_num,
                    sbuf_tokens_per_rank=0, sbuf_free_dim_per_rank=0,
                    sbuf_free_dim_pad_per_rank=0, sbuf_byte_offset=0))
        ZTPB = dims.get("ztpb", 1)

        # ---- resident tiles
        src16 = cp.tile([P, C * 8], I16)
        sync.dma_start(out=src16[:], in_=pr["SRC16"][:, :])
        dstr = cp.tile([P, C], BF16)
        gps.dma_start(out=dstr[:], in_=pr["DSTR"][:, :])   # f32 -> bf16 cast
        batcht = cp.tile([P, NW], F32)
        sync.dma_start(out=batcht[:], in_=pr["BATCH"][:, :])
        xT_sb = cp.tile([8, NW * P], BF16)
        gps.dma_start(out=xT_sb[:], in_=pr["XT"][:, :])

        iota_i = cp.tile([P, BIN], I32)
        gps.iota(iota_i[:], pattern=[[1, BIN]], base=0, channel_multiplier=0)
        iotab = cp.tile([P, BIN], BF16)
        vec.tensor_copy(iotab[:], iota_i[:])
        iotag_i = cp.tile([P, Gn], I32)
        gps.iota(iotag_i[:], pattern=[[1, Gn]], base=0, channel_multiplier=0)
        iotagf = cp.tile([P, Gn], F32)
        vec.tensor_copy(iotagf[:], iotag_i[:])
        identf = cp.tile([P, P], F32)
        make_identity(nc, identf[:])

        w1_sb = cp.tile([8, 132], BF16)
        gps.dma_start(out=w1_sb[:], in_=pr["W1"][:, :])
        wl_sb = [None,
                 cp.tile([128, 132], BF16, name="wl2", tag="wl2"),
                 cp.tile([128, 132], BF16, name="wl3", tag="wl3"),
                 cp.tile([128, 33], BF16, name="wl4", tag="wl4")]
        gps.dma_start(out=wl_sb[1][:], in_=pr["WL2"][:, :])   # gpsimd casts f32->bf16
        gps.dma_start(out=wl_sb[2][:], in_=pr["WL3"][:, :])
        gps.dma_start(out=wl_sb[3][:], in_=pr["WL4"][:, :])
        w5x10 = cp.tile([5, 10], BF16)
        gps.dma_start(out=w5x10[:], in_=pr["W5X10"][:, :])
        bout_t = []
        for l in range(4):
            t3 = cp.tile([P, 128], F32, tag=f"bout{l}")
            sync.dma_start(out=t3[:], in_=pr["BOUT"][l:l + 1, :].to_broadcast([P, 128]))
            bout_t.append(t3)

        etc = cp.tile([P, C, 10], BF16)      # eterm9 | cnt  per edge
        pt_all = cp.tile([P, C, BIN], BF16)  # one-hot dst rows per edge
        loop_sb = cp.tile([P, NW, 10], F32)
        gsp = ctx.enter_context(tc.tile_pool(name="gsp", bufs=1, space="PSUM"))
        eap = ctx.enter_context(tc.tile_pool(name="eap", bufs=1))
        gsum_ps = None
        n_pool_mm = [0]

        # ---- readout head start: descriptor branch is input-independent
        comb = cp.tile([64, Gn], F32)
        wd_sb = cp.tile([48, 32], F32)
        sync.dma_start(out=wd_sb[:], in_=pr["WD"][:, :])
        desct_sb = cp.tile([48, Gn], F32)
        sync.dma_start(out=desct_sb[:], in_=pr["DESCT"][:, :])
        bd_sb = cp.tile([32, 1], F32)
        sync.dma_start(out=bd_sb[:], in_=pr["BD"][:, :])
        dps = pp.tile([32, Gn], F32, tag="hps", bufs=4 - ZTPB)
        pe.matmul(out=dps[:], lhsT=wd_sb[:], rhs=desct_sb[:], start=True, stop=True)
        act.activation(out=comb[32:64, :], in_=dps[:], func=AF.Relu, bias=bd_sb[:])
        wlin_sb = cp.tile([64, 1], F32)
        sync.dma_start(out=wlin_sb[:], in_=pr["WLIN"][:, :])
        cntrb = cp.tile([32, Gn], F32)
        sync.dma_start(out=cntrb[:], in_=pr["CNTR"][0:1, :].to_broadcast([32, Gn]))

        WG = dims.get("wg", 5)  # max windows per epilogue group
        # non-uniform groups: taper toward the end so the serial layer-boundary
        # tail (last epilogue -> node phase -> AllGather) shrinks
        grp_bounds = []
        w0_ = 0
        while NW - w0_ > 10:
            grp_bounds.append((w0_, WG))
            w0_ += WG
        for t_ in dims.get("taper", (4, 3, 2, 1)):
            if NW - w0_ > t_:
                grp_bounds.append((w0_, t_))
                w0_ += t_
        if NW > w0_:
            grp_bounds.append((w0_, NW - w0_))
        NG = len(grp_bounds)
        grp_of_win = {}
        for gi, (gw0, gsz_) in enumerate(grp_bounds):
            for w_ in range(gw0, gw0 + gsz_):
                grp_of_win[w_] = gi
        last_chunk_of_grp = {}
        for b in range(NBINS):
            g_ = grp_of_win[win_of_bin[b]]
            last_chunk_of_grp[g_] = max(last_chunk_of_grp.get(g_, -1),
                                        last_chunk_of_bin[b])

        # T_sb tables: [P, NW, 132] (h | b); layer l+1's is built during
        # layer l's edge phase, group by group.
        def node_phase_group(l, g_, T_next, z_src):
            """Build T_next rows for group g_ of layer l (0-based), write T_loc."""
            w0, gsz = grp_bounds[g_]
            HWn = LP[l]["HW"]
            BW = HWn + LP[l]["AW"]  # table row width
            for w_ in range(w0, w0 + gsz):
                if l == 0:
                    hps = pp.tile([P, 132], F32, tag="hps", bufs=4 - ZTPB)
                    pe.matmul(out=hps[:, 0:BW], lhsT=xT_sb[:, w_ * P:(w_ + 1) * P],
                              rhs=w1_sb[:], start=True, stop=True)
                else:
                    ztp = pp.tile([P, P], F32, tag="ztp", bufs=ZTPB)
                    pe.transpose(out=ztp[:], in_=z_src[:, w_ - w0, 0:128],
                                 identity=identf[:])
                    zt_sb = wp.tile([P, P], BF16, tag="ztsb")
                    act.copy(out=zt_sb[:], in_=ztp[:])
                    hps = pp.tile([P, 132], F32, tag="hps", bufs=4 - ZTPB)
                    pe.matmul(out=hps[:, 0:BW], lhsT=zt_sb[:], rhs=wl_sb[l][:],
                              start=True, stop=True)
                act.copy(out=T_next[:, w_, 0:BW], in_=hps[:, 0:BW])
                if l > 0 and LP[l]["PK"]:
                    act.copy(out=Tpk[:, w_, 0:4], in_=hps[:, 128:132])
                    act.copy(out=Tpk[:, w_, 4:68].bitcast(FP8), in_=hps[:, 0:128])
            if l == 0:
                return  # layer-1 table ships as the TG0 param; SBUF copy only
            stage, SW = (Tpk, 68) if LP[l]["PK"] else (T_next, BW)
            # batched table write: full windows in one DMA, ragged tail apart
            wfull = gsz - (1 if (w0 + gsz) * P > NPC else 0)
            if wfull > 0:
                sync.dma_start(
                    out=T_loc[l][w0 * P:(w0 + wfull) * P, 0:SW]
                        .rearrange("(w p) e -> p w e", p=P),
                    in_=stage[:, w0:w0 + wfull, 0:SW])
            if wfull < gsz:
                w_ = w0 + wfull
                nr = NPC - w_ * P
                sync.dma_start(out=T_loc[l][w_ * P:w_ * P + nr, 0:SW],
                               in_=stage[0:nr, w_, 0:SW])

        PT_AHEAD = dims.get("pt_ahead", 64)
        # prebuild the one-hot cache for the first chunks while the layer-0
        # node phase occupies PE/Act
        for g in range(0, PT_AHEAD, 8):
            vec.tensor_tensor(
                out=pt_all[:, g:g + 8, :],
                in0=dstr[:, g:g + 8].unsqueeze(2).to_broadcast([P, 8, BIN]),
                in1=iotab[:].unsqueeze(1).to_broadcast([P, 8, BIN]),
                op=ALU.is_equal)

        # ---- layer 0 node phase (all groups up front)
        T_sb = wp.tile([P, NW, 132], BF16, tag="tsb")
        for g_ in range(NG):
            node_phase_group(0, g_, T_sb, None)

        for l in range(4):
            HW, AW, RW, EL, GEL, PK, HX = (
                LP[l][k] for k in ("HW", "AW", "RW", "EL", "GEL", "PK", "HX"))
            BW = HW + AW

            T_next = None
            if l < 3:
                T_next = wp.tile([P, NW, 132], BF16, name="tnext", tag="tsb")
                if LP[l + 1]["PK"]:
                    Tpk = wp.tile([P, NW, 68], BF16, name="tpk", tag="tpk", bufs=1)

            grp_tiles = {}
            grp_done = set()

            def open_group(g_):
                t = vp.tile([P, WG, 142], F32, name="wingrp", tag="wingrp")
                grp_tiles[g_] = t
                return t

            def epilogue_group(g_):
                w0, gsz = grp_bounds[g_]
                wg = grp_tiles[g_]
                scr = wp.tile([P, WG, 12], F32, name="scr", tag="scr")
                # self-loop alpha (= b_own [+ eterm means]) -> exp
                if l > 0:
                    sl = [None, (0, 4), (4, 8), (8, 9)][l]
                    vec.tensor_tensor(out=scr[:, 0:gsz, 0:AW],
                                      in0=T_sb[:, w0:w0 + gsz, HW:HW + AW],
                                      in1=loop_sb[:, w0:w0 + gsz, sl[0]:sl[1]],
                                      op=ALU.add)
                else:
                    act.copy(out=scr[:, 0:gsz, 0:AW],
                             in_=T_sb[:, w0:w0 + gsz, HW:HW + AW])
                vec.tensor_scalar_mul(out=scr[:, 0:gsz, 4:4 + AW],
                                      in0=scr[:, 0:gsz, 0:AW], scalar1=0.2)
                vec.tensor_tensor(out=scr[:, 0:gsz, 0:AW], in0=scr[:, 0:gsz, 0:AW],
                                  in1=scr[:, 0:gsz, 4:4 + AW], op=ALU.max)
                act.activation(out=scr[:, 0:gsz, 0:AW], in_=scr[:, 0:gsz, 0:AW],
                               func=AF.Exp)
                # num += h_own * ex_loop
                nt = wp.tile([P, WG, 128], BF16, name="nt", tag="nt")
                vec.tensor_tensor(
                    out=nt[:, 0:gsz, 0:HW].rearrange("p g (c a) -> p g c a", a=AW),
                    in0=T_sb[:, w0:w0 + gsz, 0:HW].rearrange("p g (c a) -> p g c a", a=AW),
                    in1=scr[:, 0:gsz, 0:AW].unsqueeze(2)
                        .to_broadcast([P, gsz, HW // AW, AW]),
                    op=ALU.mult)
                vec.tensor_tensor(out=wg[:, 0:gsz, 0:HW], in0=wg[:, 0:gsz, 0:HW],
                                  in1=nt[:, 0:gsz, 0:HW], op=ALU.add)
                # den -> reciprocal
                vec.tensor_tensor(out=scr[:, 0:gsz, 4:4 + AW],
                                  in0=wg[:, 0:gsz, HW:HW + AW],
                                  in1=scr[:, 0:gsz, 0:AW], op=ALU.add)
                vec.tensor_scalar_add(out=scr[:, 0:gsz, 4:4 + AW],
                                      in0=scr[:, 0:gsz, 4:4 + AW], scalar1=1e-16)
                vec.reciprocal(out=scr[:, 0:gsz, 4:4 + AW], in_=scr[:, 0:gsz, 4:4 + AW])
                if l == 0:
                    vec.tensor_scalar_max(out=scr[:, 0:gsz, 8:9],
                                          in0=wg[:, 0:gsz, 141:142], scalar1=1.0)
                    vec.reciprocal(out=scr[:, 0:gsz, 8:9], in_=scr[:, 0:gsz, 8:9])
                    vec.tensor_tensor(
                        out=loop_sb[:, w0:w0 + gsz, 0:9], in0=wg[:, 0:gsz, 132:141],
                        in1=scr[:, 0:gsz, 8:9].to_broadcast([P, gsz, 9]), op=ALU.mult)
                # z = num * recip(den) + bias [+ relu]
                vec.tensor_tensor(
                    out=wg[:, 0:gsz, 0:HW].rearrange("p g (c a) -> p g c a", a=AW),
                    in0=wg[:, 0:gsz, 0:HW].rearrange("p g (c a) -> p g c a", a=AW),
                    in1=scr[:, 0:gsz, 4:4 + AW].unsqueeze(2)
                        .to_broadcast([P, gsz, HW // AW, AW]),
                    op=ALU.mult)
                vec.tensor_tensor(
                    out=wg[:, 0:gsz, 0:HW], in0=wg[:, 0:gsz, 0:HW],
                    in1=bout_t[l][:, 0:HW].unsqueeze(1).to_broadcast([P, gsz, HW]),
                    op=ALU.add)
                if l < 3:
                    act.activation(out=wg[:, 0:gsz, 0:128], in_=wg[:, 0:gsz, 0:128],
                                   func=AF.Relu)
                    node_phase_group(l + 1, g_, T_next, wg)
                else:
                    nonlocal gsum_ps
                    pool_sb = wp.tile([P, WG, 32], BF16, name="pool_sb", tag="poolsb")
                    act.copy(out=pool_sb[:, 0:gsz, 0:32], in_=wg[:, 0:gsz, 0:32])
                    bt = wp.tile([P, WG, Gn], BF16, name="bt", tag="bt", bufs=1)
                    vec.tensor_tensor(
                        out=bt[:, 0:gsz, :],
                        in0=batcht[:, w0:w0 + gsz].unsqueeze(2).to_broadcast([P, gsz, Gn]),
                        in1=iotagf[:].unsqueeze(1).to_broadcast([P, gsz, Gn]),
                        op=ALU.is_equal)
                    if gsum_ps is None:
                        gsum_ps = gsp.tile([32, Gn], F32, name="gsum_ps")
                    for j_ in range(gsz):
                        n_pool_mm[0] += 1
                        pe.matmul(out=gsum_ps[:], lhsT=pool_sb[:, j_, :],
                                  rhs=bt[:, j_, :],
                                  start=(n_pool_mm[0] == 1),
                                  stop=(n_pool_mm[0] == NW))
                grp_done.add(g_)

            cur_bin_tile = {}
            SSL = dims.get("ss4", 96) if l == 3 else SS
            ss_plan = []
            rem_ = C
            while rem_ > 0:
                n_ = min(SSL, rem_)
                ss_plan.append(n_)
                rem_ -= n_
            for t_ in dims.get("ss_tail", (8,)):
                if ss_plan[-1] > t_:
                    ss_plan[-1] -= t_
                    ss_plan.append(t_)
            s0 = 0
            GW = GEL if PK else max(GEL, RW)
            for ss, NCH in enumerate(ss_plan):
                Gt = wp.tile([P, SSL, GW], BF16, tag="gt" if GW > 68 else "gtp", bufs=4)
                if GEL == EL:
                    gps.dma_gather(
                        out_ap=Gt[:, 0:NCH, 0:GEL], in_ap=T_glob[l][:, :],
                        idxs_ap=src16[:, s0 * 8:(s0 + NCH) * 8],
                        num_idxs=NCH * CHUNK, num_idxs_reg=NCH * CHUNK,
                        elem_size=EL, single_packet=False, queue_num=ss % 2)
                else:
                    dma_gather_short(
                        out_ap=Gt[:, 0:NCH, 0:GEL], in_ap=T_glob[l][:, 0:GEL],
                        idxs_ap=src16[:, s0 * 8:(s0 + NCH) * 8],
                        num_idxs=NCH * CHUNK, elem_size=GEL, elem_step=EL,
                        queue_num=ss % 2)
                if l == 0:
                    # edge-term + mask precompute (feeds rhs cols 132:142 +
                    # later layers' alpha); mask folded into EAT row 5.
                    eaT_sl = eap.tile([5, SS * CHUNK], BF16, name="easl", tag="eat")
                    sync.dma_start(
                        out=eaT_sl[:, 0:NCH * CHUNK],
                        in_=pr["EAT"][:, s0 * CHUNK:(s0 + NCH) * CHUNK])
                    for q0 in range(0, NCH, 16):
                        qn = min(16, NCH - q0)
                        etp = pp.tile([P, 160], F32, tag="etp", bufs=1)
                        for j in range(qn):
                            ci = q0 + j
                            pe.matmul(out=etp[:, j * 10:(j + 1) * 10],
                                      lhsT=eaT_sl[:, ci * CHUNK:(ci + 1) * CHUNK],
                                      rhs=w5x10[:], start=True, stop=True)
                        act.copy(out=etc[:, s0 + q0:s0 + q0 + qn, :]
                                 .rearrange("p a b -> p (a b)"),
                                 in_=etp[:, 0:qn * 10])
                    # staircase one-hots built once, reused by all layers;
                    # built PT_AHEAD chunks ahead so the DVE cost sits in the
                    # pipeline's slack instead of its critical phase
                    pb0 = PT_AHEAD + s0
                    pb1 = min(pb0 + NCH, C)
                    for g in range(pb0, pb1, 8):
                        gn = min(8, pb1 - g)
                        vec.tensor_tensor(
                            out=pt_all[:, g:g + gn, :],
                            in0=dstr[:, g:g + gn].unsqueeze(2).to_broadcast([P, gn, BIN]),
                            in1=iotab[:].unsqueeze(1).to_broadcast([P, gn, BIN]),
                            op=ALU.is_equal)
                # alpha = b[src] (+ eterm) -> leaky relu -> exp
                AT = wp.tile([P, SSL, 8], BF16, tag="at", bufs=2)
                if PK:
                    SCT = wp.tile([P, SS, 142], BF16, tag="rhs", bufs=3)
                    BS = 0            # b slot in the packed gathered row
                else:
                    SCT = Gt
                    BS = HW
                if l > 0:
                    sl = [None, (0, 4), (4, 8), (8, 9)][l]
                    vec.tensor_tensor(out=AT[:, 0:NCH, 0:AW],
                                      in0=Gt[:, 0:NCH, BS:BS + AW],
                                      in1=etc[:, s0:s0 + NCH, sl[0]:sl[1]],
                                      op=ALU.add)
                    vec.tensor_scalar_mul(out=AT[:, 0:NCH, AW:2 * AW],
                                          in0=AT[:, 0:NCH, 0:AW], scalar1=0.2)
                    vec.tensor_tensor(out=AT[:, 0:NCH, 0:AW], in0=AT[:, 0:NCH, 0:AW],
                                      in1=AT[:, 0:NCH, AW:2 * AW], op=ALU.max)
                else:
                    vec.tensor_scalar_mul(out=AT[:, 0:NCH, AW:2 * AW],
                                          in0=Gt[:, 0:NCH, BS:BS + AW], scalar1=0.2)
                    vec.tensor_tensor(out=AT[:, 0:NCH, 0:AW],
                                      in0=Gt[:, 0:NCH, BS:BS + AW],
                                      in1=AT[:, 0:NCH, AW:2 * AW], op=ALU.max)
                act.activation(out=SCT[:, 0:NCH, HW:HW + AW], in_=AT[:, 0:NCH, 0:AW],
                               func=AF.Exp)
                if PK:
                    # h x ex: leading HX columns straight from fp8 on DVE (1x);
                    # the rest cast to bf16 on Act, then multiplied in 2x mode
                    vec.tensor_tensor(
                        out=SCT[:, 0:NCH, 0:HX].rearrange("p s (c a) -> p s c a", a=AW),
                        in0=Gt[:, 0:NCH, 4:4 + HX // 2].bitcast(FP8)
                            .rearrange("p s (c a) -> p s c a", a=AW),
                        in1=SCT[:, 0:NCH, HW:HW + AW].unsqueeze(2)
                            .to_broadcast([P, NCH, HX // AW, AW]),
                        op=ALU.mult)
                    if HX < HW:
                        act.copy(out=SCT[:, 0:NCH, HX:HW],
                                 in_=Gt[:, 0:NCH, 4 + HX // 2:4 + HW // 2].bitcast(FP8))
                        PHX = dims.get("phx", 16) if l in (1, 2) else 0
                        DH = HW - PHX
                        vec.tensor_tensor(
                            out=SCT[:, 0:NCH, HX:DH].rearrange("p s (c a) -> p s c a", a=AW),
                            in0=SCT[:, 0:NCH, HX:DH].rearrange("p s (c a) -> p s c a", a=AW),
                            in1=SCT[:, 0:NCH, HW:HW + AW].unsqueeze(2)
                                .to_broadcast([P, NCH, (DH - HX) // AW, AW]),
                            op=ALU.mult)
                        if PHX:
                            gps.tensor_tensor(
                                out=SCT[:, 0:NCH, DH:HW].rearrange("p s (c a) -> p s c a", a=AW),
                                in0=SCT[:, 0:NCH, DH:HW].rearrange("p s (c a) -> p s c a", a=AW),
                                in1=SCT[:, 0:NCH, HW:HW + AW].unsqueeze(2)
                                    .to_broadcast([P, NCH, PHX // AW, AW]),
                                op=ALU.mult)
                else:
                    vec.tensor_tensor(
                        out=SCT[:, 0:NCH, 0:HW].rearrange("p s (c a) -> p s c a", a=AW),
                        in0=SCT[:, 0:NCH, 0:HW].rearrange("p s (c a) -> p s c a", a=AW),
                        in1=SCT[:, 0:NCH, HW:HW + AW].unsqueeze(2)
                            .to_broadcast([P, NCH, HW // AW, AW]),
                        op=ALU.mult)
                if l == 0:
                    # append eterm9|cnt as rhs cols 132:142
                    act.copy(out=SCT[:, 0:NCH, 132:142],
                             in_=etc[:, s0:s0 + NCH, :])
                # scatter matmuls
                for c_i in range(NCH):
                    gc = s0 + c_i
                    b = bin_of_chunk[gc]
                    w_ = win_of_bin[b]
                    g_ = grp_of_win[w_]
                    if g_ not in grp_tiles:
                        open_group(g_)
                    if gc == first_chunk_of_bin[b]:
                        cur_bin_tile[b] = bp.tile([BIN, 142], F32, name="binacc",
                                                  tag="binacc")
                    pe.matmul(out=cur_bin_tile[b][:, 0:RW],
                              lhsT=pt_all[:, gc, :], rhs=SCT[:, c_i, 0:RW],
                              start=(gc == first_chunk_of_bin[b]),
                              stop=(gc == last_chunk_of_bin[b]))
                    if gc == last_chunk_of_bin[b]:
                        j = b % BPW
                        wrel = w_ - grp_bounds[g_][0]
                        act.copy(out=grp_tiles[g_][BIN * j:BIN * (j + 1), wrel, 0:RW],
                                 in_=cur_bin_tile[b][:, 0:RW])
                        del cur_bin_tile[b]
                    if gc == last_chunk_of_grp.get(g_, None):
                        epilogue_group(g_)
                s0 += NCH
            # groups never triggered (e.g. all-empty windows)
            for g_ in range(NG):
                if g_ not in grp_done:
                    if g_ not in grp_tiles:
                        open_group(g_)
                    epilogue_group(g_)
            if l < 3:
                if SIM1:
                    sync.dma_start(out=T_glob[l + 1][0:NPC, :], in_=T_loc[l + 1][:, :])
                else:
                    gps.collective_compute(
                        "AllGather", ALU.bypass, replica_groups=[list(range(NCORES))],
                        ins=[T_loc[l + 1][:, :]], outs=[T_glob[l + 1][:, :]])
                T_sb = T_next

        # ============ readout
        gsum_sb = cp.tile([32, Gn], F32)
        act.copy(out=gsum_sb[:], in_=gsum_ps[:])
        sync.dma_start(out=ar_in[:], in_=gsum_sb[:])
        if SIM1:
            sync.dma_start(out=ar_out[:], in_=ar_in[:])
        else:
            gps.collective_compute("AllReduce", ALU.add,
                                   replica_groups=[list(range(NCORES))],
                                   ins=[ar_in[:]], outs=[ar_out[:]])
        gs = cp.tile([32, Gn], F32)
        sync.dma_start(out=gs[:], in_=ar_out[:])
        vec.tensor_tensor(out=comb[0:32, :], in0=gs[:, :], in1=cntrb[:],
                          op=ALU.mult)
        blt = cp.tile([1, 1], F32)
        vec.memset(blt[:], bl)
        fin = pp.tile([1, Gn], F32, tag="hps", bufs=4 - ZTPB)
        pe.matmul(out=fin[:], lhsT=wlin_sb[:], rhs=comb[:], start=True, stop=True)
        res_sb = cp.tile([1, Gn], F32)
        act.activation(out=res_sb[:], in_=fin[:], func=AF.Sigmoid, bias=blt[:])
        sync.dma_start(out=out_p[:, :], in_=res_sb[:])

    nc.finalize()
    return nc


# ------------------------------------------------------------------ entry
def _run(inputs, trace=False, debug=False):
    dims, shared, per_core = host_prep(inputs)
    nc = build_program(dims, shared)
    in_maps = [{**shared, **pc} for pc in per_core]
    from concourse.bass_utils import run_bass_kernel_spmd
    return run_bass_kernel_spmd(nc, in_maps, list(range(NCORES)), trace=trace)


def kernel(**inputs):
    res = _run(inputs)
    return res.results[0]["out"].reshape(-1).astype(np.float32)


# revision 41
# speedup vs baseline: 1.0346x; 1.0081x over previous
"""EnhancedGAT Trainium2 Bass kernel (8 NeuronCores, SPMD).

Strategy:
  - Edges are sorted by destination node on the host; core k owns dst nodes
    [k*N/8, (k+1)*N/8) and every edge targeting them. Per-core edge lists are
    bucketed into 64-node bins and padded to 128-edge chunks with a per-bin
    chunk count shared across cores (SPMD uniformity). Dummy (padding) edges
    carry dst-offset 64, which falls outside the 64-wide one-hot used by the
    scatter matmuls, so they contribute exactly nothing.
  - Each GAT layer:
      node phase: every core computes a table row [h(128) | b(4)] for its own
        nodes, where b = per-head <h, att_s + att_d> comes directly out of the
        h matmul via 4 extra weight columns W @ A. Rows live in a [NPC, 256]
        bf16 DRAM table (512B stride for the gather); an AllGather replicates
        it to every core.
      edge phase: per 4096-edge superstep one dma_gather pulls the rows for
        the edges' sources; attention coefficients alpha = b[src] (+ edge
        term) are leaky-relu'd and exp'd in place, messages h*ex are scattered
        into per-bin PSUM accumulators via one-hot matmuls. Softmax is
        unnormalized (max-subtraction skipped; alphas are O(0.3)); the divide
        happens per node at the group epilogue, where self-loop contributions
        are added. As soon as a window-group's epilogue finishes, the NEXT
        layer's node phase for those windows runs (transpose + matmul + table
        write), hiding the layer boundary behind the remaining gathers.
  - Layer 1 additionally computes, per edge, the folded edge-attention terms
    for layers 2-4 (eterm = ea @ V + be, with the padding mask folded in as a
    fifth all-ones/zeros EAT row) plus the per-edge mask into an [C,10] SBUF
    cache, and accumulates per-node mean edge-feature terms and in-degrees
    (extra scatter-matmul columns) used by the self-loops of layers 2-4.
  - Final graph mean-pool via one-hot matmuls into a [33, G] accumulator,
    AllReduce across cores, tiny dense readout replicated on every core.
"""
import sys
import numpy as np

sys.path.insert(0, "/opt/trn_rl_repo")

HID = 32
NCORES = 8
P = 128
BIN = 64
SS = 32          # chunks per superstep
CHUNK = 128
ROW = 256        # table row elements (bf16) for layers 1-3 (512B stride)
ROW4 = 128       # layer-4 table row elements


# ----------------------------------------------------------------- host prep
def host_prep(inputs):
    x = np.asarray(inputs["x"], np.float32)
    ei = np.asarray(inputs["edge_index"]).astype(np.int64)
    ea = np.asarray(inputs["edge_attr"], np.float32)
    batch = np.asarray(inputs["batch"]).astype(np.int64)
    desc = np.asarray(inputs["descriptors"], np.float32)

    N = x.shape[0]
    E = ei.shape[1]
    Gn = desc.shape[0]
    NPC = N // NCORES
    NW = -(-NPC // P)
    NBINS = -(-NPC // BIN)

    src_all, dst_all = ei[0], ei[1]
    order = np.argsort(dst_all, kind="stable")
    src_s, dst_s = src_all[order], dst_all[order]
    ea_s = ea[order]
    core_of = dst_s // NPC
    local = dst_s - core_of * NPC
    bin_of = local // BIN

    cnt = np.zeros((NCORES, NBINS), np.int64)
    np.add.at(cnt, (core_of, bin_of), 1)
    cpb = np.max(-(-cnt // CHUNK), axis=0)          # chunks per bin (shared)
    cpb = np.maximum(cpb, 1)                        # every bin gets a chunk
    C_total = int(cpb.sum())
    off = np.zeros(NBINS, np.int64)
    off[1:] = np.cumsum(cpb)[:-1]
    EP = C_total * CHUNK                            # padded edges per core

    per_core = []
    for k in range(NCORES):
        srck = np.zeros(EP, np.int64)
        dstrk = np.full(EP, float(BIN), np.float32)  # dummies -> dead one-hot
        maskk = np.zeros(EP, np.float32)
        eak = np.zeros((EP, 4), np.float32)
        sel = core_of == k
        bins_k = bin_of[sel]
        start = np.searchsorted(bins_k, np.arange(NBINS))
        pos = np.arange(bins_k.size) - start[bins_k]
        slot = off[bins_k] * CHUNK + pos
        srck[slot] = src_s[sel]
        dstrk[slot] = (local[sel] - bins_k * BIN).astype(np.float32)
        maskk[slot] = 1.0
        eak[slot] = ea_s[sel]

        # device layouts: edge e = c*128 + p
        src16 = np.tile(srck.reshape(-1, 16).T.astype(np.int16), (8, 1))
        dstr_d = dstrk.reshape(C_total, P).T.copy()
        import ml_dtypes
        ea5 = np.concatenate([eak.T, maskk[None, :]], axis=0).astype(ml_dtypes.bfloat16)

        xk = x[k * NPC:(k + 1) * NPC]
        xT = np.zeros((8, NW * P), np.float32)
        xT[:, :NPC] = xk.T
        bk = np.full(NW * P, Gn + 5, np.float32)
        bk[:NPC] = batch[k * NPC:(k + 1) * NPC].astype(np.float32)
        batch_d = bk.reshape(NW, P).T.copy()

        per_core.append(dict(SRC16=src16, DSTR=dstr_d, EAT=ea5,
                             XT=xT, BATCH=batch_d))

    # ---- weight folding
    w = {k: np.asarray(v, np.float32) for k, v in inputs.items()
         if k not in ("x", "edge_index", "edge_attr", "batch", "descriptors")}

    def vfold(We, ae, heads):
        Vp = (We.reshape(w["We_enc"].shape[1], heads, HID) * ae[None]).sum(-1)
        return w["We_enc"] @ Vp, w["be_enc"] @ Vp      # [4,heads],[heads]

    V2, bv2 = vfold(w["We2"], w["ae2"], 4)
    V3, bv3 = vfold(w["We3"], w["ae3"], 4)
    V4, bv4 = vfold(w["We4"], w["ae4"], 1)
    # [5,10]: rows = 4 edge-attr dims + mask; cols = 9 eterms + cnt
    W5x10 = np.zeros((5, 10), np.float32)
    W5x10[0:4, 0:9] = np.concatenate([V2, V3, V4], axis=1)
    W5x10[4, 0:9] = np.concatenate([bv2, bv3, bv4])
    W5x10[4, 9] = 1.0

    def padr(v, n):
        o = np.zeros(n, np.float32)
        o[: v.size] = v
        return o

    # channel-major reorder of the 128-wide (4 heads x 32 ch) dimension:
    # new position c*4+a holds old a*32+c. Keeps per-head broadcasts
    # innermost-packed on DVE (2x mode).
    cm = (np.arange(128) % 4) * 32 + np.arange(128) // 4

    def wext(W, att_s, att_d, heads):
        # append per-head b-columns: b_a = h . (att_s+att_d)_a
        att = (att_s + att_d).reshape(-1)  # [heads*HID] head-major
        if heads == 4:
            attc = att[cm]                 # channel-major to match W cols
            A = np.zeros((128, 4), np.float32)
            A[np.arange(128), np.arange(128) % 4] = attc
        else:
            A = att[:, None]               # [32,1]
        return np.concatenate([W, W @ A], axis=1)

    W1e = wext(w["W1"][:, cm], w["as1"], w["ad1"], 4)            # [8,132]
    W2e = wext(w["W2"][cm][:, cm], w["as2"], w["ad2"], 4)        # [128,132]
    W3e = wext(w["W3"][cm][:, cm], w["as3"], w["ad3"], 4)
    W4e = wext(w["W4"][cm], w["as4"], w["ad4"], 1)               # [128,33]

    bout = np.stack([padr(w["b1"][cm], 128), padr(w["b2"][cm], 128),
                     padr(w["b3"][cm], 128), padr(w["b4"], 128)])

    import ml_dtypes
    T0h = (x @ W1e).astype(np.float32)
    pk0 = np.zeros((N, 256), np.uint8)
    pk0[:, 0:8] = T0h[:, 128:132].astype(ml_dtypes.bfloat16).view(np.uint8)
    pk0[:, 8:136] = T0h[:, 0:128].astype(ml_dtypes.float8_e4m3).view(np.uint8)
    TG0 = pk0.view(ml_dtypes.bfloat16)

    gcnt = np.bincount(batch, minlength=Gn).astype(np.float32)
    cntr = (1.0 / np.maximum(gcnt, 1.0))[None, :]           # [1, Gn]
    shared = dict(
        W1=W1e, WL2=W2e, WL3=W3e, WL4=W4e, TG0=TG0,
        W5X10=W5x10, BOUT=bout, CNTR=cntr,
        WD=w["Wd"], BD=w["bd"][:, None], WLIN=w["Wl"], DESCT=desc.T.copy(),
    )
    bl = float(np.asarray(w["bl"]).reshape(-1)[0])

    dims = dict(N=N, E=E, Gn=Gn, NPC=NPC, NW=NW, NBINS=NBINS,
                C=C_total, cpb=cpb, off=off, bl=bl)
    return dims, shared, per_core


# ------------------------------------------------------------- program build
def build_program(dims, shared):
    import concourse.bass as bass
    import concourse.mybir as mybir
    import concourse.tile as tile
    import concourse.bacc as bacc
    from concourse.masks import make_identity
    from contextlib import ExitStack

    F32 = mybir.dt.float32
    FP8 = mybir.dt.float8e4
    BF16 = mybir.dt.bfloat16
    I32 = mybir.dt.int32
    I16 = mybir.dt.int16
    AF = mybir.ActivationFunctionType
    ALU = mybir.AluOpType
    AX = mybir.AxisListType

    N, Gn, NPC, NW, NBINS, C = (dims[k] for k in ("N", "Gn", "NPC", "NW", "NBINS", "C"))
    cpb, off, bl = dims["cpb"], dims["off"], dims["bl"]
    NSS = C // SS
    # layer params: h width, heads, rhs width, gather row elems
    # PK tables pack rows as [b bf16 x4 | h fp8 x128] (136B) in a 256B stride;
    # HX = leading h-columns multiplied on DVE straight from fp8 (1x mode), the
    # rest is cast to bf16 on Act first so the DVE part runs in 2x mode.
    HXD = dims.get("hx", {0: 64, 1: 48, 2: 48})
    PKL = dims.get("pk_layers", (0, 1, 2))
    LP = [dict(HW=128, AW=4, RW=142, EL=128 if 0 in PKL else ROW,
               GEL=68 if 0 in PKL else 132, PK=0 in PKL, HX=HXD[0]),
          dict(HW=128, AW=4, RW=132, EL=128, GEL=68, PK=1 in PKL, HX=HXD[1]),
          dict(HW=128, AW=4, RW=132, EL=128, GEL=68, PK=2 in PKL, HX=HXD[2]),
          dict(HW=32, AW=1, RW=33, EL=ROW4, GEL=34, PK=False, HX=32)]
    for l_ in (1, 2):
        if not LP[l_]["PK"]:
            LP[l_].update(EL=ROW, GEL=ROW)

    nc = bacc.Bacc(num_swdge_queues=2)
    SIM1 = dims.get("sim1", False)

    # ---- params
    pr = {}
    for nm, shp, dt in [("SRC16", [P, C * 8], I16), ("DSTR", [P, C], F32),
                        ("EAT", [5, C * CHUNK], BF16), ("XT", [8, NW * P], F32),
                        ("BATCH", [P, NW], F32), ("W1", [8, 132], F32),
                        ("WL2", [128, 132], F32), ("WL3", [128, 132], F32),
                        ("WL4", [128, 33], F32), ("W5X10", [5, 10], F32),
                        ("BOUT", [4, 128], F32),
                        ("WD", [48, 32], F32), ("BD", [32, 1], F32),
                        ("WLIN", [64, 1], F32), ("DESCT", [48, Gn], F32),
                        ("CNTR", [1, Gn], F32), ("TG0", [N, 128], BF16)]:
        pr[nm] = nc.declare_dram_parameter(nm, shp, dt, isOutput=False)
    out_p = nc.declare_dram_parameter("out", [1, Gn], F32, isOutput=True)
    pr_TG0_ph = pr["TG0"]

    # ---- internal DRAM
    T_loc = [None] + [nc.dram_tensor(f"T_loc{l}", [NPC, LP[l]["EL"]], BF16)
                      for l in range(1, 4)]
    T_glob = [pr_TG0_ph] + [nc.dram_tensor(f"T_glob{l}", [N, LP[l]["EL"]], BF16,
                                           addr_space="Shared")
                            for l in range(1, 4)]
    ar_in = nc.dram_tensor("ar_in", [32, Gn], F32)
    ar_out = nc.dram_tensor("ar_out", [32, Gn], F32, addr_space="Shared")

    # bin/window bookkeeping (compile-time)
    bin_of_chunk = []
    for b in range(NBINS):
        bin_of_chunk += [b] * int(cpb[b])
    BPW = P // BIN  # bins per window
    win_of_bin = [b // BPW for b in range(NBINS)]
    last_chunk_of_bin = {}
    first_chunk_of_bin = {}
    for c_i, b in enumerate(bin_of_chunk):
        last_chunk_of_bin[b] = c_i
        first_chunk_of_bin.setdefault(b, c_i)

    with tile.TileContext(nc) as tc, ExitStack() as ctx:
        cp = ctx.enter_context(tc.tile_pool(name="const", bufs=1))
        wp = ctx.enter_context(tc.tile_pool(name="work", bufs=2))
        vp = ctx.enter_context(tc.tile_pool(name="win", bufs=2))
        pp = ctx.enter_context(tc.tile_pool(name="psum", bufs=2, space="PSUM"))
        bp = ctx.enter_context(tc.tile_pool(name="binp", bufs=2, space="PSUM"))

        sync, gps, vec, act, pe = nc.sync, nc.gpsimd, nc.vector, nc.scalar, nc.tensor

        def dma_gather_short(out_ap, in_ap, idxs_ap, num_idxs, elem_size,
                             elem_step, queue_num):
            from concourse.bass import exact_div
            eng = gps
            _in_ap = eng.lower_ap_dma(in_ap, for_custom_bir_dma=True)
            _idxs_ap = eng.lower_ap(idxs_ap)
            _out_ap = eng.lower_ap(out_ap)
            stride_bytes_256 = exact_div(elem_step * 2, 256)
            return eng.add_instruction(
                mybir.InstDMAGatherAnt(
                    name=eng.bass.get_next_instruction_name(),
                    ins=[*_in_ap, _idxs_ap,
                         eng.lower_val_access(eng.to_reg(num_idxs))],
                    outs=[_out_ap],
                    transpose=False, num_idxs=num_idxs, elem_size=elem_size,
                    stride_bytes_256=stride_bytes_256, gen_mode=0,
                    single_packet=False, queue_num=queue_num,
                    sbuf_tokens_per_rank=0, sbuf_free_dim_per_rank=0,
                    sbuf_free_dim_pad_per_rank=0, sbuf_byte_offset=0))
        ZTPB = dims.get("ztpb", 1)

        # ---- resident tiles
        src16 = cp.tile([P, C * 8], I16)
        sync.dma_start(out=src16[:], in_=pr["SRC16"][:, :])
        dstr = cp.tile([P, C], BF16)
        gps.dma_start(out=dstr[:], in_=pr["DSTR"][:, :])   # f32 -> bf16 cast
        batcht = cp.tile([P, NW], F32)
        sync.dma_start(out=batcht[:], in_=pr["BATCH"][:, :])
        xT_sb = cp.tile([8, NW * P], BF16)
        gps.dma_start(out=xT_sb[:], in_=pr["XT"][:, :])

        iota_i = cp.tile([P, BIN], I32)
        gps.iota(iota_i[:], pattern=[[1, BIN]], base=0, channel_multiplier=0)
        iotab = cp.tile([P, BIN], BF16)
        vec.tensor_copy(iotab[:], iota_i[:])
        iotag_i = cp.tile([P, Gn], I32)
        gps.iota(iotag_i[:], pattern=[[1, Gn]], base=0, channel_multiplier=0)
        iotagf = cp.tile([P, Gn], F32)
        vec.tensor_copy(iotagf[:], iotag_i[:])
        identf = cp.tile([P, P], F32)
        make_identity(nc, identf[:])

        w1_sb = cp.tile([8, 132], BF16)
        gps.dma_start(out=w1_sb[:], in_=pr["W1"][:, :])
        wl_sb = [None,
                 cp.tile([128, 132], BF16, name="wl2", tag="wl2"),
                 cp.tile([128, 132], BF16, name="wl3", tag="wl3"),
                 cp.tile([128, 33], BF16, name="wl4", tag="wl4")]
        gps.dma_start(out=wl_sb[1][:], in_=pr["WL2"][:, :])   # gpsimd casts f32->bf16
        gps.dma_start(out=wl_sb[2][:], in_=pr["WL3"][:, :])
        gps.dma_start(out=wl_sb[3][:], in_=pr["WL4"][:, :])
        w5x10 = cp.tile([5, 10], BF16)
        gps.dma_start(out=w5x10[:], in_=pr["W5X10"][:, :])
        bout_t = []
        for l in range(4):
            t3 = cp.tile([P, 128], F32, tag=f"bout{l}")
            sync.dma_start(out=t3[:], in_=pr["BOUT"][l:l + 1, :].to_broadcast([P, 128]))
            bout_t.append(t3)

        etc = cp.tile([P, C, 10], BF16)      # eterm9 | cnt  per edge
        pt_all = cp.tile([P, C, BIN], BF16)  # one-hot dst rows per edge
        loop_sb = cp.tile([P, NW, 10], F32)
        gsp = ctx.enter_context(tc.tile_pool(name="gsp", bufs=1, space="PSUM"))
        eap = ctx.enter_context(tc.tile_pool(name="eap", bufs=1))
        gsum_ps = None
        n_pool_mm = [0]

        # ---- readout head start: descriptor branch is input-independent
        comb = cp.tile([64, Gn], F32)
        wd_sb = cp.tile([48, 32], F32)
        sync.dma_start(out=wd_sb[:], in_=pr["WD"][:, :])
        desct_sb = cp.tile([48, Gn], F32)
        sync.dma_start(out=desct_sb[:], in_=pr["DESCT"][:, :])
        bd_sb = cp.tile([32, 1], F32)
        sync.dma_start(out=bd_sb[:], in_=pr["BD"][:, :])
        dps = pp.tile([32, Gn], F32, tag="hps", bufs=4 - ZTPB)
        pe.matmul(out=dps[:], lhsT=wd_sb[:], rhs=desct_sb[:], start=True, stop=True)
        act.activation(out=comb[32:64, :], in_=dps[:], func=AF.Relu, bias=bd_sb[:])
        wlin_sb = cp.tile([64, 1], F32)
        sync.dma_start(out=wlin_sb[:], in_=pr["WLIN"][:, :])
        cntrb = cp.tile([32, Gn], F32)
        sync.dma_start(out=cntrb[:], in_=pr["CNTR"][0:1, :].to_broadcast([32, Gn]))

        WG = dims.get("wg", 5)  # max windows per epilogue group
        # non-uniform groups: taper toward the end so the serial layer-boundary
        # tail (last epilogue -> node phase -> AllGather) shrinks
        grp_bounds = []
        w0_ = 0
        while NW - w0_ > 10:
            grp_bounds.append((w0_, WG))
            w0_ += WG
        for t_ in dims.get("taper", (4, 3, 2, 1)):
            if NW - w0_ > t_:
                grp_bounds.append((w0_, t_))
                w0_ += t_
        if NW > w0_:
            grp_bounds.append((w0_, NW - w0_))
        NG = len(grp_bounds)
        grp_of_win = {}
        for gi, (gw0, gsz_) in enumerate(grp_bounds):
            for w_ in range(gw0, gw0 + gsz_):
                grp_of_win[w_] = gi
        last_chunk_of_grp = {}
        for b in range(NBINS):
            g_ = grp_of_win[win_of_bin[b]]
            last_chunk_of_grp[g_] = max(last_chunk_of_grp.get(g_, -1),
                                        last_chunk_of_bin[b])

        # T_sb tables: [P, NW, 132] (h | b); layer l+1's is built during
        # layer l's edge phase, group by group.
        def node_phase_group(l, g_, T_next, z_src):
            """Build T_next rows for group g_ of layer l (0-based), write T_loc."""
            w0, gsz = grp_bounds[g_]
            HWn = LP[l]["HW"]
            BW = HWn + LP[l]["AW"]  # table row width
            for w_ in range(w0, w0 + gsz):
                if l == 0:
                    hps = pp.tile([P, 132], F32, tag="hps", bufs=4 - ZTPB)
                    pe.matmul(out=hps[:, 0:BW], lhsT=xT_sb[:, w_ * P:(w_ + 1) * P],
                              rhs=w1_sb[:], start=True, stop=True)
                else:
                    ztp = pp.tile([P, P], F32, tag="ztp", bufs=ZTPB)
                    pe.transpose(out=ztp[:], in_=z_src[:, w_ - w0, 0:128],
                                 identity=identf[:])
                    zt_sb = wp.tile([P, P], BF16, tag="ztsb")
                    act.copy(out=zt_sb[:], in_=ztp[:])
                    hps = pp.tile([P, 132], F32, tag="hps", bufs=4 - ZTPB)
                    pe.matmul(out=hps[:, 0:BW], lhsT=zt_sb[:], rhs=wl_sb[l][:],
                              start=True, stop=True)
                act.copy(out=T_next[:, w_, 0:BW], in_=hps[:, 0:BW])
                if l > 0 and LP[l]["PK"]:
                    act.copy(out=Tpk[:, w_, 0:4], in_=hps[:, 128:132])
                    act.copy(out=Tpk[:, w_, 4:68].bitcast(FP8), in_=hps[:, 0:128])
            if l == 0:
                return  # layer-1 table ships as the TG0 param; SBUF copy only
            stage, SW = (Tpk, 68) if LP[l]["PK"] else (T_next, BW)
            # batched table write: full windows in one DMA, ragged tail apart
            wfull = gsz - (1 if (w0 + gsz) * P > NPC else 0)
            if wfull > 0:
                sync.dma_start(
                    out=T_loc[l][w0 * P:(w0 + wfull) * P, 0:SW]
                        .rearrange("(w p) e -> p w e", p=P),
                    in_=stage[:, w0:w0 + wfull, 0:SW])
            if wfull < gsz:
                w_ = w0 + wfull
                nr = NPC - w_ * P
                sync.dma_start(out=T_loc[l][w_ * P:w_ * P + nr, 0:SW],
                               in_=stage[0:nr, w_, 0:SW])

        PT_AHEAD = dims.get("pt_ahead", 64)
        # prebuild the one-hot cache for the first chunks while the layer-0
        # node phase occupies PE/Act
        for g in range(0, PT_AHEAD, 8):
            vec.tensor_tensor(
                out=pt_all[:, g:g + 8, :],
                in0=dstr[:, g:g + 8].unsqueeze(2).to_broadcast([P, 8, BIN]),
                in1=iotab[:].unsqueeze(1).to_broadcast([P, 8, BIN]),
                op=ALU.is_equal)

        # ---- layer 0 node phase (all groups up front)
        T_sb = wp.tile([P, NW, 132], BF16, tag="tsb")
        for g_ in range(NG):
            node_phase_group(0, g_, T_sb, None)

        for l in range(4):
            HW, AW, RW, EL, GEL, PK, HX = (
                LP[l][k] for k in ("HW", "AW", "RW", "EL", "GEL", "PK", "HX"))
            BW = HW + AW

            T_next = None
            if l < 3:
                T_next = wp.tile([P, NW, 132], BF16, name="tnext", tag="tsb")
                if LP[l + 1]["PK"]:
                    Tpk = wp.tile([P, NW, 68], BF16, name="tpk", tag="tpk", bufs=1)

            grp_tiles = {}
            grp_done = set()

            def open_group(g_):
                t = vp.tile([P, WG, 142], F32, name="wingrp", tag="wingrp")
                grp_tiles[g_] = t
                return t

            def epilogue_group(g_):
                w0, gsz = grp_bounds[g_]
                wg = grp_tiles[g_]
                scr = wp.tile([P, WG, 12], F32, name="scr", tag="scr")
                # self-loop alpha (= b_own [+ eterm means]) -> exp
                if l > 0:
                    sl = [None, (0, 4), (4, 8), (8, 9)][l]
                    vec.tensor_tensor(out=scr[:, 0:gsz, 0:AW],
                                      in0=T_sb[:, w0:w0 + gsz, HW:HW + AW],
                                      in1=loop_sb[:, w0:w0 + gsz, sl[0]:sl[1]],
                                      op=ALU.add)
                else:
                    act.copy(out=scr[:, 0:gsz, 0:AW],
                             in_=T_sb[:, w0:w0 + gsz, HW:HW + AW])
                vec.tensor_scalar_mul(out=scr[:, 0:gsz, 4:4 + AW],
                                      in0=scr[:, 0:gsz, 0:AW], scalar1=0.2)
                vec.tensor_tensor(out=scr[:, 0:gsz, 0:AW], in0=scr[:, 0:gsz, 0:AW],
                                  in1=scr[:, 0:gsz, 4:4 + AW], op=ALU.max)
                act.activation(out=scr[:, 0:gsz, 0:AW], in_=scr[:, 0:gsz, 0:AW],
                               func=AF.Exp)
                # num += h_own * ex_loop
                nt = wp.tile([P, WG, 128], BF16, name="nt", tag="nt")
                vec.tensor_tensor(
                    out=nt[:, 0:gsz, 0:HW].rearrange("p g (c a) -> p g c a", a=AW),
                    in0=T_sb[:, w0:w0 + gsz, 0:HW].rearrange("p g (c a) -> p g c a", a=AW),
                    in1=scr[:, 0:gsz, 0:AW].unsqueeze(2)
                        .to_broadcast([P, gsz, HW // AW, AW]),
                    op=ALU.mult)
                vec.tensor_tensor(out=wg[:, 0:gsz, 0:HW], in0=wg[:, 0:gsz, 0:HW],
                                  in1=nt[:, 0:gsz, 0:HW], op=ALU.add)
                # den -> reciprocal
                vec.tensor_tensor(out=scr[:, 0:gsz, 4:4 + AW],
                                  in0=wg[:, 0:gsz, HW:HW + AW],
                                  in1=scr[:, 0:gsz, 0:AW], op=ALU.add)
                vec.tensor_scalar_add(out=scr[:, 0:gsz, 4:4 + AW],
                                      in0=scr[:, 0:gsz, 4:4 + AW], scalar1=1e-16)
                vec.reciprocal(out=scr[:, 0:gsz, 4:4 + AW], in_=scr[:, 0:gsz, 4:4 + AW])
                if l == 0:
                    vec.tensor_scalar_max(out=scr[:, 0:gsz, 8:9],
                                          in0=wg[:, 0:gsz, 141:142], scalar1=1.0)
                    vec.reciprocal(out=scr[:, 0:gsz, 8:9], in_=scr[:, 0:gsz, 8:9])
                    vec.tensor_tensor(
                        out=loop_sb[:, w0:w0 + gsz, 0:9], in0=wg[:, 0:gsz, 132:141],
                        in1=scr[:, 0:gsz, 8:9].to_broadcast([P, gsz, 9]), op=ALU.mult)
                # z = num * recip(den) + bias [+ relu]
                vec.tensor_tensor(
                    out=wg[:, 0:gsz, 0:HW].rearrange("p g (c a) -> p g c a", a=AW),
                    in0=wg[:, 0:gsz, 0:HW].rearrange("p g (c a) -> p g c a", a=AW),
                    in1=scr[:, 0:gsz, 4:4 + AW].unsqueeze(2)
                        .to_broadcast([P, gsz, HW // AW, AW]),
                    op=ALU.mult)
                vec.tensor_tensor(
                    out=wg[:, 0:gsz, 0:HW], in0=wg[:, 0:gsz, 0:HW],
                    in1=bout_t[l][:, 0:HW].unsqueeze(1).to_broadcast([P, gsz, HW]),
                    op=ALU.add)
                if l < 3:
                    act.activation(out=wg[:, 0:gsz, 0:128], in_=wg[:, 0:gsz, 0:128],
                                   func=AF.Relu)
                    node_phase_group(l + 1, g_, T_next, wg)
                else:
                    nonlocal gsum_ps
                    pool_sb = wp.tile([P, WG, 32], BF16, name="pool_sb", tag="poolsb")
                    act.copy(out=pool_sb[:, 0:gsz, 0:32], in_=wg[:, 0:gsz, 0:32])
                    bt = wp.tile([P, WG, Gn], BF16, name="bt", tag="bt", bufs=1)
                    vec.tensor_tensor(
                        out=bt[:, 0:gsz, :],
                        in0=batcht[:, w0:w0 + gsz].unsqueeze(2).to_broadcast([P, gsz, Gn]),
                        in1=iotagf[:].unsqueeze(1).to_broadcast([P, gsz, Gn]),
                        op=ALU.is_equal)
                    if gsum_ps is None:
                        gsum_ps = gsp.tile([32, Gn], F32, name="gsum_ps")
                    for j_ in range(gsz):
                        n_pool_mm[0] += 1
                        pe.matmul(out=gsum_ps[:], lhsT=pool_sb[:, j_, :],
                                  rhs=bt[:, j_, :],
                                  start=(n_pool_mm[0] == 1),
                                  stop=(n_pool_mm[0] == NW))
                grp_done.add(g_)

            cur_bin_tile = {}
            SSL = dims.get("ss4", 96) if l == 3 else SS
            ss_plan = []
            rem_ = C
            while rem_ > 0:
                n_ = min(SSL, rem_)
                ss_plan.append(n_)
                rem_ -= n_
            for t_ in dims.get("ss_tail", (8,)):
                if ss_plan[-1] > t_:
                    ss_plan[-1] -= t_
                    ss_plan.append(t_)
            s0 = 0
            GW = GEL if PK else max(GEL, RW)
            for ss, NCH in enumerate(ss_plan):
                Gt = wp.tile([P, SSL, GW], BF16, tag="gt" if GW > 68 else "gtp", bufs=4)
                if GEL == EL:
                    gps.dma_gather(
                        out_ap=Gt[:, 0:NCH, 0:GEL], in_ap=T_glob[l][:, :],
                        idxs_ap=src16[:, s0 * 8:(s0 + NCH) * 8],
                        num_idxs=NCH * CHUNK, num_idxs_reg=NCH * CHUNK,
                        elem_size=EL, single_packet=False, queue_num=ss % 2)
                else:
                    dma_gather_short(
                        out_ap=Gt[:, 0:NCH, 0:GEL], in_ap=T_glob[l][:, 0:GEL],
                        idxs_ap=src16[:, s0 * 8:(s0 + NCH) * 8],
                        num_idxs=NCH * CHUNK, elem_size=GEL, elem_step=EL,
                        queue_num=ss % 2)
                if l == 0:
                    # edge-term + mask precompute (feeds rhs cols 132:142 +
                    # later layers' alpha); mask folded into EAT row 5.
                    eaT_sl = eap.tile([5, SS * CHUNK], BF16, name="easl", tag="eat")
                    sync.dma_start(
                        out=eaT_sl[:, 0:NCH * CHUNK],
                        in_=pr["EAT"][:, s0 * CHUNK:(s0 + NCH) * CHUNK])
                    for q0 in range(0, NCH, 16):
                        qn = min(16, NCH - q0)
                        etp = pp.tile([P, 160], F32, tag="etp", bufs=1)
                        for j in range(qn):
                            ci = q0 + j
                            pe.matmul(out=etp[:, j * 10:(j + 1) * 10],
                                      lhsT=eaT_sl[:, ci * CHUNK:(ci + 1) * CHUNK],
                                      rhs=w5x10[:], start=True, stop=True)
                        act.copy(out=etc[:, s0 + q0:s0 + q0 + qn, :]
                                 .rearrange("p a b -> p (a b)"),
                                 in_=etp[:, 0:qn * 10])
                    # staircase one-hots built once, reused by all layers;
                    # built PT_AHEAD chunks ahead so the DVE cost sits in the
                    # pipeline's slack instead of its critical phase
                    pb0 = PT_AHEAD + s0
                    pb1 = min(pb0 + NCH, C)
                    for g in range(pb0, pb1, 8):
                        gn = min(8, pb1 - g)
                        vec.tensor_tensor(
                            out=pt_all[:, g:g + gn, :],
                            in0=dstr[:, g:g + gn].unsqueeze(2).to_broadcast([P, gn, BIN]),
                            in1=iotab[:].unsqueeze(1).to_broadcast([P, gn, BIN]),
                            op=ALU.is_equal)
                # alpha = b[src] (+ eterm) -> leaky relu -> exp
                AT = wp.tile([P, SSL, 8], BF16, tag="at", bufs=2)
                if PK:
                    SCT = wp.tile([P, SS, 142], BF16, tag="rhs", bufs=3)
                    BS = 0            # b slot in the packed gathered row
                else:
                    SCT = Gt
                    BS = HW
                if l > 0:
                    sl = [None, (0, 4), (4, 8), (8, 9)][l]
                    vec.tensor_tensor(out=AT[:, 0:NCH, 0:AW],
                                      in0=Gt[:, 0:NCH, BS:BS + AW],
                                      in1=etc[:, s0:s0 + NCH, sl[0]:sl[1]],
                                      op=ALU.add)
                    vec.tensor_scalar_mul(out=AT[:, 0:NCH, AW:2 * AW],
                                          in0=AT[:, 0:NCH, 0:AW], scalar1=0.2)
                    vec.tensor_tensor(out=AT[:, 0:NCH, 0:AW], in0=AT[:, 0:NCH, 0:AW],
                                      in1=AT[:, 0:NCH, AW:2 * AW], op=ALU.max)
                else:
                    vec.tensor_scalar_mul(out=AT[:, 0:NCH, AW:2 * AW],
                                          in0=Gt[:, 0:NCH, BS:BS + AW], scalar1=0.2)
                    vec.tensor_tensor(out=AT[:, 0:NCH, 0:AW],
                                      in0=Gt[:, 0:NCH, BS:BS + AW],
                                      in1=AT[:, 0:NCH, AW:2 * AW], op=ALU.max)
                act.activation(out=SCT[:, 0:NCH, HW:HW + AW], in_=AT[:, 0:NCH, 0:AW],
                               func=AF.Exp)
                if PK:
                    # h x ex: leading HX columns straight from fp8 on DVE (1x);
                    # the rest cast to bf16 on Act, then multiplied in 2x mode
                    vec.tensor_tensor(
                        out=SCT[:, 0:NCH, 0:HX].rearrange("p s (c a) -> p s c a", a=AW),
                        in0=Gt[:, 0:NCH, 4:4 + HX // 2].bitcast(FP8)
                            .rearrange("p s (c a) -> p s c a", a=AW),
                        in1=SCT[:, 0:NCH, HW:HW + AW].unsqueeze(2)
                            .to_broadcast([P, NCH, HX // AW, AW]),
                        op=ALU.mult)
                    if HX < HW:
                        act.copy(out=SCT[:, 0:NCH, HX:HW],
                                 in_=Gt[:, 0:NCH, 4 + HX // 2:4 + HW // 2].bitcast(FP8))
                        PHX = (dims.get("phx", 16) if l in (1, 2)
                               else (dims.get("phx0", 16) if l == 0 else 0))
                        DH = HW - PHX
                        vec.tensor_tensor(
                            out=SCT[:, 0:NCH, HX:DH].rearrange("p s (c a) -> p s c a", a=AW),
                            in0=SCT[:, 0:NCH, HX:DH].rearrange("p s (c a) -> p s c a", a=AW),
                            in1=SCT[:, 0:NCH, HW:HW + AW].unsqueeze(2)
                                .to_broadcast([P, NCH, (DH - HX) // AW, AW]),
                            op=ALU.mult)
                        if PHX:
                            gps.tensor_tensor(
                                out=SCT[:, 0:NCH, DH:HW].rearrange("p s (c a) -> p s c a", a=AW),
                                in0=SCT[:, 0:NCH, DH:HW].rearrange("p s (c a) -> p s c a", a=AW),
                                in1=SCT[:, 0:NCH, HW:HW + AW].unsqueeze(2)
                                    .to_broadcast([P, NCH, PHX // AW, AW]),
                                op=ALU.mult)
                else:
                    vec.tensor_tensor(
                        out=SCT[:, 0:NCH, 0:HW].rearrange("p s (c a) -> p s c a", a=AW),
                        in0=SCT[:, 0:NCH, 0:HW].rearrange("p s (c a) -> p s c a", a=AW),
                        in1=SCT[:, 0:NCH, HW:HW + AW].unsqueeze(2)
                            .to_broadcast([P, NCH, HW // AW, AW]),
                        op=ALU.mult)
                if l == 0:
                    # append eterm9|cnt as rhs cols 132:142
                    act.copy(out=SCT[:, 0:NCH, 132:142],
                             in_=etc[:, s0:s0 + NCH, :])
                # scatter matmuls
                for c_i in range(NCH):
                    gc = s0 + c_i
                    b = bin_of_chunk[gc]
                    w_ = win_of_bin[b]
                    g_ = grp_of_win[w_]
                    if g_ not in grp_tiles:
                        open_group(g_)
                    if gc == first_chunk_of_bin[b]:
                        cur_bin_tile[b] = bp.tile([BIN, 142], F32, name="binacc",
                                                  tag="binacc")
                    pe.matmul(out=cur_bin_tile[b][:, 0:RW],
                              lhsT=pt_all[:, gc, :], rhs=SCT[:, c_i, 0:RW],
                              start=(gc == first_chunk_of_bin[b]),
                              stop=(gc == last_chunk_of_bin[b]))
                    if gc == last_chunk_of_bin[b]:
                        j = b % BPW
                        wrel = w_ - grp_bounds[g_][0]
                        act.copy(out=grp_tiles[g_][BIN * j:BIN * (j + 1), wrel, 0:RW],
                                 in_=cur_bin_tile[b][:, 0:RW])
                        del cur_bin_tile[b]
                    if gc == last_chunk_of_grp.get(g_, None):
                        epilogue_group(g_)
                s0 += NCH
            # groups never triggered (e.g. all-empty windows)
            for g_ in range(NG):
                if g_ not in grp_done:
                    if g_ not in grp_tiles:
                        open_group(g_)
                    epilogue_group(g_)
            if l < 3:
                if SIM1:
                    sync.dma_start(out=T_glob[l + 1][0:NPC, :], in_=T_loc[l + 1][:, :])
                else:
                    gps.collective_compute(
                        "AllGather", ALU.bypass, replica_groups=[list(range(NCORES))],
                        ins=[T_loc[l + 1][:, :]], outs=[T_glob[l + 1][:, :]])
                T_sb = T_next

        # ============ readout
        gsum_sb = cp.tile([32, Gn], F32)
        act.copy(out=gsum_sb[:], in_=gsum_ps[:])
        sync.dma_start(out=ar_in[:], in_=gsum_sb[:])
        if SIM1:
            sync.dma_start(out=ar_out[:], in_=ar_in[:])
        else:
            gps.collective_compute("AllReduce", ALU.add,
                                   replica_groups=[list(range(NCORES))],
                                   ins=[ar_in[:]], outs=[ar_out[:]])
        gs = cp.tile([32, Gn], F32)
        sync.dma_start(out=gs[:], in_=ar_out[:])
        vec.tensor_tensor(out=comb[0:32, :], in0=gs[:, :], in1=cntrb[:],
                          op=ALU.mult)
        blt = cp.tile([1, 1], F32)
        vec.memset(blt[:], bl)
        fin = pp.tile([1, Gn], F32, tag="hps", bufs=4 - ZTPB)
        pe.matmul(out=fin[:], lhsT=wlin_sb[:], rhs=comb[:], start=True, stop=True)
        res_sb = cp.tile([1, Gn], F32)
        act.activation(out=res_sb[:], in_=fin[:], func=AF.Sigmoid, bias=blt[:])
        sync.dma_start(out=out_p[:, :], in_=res_sb[:])

    nc.finalize()
    return nc


# ------------------------------------------------------------------ entry
def _run(inputs, trace=False, debug=False):
    dims, shared, per_core = host_prep(inputs)
    nc = build_program(dims, shared)
    in_maps = [{**shared, **pc} for pc in per_core]
    from concourse.bass_utils import run_bass_kernel_spmd
    return run_bass_kernel_spmd(nc, in_maps, list(range(NCORES)), trace=trace)


def kernel(**inputs):
    res = _run(inputs)
    return res.results[0]["out"].reshape(-1).astype(np.float32)


# revision 43
# speedup vs baseline: 1.0364x; 1.0018x over previous
"""EnhancedGAT Trainium2 Bass kernel (8 NeuronCores, SPMD).

Strategy:
  - Edges are sorted by destination node on the host; core k owns dst nodes
    [k*N/8, (k+1)*N/8) and every edge targeting them. Per-core edge lists are
    bucketed into 64-node bins and padded to 128-edge chunks with a per-bin
    chunk count shared across cores (SPMD uniformity). Dummy (padding) edges
    carry dst-offset 64, which falls outside the 64-wide one-hot used by the
    scatter matmuls, so they contribute exactly nothing.
  - Each GAT layer:
      node phase: every core computes a table row [h(128) | b(4)] for its own
        nodes, where b = per-head <h, att_s + att_d> comes directly out of the
        h matmul via 4 extra weight columns W @ A. Rows live in a [NPC, 256]
        bf16 DRAM table (512B stride for the gather); an AllGather replicates
        it to every core.
      edge phase: per 4096-edge superstep one dma_gather pulls the rows for
        the edges' sources; attention coefficients alpha = b[src] (+ edge
        term) are leaky-relu'd and exp'd in place, messages h*ex are scattered
        into per-bin PSUM accumulators via one-hot matmuls. Softmax is
        unnormalized (max-subtraction skipped; alphas are O(0.3)); the divide
        happens per node at the group epilogue, where self-loop contributions
        are added. As soon as a window-group's epilogue finishes, the NEXT
        layer's node phase for those windows runs (transpose + matmul + table
        write), hiding the layer boundary behind the remaining gathers.
  - Layer 1 additionally computes, per edge, the folded edge-attention terms
    for layers 2-4 (eterm = ea @ V + be, with the padding mask folded in as a
    fifth all-ones/zeros EAT row) plus the per-edge mask into an [C,10] SBUF
    cache, and accumulates per-node mean edge-feature terms and in-degrees
    (extra scatter-matmul columns) used by the self-loops of layers 2-4.
  - Final graph mean-pool via one-hot matmuls into a [33, G] accumulator,
    AllReduce across cores, tiny dense readout replicated on every core.
"""
import sys
import numpy as np

sys.path.insert(0, "/opt/trn_rl_repo")

HID = 32
NCORES = 8
P = 128
BIN = 64
SS = 32          # chunks per superstep
CHUNK = 128
ROW = 256        # table row elements (bf16) for layers 1-3 (512B stride)
ROW4 = 128       # layer-4 table row elements


# ----------------------------------------------------------------- host prep
def host_prep(inputs):
    x = np.asarray(inputs["x"], np.float32)
    ei = np.asarray(inputs["edge_index"]).astype(np.int64)
    ea = np.asarray(inputs["edge_attr"], np.float32)
    batch = np.asarray(inputs["batch"]).astype(np.int64)
    desc = np.asarray(inputs["descriptors"], np.float32)

    N = x.shape[0]
    E = ei.shape[1]
    Gn = desc.shape[0]
    NPC = N // NCORES
    NW = -(-NPC // P)
    NBINS = -(-NPC // BIN)

    src_all, dst_all = ei[0], ei[1]
    order = np.argsort(dst_all, kind="stable")
    src_s, dst_s = src_all[order], dst_all[order]
    ea_s = ea[order]
    core_of = dst_s // NPC
    local = dst_s - core_of * NPC
    bin_of = local // BIN

    cnt = np.zeros((NCORES, NBINS), np.int64)
    np.add.at(cnt, (core_of, bin_of), 1)
    cpb = np.max(-(-cnt // CHUNK), axis=0)          # chunks per bin (shared)
    cpb = np.maximum(cpb, 1)                        # every bin gets a chunk
    C_total = int(cpb.sum())
    off = np.zeros(NBINS, np.int64)
    off[1:] = np.cumsum(cpb)[:-1]
    EP = C_total * CHUNK                            # padded edges per core

    per_core = []
    for k in range(NCORES):
        srck = np.zeros(EP, np.int64)
        dstrk = np.full(EP, float(BIN), np.float32)  # dummies -> dead one-hot
        maskk = np.zeros(EP, np.float32)
        eak = np.zeros((EP, 4), np.float32)
        sel = core_of == k
        bins_k = bin_of[sel]
        start = np.searchsorted(bins_k, np.arange(NBINS))
        pos = np.arange(bins_k.size) - start[bins_k]
        slot = off[bins_k] * CHUNK + pos
        srck[slot] = src_s[sel]
        dstrk[slot] = (local[sel] - bins_k * BIN).astype(np.float32)
        maskk[slot] = 1.0
        eak[slot] = ea_s[sel]

        # device layouts: edge e = c*128 + p
        src16 = np.tile(srck.reshape(-1, 16).T.astype(np.int16), (8, 1))
        dstr_d = dstrk.reshape(C_total, P).T.copy()
        import ml_dtypes
        ea5 = np.concatenate([eak.T, maskk[None, :]], axis=0).astype(ml_dtypes.bfloat16)

        xk = x[k * NPC:(k + 1) * NPC]
        xT = np.zeros((8, NW * P), np.float32)
        xT[:, :NPC] = xk.T
        bk = np.full(NW * P, Gn + 5, np.float32)
        bk[:NPC] = batch[k * NPC:(k + 1) * NPC].astype(np.float32)
        batch_d = bk.reshape(NW, P).T.copy()

        per_core.append(dict(SRC16=src16, DSTR=dstr_d, EAT=ea5,
                             XT=xT, BATCH=batch_d))

    # ---- weight folding
    w = {k: np.asarray(v, np.float32) for k, v in inputs.items()
         if k not in ("x", "edge_index", "edge_attr", "batch", "descriptors")}

    def vfold(We, ae, heads):
        Vp = (We.reshape(w["We_enc"].shape[1], heads, HID) * ae[None]).sum(-1)
        return w["We_enc"] @ Vp, w["be_enc"] @ Vp      # [4,heads],[heads]

    V2, bv2 = vfold(w["We2"], w["ae2"], 4)
    V3, bv3 = vfold(w["We3"], w["ae3"], 4)
    V4, bv4 = vfold(w["We4"], w["ae4"], 1)
    # [5,10]: rows = 4 edge-attr dims + mask; cols = 9 eterms + cnt
    W5x10 = np.zeros((5, 10), np.float32)
    W5x10[0:4, 0:9] = np.concatenate([V2, V3, V4], axis=1)
    W5x10[4, 0:9] = np.concatenate([bv2, bv3, bv4])
    W5x10[4, 9] = 1.0

    def padr(v, n):
        o = np.zeros(n, np.float32)
        o[: v.size] = v
        return o

    # channel-major reorder of the 128-wide (4 heads x 32 ch) dimension:
    # new position c*4+a holds old a*32+c. Keeps per-head broadcasts
    # innermost-packed on DVE (2x mode).
    cm = (np.arange(128) % 4) * 32 + np.arange(128) // 4

    def wext(W, att_s, att_d, heads):
        # append per-head b-columns: b_a = h . (att_s+att_d)_a
        att = (att_s + att_d).reshape(-1)  # [heads*HID] head-major
        if heads == 4:
            attc = att[cm]                 # channel-major to match W cols
            A = np.zeros((128, 4), np.float32)
            A[np.arange(128), np.arange(128) % 4] = attc
        else:
            A = att[:, None]               # [32,1]
        return np.concatenate([W, W @ A], axis=1)

    W1e = wext(w["W1"][:, cm], w["as1"], w["ad1"], 4)            # [8,132]
    W2e = wext(w["W2"][cm][:, cm], w["as2"], w["ad2"], 4)        # [128,132]
    W3e = wext(w["W3"][cm][:, cm], w["as3"], w["ad3"], 4)
    W4e = wext(w["W4"][cm], w["as4"], w["ad4"], 1)               # [128,33]

    bout = np.stack([padr(w["b1"][cm], 128), padr(w["b2"][cm], 128),
                     padr(w["b3"][cm], 128), padr(w["b4"], 128)])

    import ml_dtypes
    T0h = (x @ W1e).astype(np.float32)
    pk0 = np.zeros((N, 256), np.uint8)
    pk0[:, 0:8] = T0h[:, 128:132].astype(ml_dtypes.bfloat16).view(np.uint8)
    pk0[:, 8:136] = T0h[:, 0:128].astype(ml_dtypes.float8_e4m3).view(np.uint8)
    TG0 = pk0.view(ml_dtypes.bfloat16)

    gcnt = np.bincount(batch, minlength=Gn).astype(np.float32)
    cntr = (1.0 / np.maximum(gcnt, 1.0))[None, :]           # [1, Gn]
    shared = dict(
        W1=W1e, WL2=W2e, WL3=W3e, WL4=W4e, TG0=TG0,
        W5X10=W5x10, BOUT=bout, CNTR=cntr,
        WD=w["Wd"], BD=w["bd"][:, None], WLIN=w["Wl"], DESCT=desc.T.copy(),
    )
    bl = float(np.asarray(w["bl"]).reshape(-1)[0])

    dims = dict(N=N, E=E, Gn=Gn, NPC=NPC, NW=NW, NBINS=NBINS,
                C=C_total, cpb=cpb, off=off, bl=bl)
    return dims, shared, per_core


# ------------------------------------------------------------- program build
def build_program(dims, shared):
    import concourse.bass as bass
    import concourse.mybir as mybir
    import concourse.tile as tile
    import concourse.bacc as bacc
    from concourse.masks import make_identity
    from contextlib import ExitStack

    F32 = mybir.dt.float32
    FP8 = mybir.dt.float8e4
    BF16 = mybir.dt.bfloat16
    I32 = mybir.dt.int32
    I16 = mybir.dt.int16
    AF = mybir.ActivationFunctionType
    ALU = mybir.AluOpType
    AX = mybir.AxisListType

    N, Gn, NPC, NW, NBINS, C = (dims[k] for k in ("N", "Gn", "NPC", "NW", "NBINS", "C"))
    cpb, off, bl = dims["cpb"], dims["off"], dims["bl"]
    NSS = C // SS
    # layer params: h width, heads, rhs width, gather row elems
    # PK tables pack rows as [b bf16 x4 | h fp8 x128] (136B) in a 256B stride;
    # HX = leading h-columns multiplied on DVE straight from fp8 (1x mode), the
    # rest is cast to bf16 on Act first so the DVE part runs in 2x mode.
    HXD = dims.get("hx", {0: 64, 1: 48, 2: 48})
    PKL = dims.get("pk_layers", (0, 1, 2))
    LP = [dict(HW=128, AW=4, RW=142, EL=128 if 0 in PKL else ROW,
               GEL=68 if 0 in PKL else 132, PK=0 in PKL, HX=HXD[0]),
          dict(HW=128, AW=4, RW=132, EL=128, GEL=68, PK=1 in PKL, HX=HXD[1]),
          dict(HW=128, AW=4, RW=132, EL=128, GEL=68, PK=2 in PKL, HX=HXD[2]),
          dict(HW=32, AW=1, RW=33, EL=ROW4, GEL=34, PK=False, HX=32)]
    for l_ in (1, 2):
        if not LP[l_]["PK"]:
            LP[l_].update(EL=ROW, GEL=ROW)

    nc = bacc.Bacc(num_swdge_queues=2)
    SIM1 = dims.get("sim1", False)

    # ---- params
    pr = {}
    for nm, shp, dt in [("SRC16", [P, C * 8], I16), ("DSTR", [P, C], F32),
                        ("EAT", [5, C * CHUNK], BF16), ("XT", [8, NW * P], F32),
                        ("BATCH", [P, NW], F32), ("W1", [8, 132], F32),
                        ("WL2", [128, 132], F32), ("WL3", [128, 132], F32),
                        ("WL4", [128, 33], F32), ("W5X10", [5, 10], F32),
                        ("BOUT", [4, 128], F32),
                        ("WD", [48, 32], F32), ("BD", [32, 1], F32),
                        ("WLIN", [64, 1], F32), ("DESCT", [48, Gn], F32),
                        ("CNTR", [1, Gn], F32), ("TG0", [N, 128], BF16)]:
        pr[nm] = nc.declare_dram_parameter(nm, shp, dt, isOutput=False)
    out_p = nc.declare_dram_parameter("out", [1, Gn], F32, isOutput=True)
    pr_TG0_ph = pr["TG0"]

    # ---- internal DRAM
    T_loc = [None] + [nc.dram_tensor(f"T_loc{l}", [NPC, LP[l]["EL"]], BF16)
                      for l in range(1, 4)]
    T_glob = [pr_TG0_ph] + [nc.dram_tensor(f"T_glob{l}", [N, LP[l]["EL"]], BF16,
                                           addr_space="Shared")
                            for l in range(1, 4)]
    ar_in = nc.dram_tensor("ar_in", [32, Gn], F32)
    ar_out = nc.dram_tensor("ar_out", [32, Gn], F32, addr_space="Shared")

    # bin/window bookkeeping (compile-time)
    bin_of_chunk = []
    for b in range(NBINS):
        bin_of_chunk += [b] * int(cpb[b])
    BPW = P // BIN  # bins per window
    win_of_bin = [b // BPW for b in range(NBINS)]
    last_chunk_of_bin = {}
    first_chunk_of_bin = {}
    for c_i, b in enumerate(bin_of_chunk):
        last_chunk_of_bin[b] = c_i
        first_chunk_of_bin.setdefault(b, c_i)

    with tile.TileContext(nc) as tc, ExitStack() as ctx:
        cp = ctx.enter_context(tc.tile_pool(name="const", bufs=1))
        wp = ctx.enter_context(tc.tile_pool(name="work", bufs=2))
        vp = ctx.enter_context(tc.tile_pool(name="win", bufs=dims.get("vpb", 2)))
        pp = ctx.enter_context(tc.tile_pool(name="psum", bufs=2, space="PSUM"))
        bp = ctx.enter_context(tc.tile_pool(name="binp", bufs=2, space="PSUM"))

        sync, gps, vec, act, pe = nc.sync, nc.gpsimd, nc.vector, nc.scalar, nc.tensor

        def dma_gather_short(out_ap, in_ap, idxs_ap, num_idxs, elem_size,
                             elem_step, queue_num):
            from concourse.bass import exact_div
            eng = gps
            _in_ap = eng.lower_ap_dma(in_ap, for_custom_bir_dma=True)
            _idxs_ap = eng.lower_ap(idxs_ap)
            _out_ap = eng.lower_ap(out_ap)
            stride_bytes_256 = exact_div(elem_step * 2, 256)
            return eng.add_instruction(
                mybir.InstDMAGatherAnt(
                    name=eng.bass.get_next_instruction_name(),
                    ins=[*_in_ap, _idxs_ap,
                         eng.lower_val_access(eng.to_reg(num_idxs))],
                    outs=[_out_ap],
                    transpose=False, num_idxs=num_idxs, elem_size=elem_size,
                    stride_bytes_256=stride_bytes_256, gen_mode=0,
                    single_packet=False, queue_num=queue_num,
                    sbuf_tokens_per_rank=0, sbuf_free_dim_per_rank=0,
                    sbuf_free_dim_pad_per_rank=0, sbuf_byte_offset=0))
        ZTPB = dims.get("ztpb", 1)

        # ---- resident tiles
        src16 = cp.tile([P, C * 8], I16)
        sync.dma_start(out=src16[:], in_=pr["SRC16"][:, :])
        dstr = cp.tile([P, C], BF16)
        gps.dma_start(out=dstr[:], in_=pr["DSTR"][:, :])   # f32 -> bf16 cast
        batcht = cp.tile([P, NW], F32)
        sync.dma_start(out=batcht[:], in_=pr["BATCH"][:, :])
        xT_sb = cp.tile([8, NW * P], BF16)
        gps.dma_start(out=xT_sb[:], in_=pr["XT"][:, :])

        iota_i = cp.tile([P, BIN], I32)
        gps.iota(iota_i[:], pattern=[[1, BIN]], base=0, channel_multiplier=0)
        iotab = cp.tile([P, BIN], BF16)
        vec.tensor_copy(iotab[:], iota_i[:])
        iotag_i = cp.tile([P, Gn], I32)
        gps.iota(iotag_i[:], pattern=[[1, Gn]], base=0, channel_multiplier=0)
        iotagf = cp.tile([P, Gn], F32)
        vec.tensor_copy(iotagf[:], iotag_i[:])
        identf = cp.tile([P, P], F32)
        make_identity(nc, identf[:])

        w1_sb = cp.tile([8, 132], BF16)
        gps.dma_start(out=w1_sb[:], in_=pr["W1"][:, :])
        wl_sb = [None,
                 cp.tile([128, 132], BF16, name="wl2", tag="wl2"),
                 cp.tile([128, 132], BF16, name="wl3", tag="wl3"),
                 cp.tile([128, 33], BF16, name="wl4", tag="wl4")]
        gps.dma_start(out=wl_sb[1][:], in_=pr["WL2"][:, :])   # gpsimd casts f32->bf16
        gps.dma_start(out=wl_sb[2][:], in_=pr["WL3"][:, :])
        gps.dma_start(out=wl_sb[3][:], in_=pr["WL4"][:, :])
        w5x10 = cp.tile([5, 10], BF16)
        gps.dma_start(out=w5x10[:], in_=pr["W5X10"][:, :])
        bout_t = []
        for l in range(4):
            t3 = cp.tile([P, 128], F32, tag=f"bout{l}")
            sync.dma_start(out=t3[:], in_=pr["BOUT"][l:l + 1, :].to_broadcast([P, 128]))
            bout_t.append(t3)

        etc = cp.tile([P, C, 10], BF16)      # eterm9 | cnt  per edge
        pt_all = cp.tile([P, C, BIN], BF16)  # one-hot dst rows per edge
        loop_sb = cp.tile([P, NW, 10], F32)
        gsp = ctx.enter_context(tc.tile_pool(name="gsp", bufs=1, space="PSUM"))
        eap = ctx.enter_context(tc.tile_pool(name="eap", bufs=1))
        gsum_ps = None
        n_pool_mm = [0]

        # ---- readout head start: descriptor branch is input-independent
        comb = cp.tile([64, Gn], F32)
        wd_sb = cp.tile([48, 32], F32)
        sync.dma_start(out=wd_sb[:], in_=pr["WD"][:, :])
        desct_sb = cp.tile([48, Gn], F32)
        sync.dma_start(out=desct_sb[:], in_=pr["DESCT"][:, :])
        bd_sb = cp.tile([32, 1], F32)
        sync.dma_start(out=bd_sb[:], in_=pr["BD"][:, :])
        dps = pp.tile([32, Gn], F32, tag="hps", bufs=4 - ZTPB)
        pe.matmul(out=dps[:], lhsT=wd_sb[:], rhs=desct_sb[:], start=True, stop=True)
        act.activation(out=comb[32:64, :], in_=dps[:], func=AF.Relu, bias=bd_sb[:])
        wlin_sb = cp.tile([64, 1], F32)
        sync.dma_start(out=wlin_sb[:], in_=pr["WLIN"][:, :])
        cntrb = cp.tile([32, Gn], F32)
        sync.dma_start(out=cntrb[:], in_=pr["CNTR"][0:1, :].to_broadcast([32, Gn]))

        WG = dims.get("wg", 5)  # max windows per epilogue group
        # non-uniform groups: taper toward the end so the serial layer-boundary
        # tail (last epilogue -> node phase -> AllGather) shrinks
        grp_bounds = []
        w0_ = 0
        while NW - w0_ > 10:
            grp_bounds.append((w0_, WG))
            w0_ += WG
        for t_ in dims.get("taper", (4, 3, 2, 1)):
            if NW - w0_ > t_:
                grp_bounds.append((w0_, t_))
                w0_ += t_
        if NW > w0_:
            grp_bounds.append((w0_, NW - w0_))
        NG = len(grp_bounds)
        grp_of_win = {}
        for gi, (gw0, gsz_) in enumerate(grp_bounds):
            for w_ in range(gw0, gw0 + gsz_):
                grp_of_win[w_] = gi
        last_chunk_of_grp = {}
        for b in range(NBINS):
            g_ = grp_of_win[win_of_bin[b]]
            last_chunk_of_grp[g_] = max(last_chunk_of_grp.get(g_, -1),
                                        last_chunk_of_bin[b])

        # T_sb tables: [P, NW, 132] (h | b); layer l+1's is built during
        # layer l's edge phase, group by group.
        def node_phase_group(l, g_, T_next, z_src):
            """Build T_next rows for group g_ of layer l (0-based), write T_loc."""
            w0, gsz = grp_bounds[g_]
            HWn = LP[l]["HW"]
            BW = HWn + LP[l]["AW"]  # table row width
            for w_ in range(w0, w0 + gsz):
                if l == 0:
                    hps = pp.tile([P, 132], F32, tag="hps", bufs=4 - ZTPB)
                    pe.matmul(out=hps[:, 0:BW], lhsT=xT_sb[:, w_ * P:(w_ + 1) * P],
                              rhs=w1_sb[:], start=True, stop=True)
                else:
                    ztp = pp.tile([P, P], F32, tag="ztp", bufs=ZTPB)
                    pe.transpose(out=ztp[:], in_=z_src[:, w_ - w0, 0:128],
                                 identity=identf[:])
                    zt_sb = wp.tile([P, P], BF16, tag="ztsb")
                    act.copy(out=zt_sb[:], in_=ztp[:])
                    hps = pp.tile([P, 132], F32, tag="hps", bufs=4 - ZTPB)
                    pe.matmul(out=hps[:, 0:BW], lhsT=zt_sb[:], rhs=wl_sb[l][:],
                              start=True, stop=True)
                act.copy(out=T_next[:, w_, 0:BW], in_=hps[:, 0:BW])
                if l > 0 and LP[l]["PK"]:
                    act.copy(out=Tpk[:, w_, 0:4], in_=hps[:, 128:132])
                    act.copy(out=Tpk[:, w_, 4:68].bitcast(FP8), in_=hps[:, 0:128])
            if l == 0:
                return  # layer-1 table ships as the TG0 param; SBUF copy only
            stage, SW = (Tpk, 68) if LP[l]["PK"] else (T_next, BW)
            # batched table write: full windows in one DMA, ragged tail apart
            wfull = gsz - (1 if (w0 + gsz) * P > NPC else 0)
            if wfull > 0:
                sync.dma_start(
                    out=T_loc[l][w0 * P:(w0 + wfull) * P, 0:SW]
                        .rearrange("(w p) e -> p w e", p=P),
                    in_=stage[:, w0:w0 + wfull, 0:SW])
            if wfull < gsz:
                w_ = w0 + wfull
                nr = NPC - w_ * P
                sync.dma_start(out=T_loc[l][w_ * P:w_ * P + nr, 0:SW],
                               in_=stage[0:nr, w_, 0:SW])

        PT_AHEAD = dims.get("pt_ahead", 64)
        # prebuild the one-hot cache for the first chunks while the layer-0
        # node phase occupies PE/Act
        for g in range(0, PT_AHEAD, 8):
            vec.tensor_tensor(
                out=pt_all[:, g:g + 8, :],
                in0=dstr[:, g:g + 8].unsqueeze(2).to_broadcast([P, 8, BIN]),
                in1=iotab[:].unsqueeze(1).to_broadcast([P, 8, BIN]),
                op=ALU.is_equal)

        # ---- layer 0 node phase (all groups up front)
        T_sb = wp.tile([P, NW, 132], BF16, tag="tsb")
        for g_ in range(NG):
            node_phase_group(0, g_, T_sb, None)

        for l in range(4):
            HW, AW, RW, EL, GEL, PK, HX = (
                LP[l][k] for k in ("HW", "AW", "RW", "EL", "GEL", "PK", "HX"))
            BW = HW + AW

            T_next = None
            if l < 3:
                T_next = wp.tile([P, NW, 132], BF16, name="tnext", tag="tsb")
                if LP[l + 1]["PK"]:
                    Tpk = wp.tile([P, NW, 68], BF16, name="tpk", tag="tpk", bufs=1)

            grp_tiles = {}
            grp_done = set()

            def open_group(g_):
                t = vp.tile([P, WG, 142], F32, name="wingrp", tag="wingrp")
                grp_tiles[g_] = t
                return t

            def epilogue_group(g_):
                w0, gsz = grp_bounds[g_]
                wg = grp_tiles[g_]
                scr = wp.tile([P, WG, 12], F32, name="scr", tag="scr")
                # self-loop alpha (= b_own [+ eterm means]) -> exp
                if l > 0:
                    sl = [None, (0, 4), (4, 8), (8, 9)][l]
                    vec.tensor_tensor(out=scr[:, 0:gsz, 0:AW],
                                      in0=T_sb[:, w0:w0 + gsz, HW:HW + AW],
                                      in1=loop_sb[:, w0:w0 + gsz, sl[0]:sl[1]],
                                      op=ALU.add)
                else:
                    act.copy(out=scr[:, 0:gsz, 0:AW],
                             in_=T_sb[:, w0:w0 + gsz, HW:HW + AW])
                vec.tensor_scalar_mul(out=scr[:, 0:gsz, 4:4 + AW],
                                      in0=scr[:, 0:gsz, 0:AW], scalar1=0.2)
                vec.tensor_tensor(out=scr[:, 0:gsz, 0:AW], in0=scr[:, 0:gsz, 0:AW],
                                  in1=scr[:, 0:gsz, 4:4 + AW], op=ALU.max)
                act.activation(out=scr[:, 0:gsz, 0:AW], in_=scr[:, 0:gsz, 0:AW],
                               func=AF.Exp)
                # num += h_own * ex_loop
                nt = wp.tile([P, WG, 128], BF16, name="nt", tag="nt")
                vec.tensor_tensor(
                    out=nt[:, 0:gsz, 0:HW].rearrange("p g (c a) -> p g c a", a=AW),
                    in0=T_sb[:, w0:w0 + gsz, 0:HW].rearrange("p g (c a) -> p g c a", a=AW),
                    in1=scr[:, 0:gsz, 0:AW].unsqueeze(2)
                        .to_broadcast([P, gsz, HW // AW, AW]),
                    op=ALU.mult)
                vec.tensor_tensor(out=wg[:, 0:gsz, 0:HW], in0=wg[:, 0:gsz, 0:HW],
                                  in1=nt[:, 0:gsz, 0:HW], op=ALU.add)
                # den -> reciprocal
                vec.tensor_tensor(out=scr[:, 0:gsz, 4:4 + AW],
                                  in0=wg[:, 0:gsz, HW:HW + AW],
                                  in1=scr[:, 0:gsz, 0:AW], op=ALU.add)
                vec.tensor_scalar_add(out=scr[:, 0:gsz, 4:4 + AW],
                                      in0=scr[:, 0:gsz, 4:4 + AW], scalar1=1e-16)
                vec.reciprocal(out=scr[:, 0:gsz, 4:4 + AW], in_=scr[:, 0:gsz, 4:4 + AW])
                if l == 0:
                    vec.tensor_scalar_max(out=scr[:, 0:gsz, 8:9],
                                          in0=wg[:, 0:gsz, 141:142], scalar1=1.0)
                    vec.reciprocal(out=scr[:, 0:gsz, 8:9], in_=scr[:, 0:gsz, 8:9])
                    vec.tensor_tensor(
                        out=loop_sb[:, w0:w0 + gsz, 0:9], in0=wg[:, 0:gsz, 132:141],
                        in1=scr[:, 0:gsz, 8:9].to_broadcast([P, gsz, 9]), op=ALU.mult)
                # z = num * recip(den) + bias [+ relu]
                vec.tensor_tensor(
                    out=wg[:, 0:gsz, 0:HW].rearrange("p g (c a) -> p g c a", a=AW),
                    in0=wg[:, 0:gsz, 0:HW].rearrange("p g (c a) -> p g c a", a=AW),
                    in1=scr[:, 0:gsz, 4:4 + AW].unsqueeze(2)
                        .to_broadcast([P, gsz, HW // AW, AW]),
                    op=ALU.mult)
                vec.tensor_tensor(
                    out=wg[:, 0:gsz, 0:HW], in0=wg[:, 0:gsz, 0:HW],
                    in1=bout_t[l][:, 0:HW].unsqueeze(1).to_broadcast([P, gsz, HW]),
                    op=ALU.add)
                if l < 3:
                    act.activation(out=wg[:, 0:gsz, 0:128], in_=wg[:, 0:gsz, 0:128],
                                   func=AF.Relu)
                    node_phase_group(l + 1, g_, T_next, wg)
                else:
                    nonlocal gsum_ps
                    pool_sb = wp.tile([P, WG, 32], BF16, name="pool_sb", tag="poolsb")
                    act.copy(out=pool_sb[:, 0:gsz, 0:32], in_=wg[:, 0:gsz, 0:32])
                    bt = wp.tile([P, WG, Gn], BF16, name="bt", tag="bt", bufs=1)
                    vec.tensor_tensor(
                        out=bt[:, 0:gsz, :],
                        in0=batcht[:, w0:w0 + gsz].unsqueeze(2).to_broadcast([P, gsz, Gn]),
                        in1=iotagf[:].unsqueeze(1).to_broadcast([P, gsz, Gn]),
                        op=ALU.is_equal)
                    if gsum_ps is None:
                        gsum_ps = gsp.tile([32, Gn], F32, name="gsum_ps")
                    for j_ in range(gsz):
                        n_pool_mm[0] += 1
                        pe.matmul(out=gsum_ps[:], lhsT=pool_sb[:, j_, :],
                                  rhs=bt[:, j_, :],
                                  start=(n_pool_mm[0] == 1),
                                  stop=(n_pool_mm[0] == NW))
                grp_done.add(g_)

            cur_bin_tile = {}
            SSL = dims.get("ss4", 96) if l == 3 else SS
            ss_plan = []
            rem_ = C
            for n_ in dims.get("ss_head", ()):
                if rem_ > n_ + SSL:
                    ss_plan.append(n_)
                    rem_ -= n_
            while rem_ > 0:
                n_ = min(SSL, rem_)
                ss_plan.append(n_)
                rem_ -= n_
            for t_ in dims.get("ss_tail", (12,)):
                if ss_plan[-1] > t_:
                    ss_plan[-1] -= t_
                    ss_plan.append(t_)
            s0 = 0
            GW = GEL if PK else max(GEL, RW)
            for ss, NCH in enumerate(ss_plan):
                Gt = wp.tile([P, SSL, GW], BF16, tag="gt" if GW > 68 else "gtp", bufs=4)
                if GEL == EL:
                    gps.dma_gather(
                        out_ap=Gt[:, 0:NCH, 0:GEL], in_ap=T_glob[l][:, :],
                        idxs_ap=src16[:, s0 * 8:(s0 + NCH) * 8],
                        num_idxs=NCH * CHUNK, num_idxs_reg=NCH * CHUNK,
                        elem_size=EL, single_packet=False, queue_num=ss % 2)
                else:
                    dma_gather_short(
                        out_ap=Gt[:, 0:NCH, 0:GEL], in_ap=T_glob[l][:, 0:GEL],
                        idxs_ap=src16[:, s0 * 8:(s0 + NCH) * 8],
                        num_idxs=NCH * CHUNK, elem_size=GEL, elem_step=EL,
                        queue_num=ss % 2)
                if l == 0:
                    # edge-term + mask precompute (feeds rhs cols 132:142 +
                    # later layers' alpha); mask folded into EAT row 5.
                    eaT_sl = eap.tile([5, SS * CHUNK], BF16, name="easl", tag="eat")
                    sync.dma_start(
                        out=eaT_sl[:, 0:NCH * CHUNK],
                        in_=pr["EAT"][:, s0 * CHUNK:(s0 + NCH) * CHUNK])
                    for q0 in range(0, NCH, 16):
                        qn = min(16, NCH - q0)
                        etp = pp.tile([P, 160], F32, tag="etp", bufs=1)
                        for j in range(qn):
                            ci = q0 + j
                            pe.matmul(out=etp[:, j * 10:(j + 1) * 10],
                                      lhsT=eaT_sl[:, ci * CHUNK:(ci + 1) * CHUNK],
                                      rhs=w5x10[:], start=True, stop=True)
                        act.copy(out=etc[:, s0 + q0:s0 + q0 + qn, :]
                                 .rearrange("p a b -> p (a b)"),
                                 in_=etp[:, 0:qn * 10])
                    # staircase one-hots built once, reused by all layers;
                    # built PT_AHEAD chunks ahead so the DVE cost sits in the
                    # pipeline's slack instead of its critical phase
                    pb0 = PT_AHEAD + s0
                    pb1 = min(pb0 + NCH, C)
                    for g in range(pb0, pb1, 8):
                        gn = min(8, pb1 - g)
                        vec.tensor_tensor(
                            out=pt_all[:, g:g + gn, :],
                            in0=dstr[:, g:g + gn].unsqueeze(2).to_broadcast([P, gn, BIN]),
                            in1=iotab[:].unsqueeze(1).to_broadcast([P, gn, BIN]),
                            op=ALU.is_equal)
                # alpha = b[src] (+ eterm) -> leaky relu -> exp
                AT = wp.tile([P, SSL, 8], BF16, tag="at", bufs=2)
                if PK:
                    SCT = wp.tile([P, SS, 142], BF16, tag="rhs", bufs=3)
                    BS = 0            # b slot in the packed gathered row
                else:
                    SCT = Gt
                    BS = HW
                if l > 0:
                    sl = [None, (0, 4), (4, 8), (8, 9)][l]
                    vec.tensor_tensor(out=AT[:, 0:NCH, 0:AW],
                                      in0=Gt[:, 0:NCH, BS:BS + AW],
                                      in1=etc[:, s0:s0 + NCH, sl[0]:sl[1]],
                                      op=ALU.add)
                    vec.tensor_scalar_mul(out=AT[:, 0:NCH, AW:2 * AW],
                                          in0=AT[:, 0:NCH, 0:AW], scalar1=0.2)
                    vec.tensor_tensor(out=AT[:, 0:NCH, 0:AW], in0=AT[:, 0:NCH, 0:AW],
                                      in1=AT[:, 0:NCH, AW:2 * AW], op=ALU.max)
                else:
                    vec.tensor_scalar_mul(out=AT[:, 0:NCH, AW:2 * AW],
                                          in0=Gt[:, 0:NCH, BS:BS + AW], scalar1=0.2)
                    vec.tensor_tensor(out=AT[:, 0:NCH, 0:AW],
                                      in0=Gt[:, 0:NCH, BS:BS + AW],
                                      in1=AT[:, 0:NCH, AW:2 * AW], op=ALU.max)
                act.activation(out=SCT[:, 0:NCH, HW:HW + AW], in_=AT[:, 0:NCH, 0:AW],
                               func=AF.Exp)
                if PK:
                    # h x ex: leading HX columns straight from fp8 on DVE (1x);
                    # the rest cast to bf16 on Act, then multiplied in 2x mode
                    vec.tensor_tensor(
                        out=SCT[:, 0:NCH, 0:HX].rearrange("p s (c a) -> p s c a", a=AW),
                        in0=Gt[:, 0:NCH, 4:4 + HX // 2].bitcast(FP8)
                            .rearrange("p s (c a) -> p s c a", a=AW),
                        in1=SCT[:, 0:NCH, HW:HW + AW].unsqueeze(2)
                            .to_broadcast([P, NCH, HX // AW, AW]),
                        op=ALU.mult)
                    if HX < HW:
                        act.copy(out=SCT[:, 0:NCH, HX:HW],
                                 in_=Gt[:, 0:NCH, 4 + HX // 2:4 + HW // 2].bitcast(FP8))
                        PHX = (dims.get("phx", 16) if l in (1, 2)
                               else (dims.get("phx0", 16) if l == 0 else 0))
                        DH = HW - PHX
                        vec.tensor_tensor(
                            out=SCT[:, 0:NCH, HX:DH].rearrange("p s (c a) -> p s c a", a=AW),
                            in0=SCT[:, 0:NCH, HX:DH].rearrange("p s (c a) -> p s c a", a=AW),
                            in1=SCT[:, 0:NCH, HW:HW + AW].unsqueeze(2)
                                .to_broadcast([P, NCH, (DH - HX) // AW, AW]),
                            op=ALU.mult)
                        if PHX:
                            gps.tensor_tensor(
                                out=SCT[:, 0:NCH, DH:HW].rearrange("p s (c a) -> p s c a", a=AW),
                                in0=SCT[:, 0:NCH, DH:HW].rearrange("p s (c a) -> p s c a", a=AW),
                                in1=SCT[:, 0:NCH, HW:HW + AW].unsqueeze(2)
                                    .to_broadcast([P, NCH, PHX // AW, AW]),
                                op=ALU.mult)
                else:
                    vec.tensor_tensor(
                        out=SCT[:, 0:NCH, 0:HW].rearrange("p s (c a) -> p s c a", a=AW),
                        in0=SCT[:, 0:NCH, 0:HW].rearrange("p s (c a) -> p s c a", a=AW),
                        in1=SCT[:, 0:NCH, HW:HW + AW].unsqueeze(2)
                            .to_broadcast([P, NCH, HW // AW, AW]),
                        op=ALU.mult)
                if l == 0:
                    # append eterm9|cnt as rhs cols 132:142
                    act.copy(out=SCT[:, 0:NCH, 132:142],
                             in_=etc[:, s0:s0 + NCH, :])
                # scatter matmuls
                for c_i in range(NCH):
                    gc = s0 + c_i
                    b = bin_of_chunk[gc]
                    w_ = win_of_bin[b]
                    g_ = grp_of_win[w_]
                    if g_ not in grp_tiles:
                        open_group(g_)
                    if gc == first_chunk_of_bin[b]:
                        cur_bin_tile[b] = bp.tile([BIN, 142], F32, name="binacc",
                                                  tag="binacc")
                    pe.matmul(out=cur_bin_tile[b][:, 0:RW],
                              lhsT=pt_all[:, gc, :], rhs=SCT[:, c_i, 0:RW],
                              start=(gc == first_chunk_of_bin[b]),
                              stop=(gc == last_chunk_of_bin[b]))
                    if gc == last_chunk_of_bin[b]:
                        j = b % BPW
                        wrel = w_ - grp_bounds[g_][0]
                        act.copy(out=grp_tiles[g_][BIN * j:BIN * (j + 1), wrel, 0:RW],
                                 in_=cur_bin_tile[b][:, 0:RW])
                        del cur_bin_tile[b]
                    if gc == last_chunk_of_grp.get(g_, None):
                        epilogue_group(g_)
                s0 += NCH
            # groups never triggered (e.g. all-empty windows)
            for g_ in range(NG):
                if g_ not in grp_done:
                    if g_ not in grp_tiles:
                        open_group(g_)
                    epilogue_group(g_)
            if l < 3:
                if SIM1:
                    sync.dma_start(out=T_glob[l + 1][0:NPC, :], in_=T_loc[l + 1][:, :])
                else:
                    gps.collective_compute(
                        "AllGather", ALU.bypass, replica_groups=[list(range(NCORES))],
                        ins=[T_loc[l + 1][:, :]], outs=[T_glob[l + 1][:, :]])
                T_sb = T_next

        # ============ readout
        gsum_sb = cp.tile([32, Gn], F32)
        act.copy(out=gsum_sb[:], in_=gsum_ps[:])
        sync.dma_start(out=ar_in[:], in_=gsum_sb[:])
        if SIM1:
            sync.dma_start(out=ar_out[:], in_=ar_in[:])
        else:
            gps.collective_compute("AllReduce", ALU.add,
                                   replica_groups=[list(range(NCORES))],
                                   ins=[ar_in[:]], outs=[ar_out[:]])
        gs = cp.tile([32, Gn], F32)
        sync.dma_start(out=gs[:], in_=ar_out[:])
        vec.tensor_tensor(out=comb[0:32, :], in0=gs[:, :], in1=cntrb[:],
                          op=ALU.mult)
        blt = cp.tile([1, 1], F32)
        vec.memset(blt[:], bl)
        fin = pp.tile([1, Gn], F32, tag="hps", bufs=4 - ZTPB)
        pe.matmul(out=fin[:], lhsT=wlin_sb[:], rhs=comb[:], start=True, stop=True)
        res_sb = cp.tile([1, Gn], F32)
        act.activation(out=res_sb[:], in_=fin[:], func=AF.Sigmoid, bias=blt[:])
        sync.dma_start(out=out_p[:, :], in_=res_sb[:])

    nc.finalize()
    return nc


# ------------------------------------------------------------------ entry
def _run(inputs, trace=False, debug=False):
    dims, shared, per_core = host_prep(inputs)
    nc = build_program(dims, shared)
    in_maps = [{**shared, **pc} for pc in per_core]
    from concourse.bass_utils import run_bass_kernel_spmd
    return run_bass_kernel_spmd(nc, in_maps, list(range(NCORES)), trace=trace)


def kernel(**inputs):
    res = _run(inputs)
    return res.results[0]["out"].reshape(-1).astype(np.float32)


# revision 45
# speedup vs baseline: 1.0441x; 1.0074x over previous
"""EnhancedGAT Trainium2 Bass kernel (8 NeuronCores, SPMD).

Strategy:
  - Edges are sorted by destination node on the host; core k owns dst nodes
    [k*N/8, (k+1)*N/8) and every edge targeting them. Per-core edge lists are
    bucketed into 64-node bins and padded to 128-edge chunks with a per-bin
    chunk count shared across cores (SPMD uniformity). Dummy (padding) edges
    carry dst-offset 64, which falls outside the 64-wide one-hot used by the
    scatter matmuls, so they contribute exactly nothing.
  - Each GAT layer:
      node phase: every core computes a table row [h(128) | b(4)] for its own
        nodes, where b = per-head <h, att_s + att_d> comes directly out of the
        h matmul via 4 extra weight columns W @ A. Rows live in a [NPC, 256]
        bf16 DRAM table (512B stride for the gather); an AllGather replicates
        it to every core.
      edge phase: per 4096-edge superstep one dma_gather pulls the rows for
        the edges' sources; attention coefficients alpha = b[src] (+ edge
        term) are leaky-relu'd and exp'd in place, messages h*ex are scattered
        into per-bin PSUM accumulators via one-hot matmuls. Softmax is
        unnormalized (max-subtraction skipped; alphas are O(0.3)); the divide
        happens per node at the group epilogue, where self-loop contributions
        are added. As soon as a window-group's epilogue finishes, the NEXT
        layer's node phase for those windows runs (transpose + matmul + table
        write), hiding the layer boundary behind the remaining gathers.
  - Layer 1 additionally computes, per edge, the folded edge-attention terms
    for layers 2-4 (eterm = ea @ V + be, with the padding mask folded in as a
    fifth all-ones/zeros EAT row) plus the per-edge mask into an [C,10] SBUF
    cache, and accumulates per-node mean edge-feature terms and in-degrees
    (extra scatter-matmul columns) used by the self-loops of layers 2-4.
  - Final graph mean-pool via one-hot matmuls into a [33, G] accumulator,
    AllReduce across cores, tiny dense readout replicated on every core.
"""
import sys
import numpy as np

sys.path.insert(0, "/opt/trn_rl_repo")

HID = 32
NCORES = 8
P = 128
BIN = 64
SS = 32          # chunks per superstep
CHUNK = 128
ROW = 256        # table row elements (bf16) for layers 1-3 (512B stride)
ROW4 = 128       # layer-4 table row elements


# ----------------------------------------------------------------- host prep
def host_prep(inputs):
    x = np.asarray(inputs["x"], np.float32)
    ei = np.asarray(inputs["edge_index"]).astype(np.int64)
    ea = np.asarray(inputs["edge_attr"], np.float32)
    batch = np.asarray(inputs["batch"]).astype(np.int64)
    desc = np.asarray(inputs["descriptors"], np.float32)

    N = x.shape[0]
    E = ei.shape[1]
    Gn = desc.shape[0]
    NPC = N // NCORES
    NW = -(-NPC // P)
    NBINS = -(-NPC // BIN)

    src_all, dst_all = ei[0], ei[1]
    order = np.argsort(dst_all, kind="stable")
    src_s, dst_s = src_all[order], dst_all[order]
    ea_s = ea[order]
    core_of = dst_s // NPC
    local = dst_s - core_of * NPC
    bin_of = local // BIN

    cnt = np.zeros((NCORES, NBINS), np.int64)
    np.add.at(cnt, (core_of, bin_of), 1)
    cpb = np.max(-(-cnt // CHUNK), axis=0)          # chunks per bin (shared)
    cpb = np.maximum(cpb, 1)                        # every bin gets a chunk
    C_total = int(cpb.sum())
    off = np.zeros(NBINS, np.int64)
    off[1:] = np.cumsum(cpb)[:-1]
    EP = C_total * CHUNK                            # padded edges per core

    per_core = []
    for k in range(NCORES):
        srck = np.zeros(EP, np.int64)
        dstrk = np.full(EP, float(BIN), np.float32)  # dummies -> dead one-hot
        maskk = np.zeros(EP, np.float32)
        eak = np.zeros((EP, 4), np.float32)
        sel = core_of == k
        bins_k = bin_of[sel]
        start = np.searchsorted(bins_k, np.arange(NBINS))
        pos = np.arange(bins_k.size) - start[bins_k]
        slot = off[bins_k] * CHUNK + pos
        srck[slot] = src_s[sel]
        dstrk[slot] = (local[sel] - bins_k * BIN).astype(np.float32)
        maskk[slot] = 1.0
        eak[slot] = ea_s[sel]

        # device layouts: edge e = c*128 + p
        src16 = np.tile(srck.reshape(-1, 16).T.astype(np.int16), (8, 1))
        dstr_d = dstrk.reshape(C_total, P).T.copy()
        import ml_dtypes
        ea5 = np.concatenate([eak.T, maskk[None, :]], axis=0).astype(ml_dtypes.bfloat16)

        xk = x[k * NPC:(k + 1) * NPC]
        xT = np.zeros((8, NW * P), np.float32)
        xT[:, :NPC] = xk.T
        bk = np.full(NW * P, Gn + 5, np.float32)
        bk[:NPC] = batch[k * NPC:(k + 1) * NPC].astype(np.float32)
        batch_d = bk.reshape(NW, P).T.copy()

        per_core.append(dict(SRC16=src16, DSTR=dstr_d, EAT=ea5,
                             XT=xT, BATCH=batch_d))

    # ---- weight folding
    w = {k: np.asarray(v, np.float32) for k, v in inputs.items()
         if k not in ("x", "edge_index", "edge_attr", "batch", "descriptors")}

    def vfold(We, ae, heads):
        Vp = (We.reshape(w["We_enc"].shape[1], heads, HID) * ae[None]).sum(-1)
        return w["We_enc"] @ Vp, w["be_enc"] @ Vp      # [4,heads],[heads]

    V2, bv2 = vfold(w["We2"], w["ae2"], 4)
    V3, bv3 = vfold(w["We3"], w["ae3"], 4)
    V4, bv4 = vfold(w["We4"], w["ae4"], 1)
    # [5,10]: rows = 4 edge-attr dims + mask; cols = 9 eterms + cnt
    W5x10 = np.zeros((5, 10), np.float32)
    W5x10[0:4, 0:9] = np.concatenate([V2, V3, V4], axis=1)
    W5x10[4, 0:9] = np.concatenate([bv2, bv3, bv4])
    W5x10[4, 9] = 1.0

    def padr(v, n):
        o = np.zeros(n, np.float32)
        o[: v.size] = v
        return o

    # channel-major reorder of the 128-wide (4 heads x 32 ch) dimension:
    # new position c*4+a holds old a*32+c. Keeps per-head broadcasts
    # innermost-packed on DVE (2x mode).
    cm = (np.arange(128) % 4) * 32 + np.arange(128) // 4

    def wext(W, att_s, att_d, heads):
        # append per-head b-columns: b_a = h . (att_s+att_d)_a
        att = (att_s + att_d).reshape(-1)  # [heads*HID] head-major
        if heads == 4:
            attc = att[cm]                 # channel-major to match W cols
            A = np.zeros((128, 4), np.float32)
            A[np.arange(128), np.arange(128) % 4] = attc
        else:
            A = att[:, None]               # [32,1]
        return np.concatenate([W, W @ A], axis=1)

    W1e = wext(w["W1"][:, cm], w["as1"], w["ad1"], 4)            # [8,132]
    W2e = wext(w["W2"][cm][:, cm], w["as2"], w["ad2"], 4)        # [128,132]
    W3e = wext(w["W3"][cm][:, cm], w["as3"], w["ad3"], 4)
    W4e = wext(w["W4"][cm], w["as4"], w["ad4"], 1)               # [128,33]

    bout = np.stack([padr(w["b1"][cm], 128), padr(w["b2"][cm], 128),
                     padr(w["b3"][cm], 128), padr(w["b4"], 128)])

    import ml_dtypes
    T0h = (x @ W1e).astype(np.float32)
    pk0 = np.zeros((N, 256), np.uint8)
    pk0[:, 0:8] = T0h[:, 128:132].astype(ml_dtypes.bfloat16).view(np.uint8)
    pk0[:, 8:136] = T0h[:, 0:128].astype(ml_dtypes.float8_e4m3).view(np.uint8)
    TG0 = pk0.view(ml_dtypes.bfloat16)

    gcnt = np.bincount(batch, minlength=Gn).astype(np.float32)
    cntr = (1.0 / np.maximum(gcnt, 1.0))[None, :]           # [1, Gn]
    shared = dict(
        W1=W1e, WL2=W2e, WL3=W3e, WL4=W4e, TG0=TG0,
        W5X10=W5x10, BOUT=bout, CNTR=cntr,
        WD=w["Wd"], BD=w["bd"][:, None], WLIN=w["Wl"], DESCT=desc.T.copy(),
    )
    bl = float(np.asarray(w["bl"]).reshape(-1)[0])

    dims = dict(N=N, E=E, Gn=Gn, NPC=NPC, NW=NW, NBINS=NBINS,
                C=C_total, cpb=cpb, off=off, bl=bl)
    return dims, shared, per_core


# ------------------------------------------------------------- program build
def build_program(dims, shared):
    import concourse.bass as bass
    import concourse.mybir as mybir
    import concourse.tile as tile
    import concourse.bacc as bacc
    from concourse.masks import make_identity
    from contextlib import ExitStack

    F32 = mybir.dt.float32
    FP8 = mybir.dt.float8e4
    BF16 = mybir.dt.bfloat16
    I32 = mybir.dt.int32
    I16 = mybir.dt.int16
    AF = mybir.ActivationFunctionType
    ALU = mybir.AluOpType
    AX = mybir.AxisListType

    N, Gn, NPC, NW, NBINS, C = (dims[k] for k in ("N", "Gn", "NPC", "NW", "NBINS", "C"))
    cpb, off, bl = dims["cpb"], dims["off"], dims["bl"]
    NSS = C // SS
    # layer params: h width, heads, rhs width, gather row elems
    # PK tables pack rows as [b bf16 x4 | h fp8 x128] (136B) in a 256B stride;
    # HX = leading h-columns multiplied on DVE straight from fp8 (1x mode), the
    # rest is cast to bf16 on Act first so the DVE part runs in 2x mode.
    HXD = dims.get("hx", {0: 64, 1: 48, 2: 48})
    PKL = dims.get("pk_layers", (0, 1, 2))
    LP = [dict(HW=128, AW=4, RW=142, EL=128 if 0 in PKL else ROW,
               GEL=68 if 0 in PKL else 132, PK=0 in PKL, HX=HXD[0]),
          dict(HW=128, AW=4, RW=132, EL=128, GEL=68, PK=1 in PKL, HX=HXD[1]),
          dict(HW=128, AW=4, RW=132, EL=128, GEL=68, PK=2 in PKL, HX=HXD[2]),
          dict(HW=32, AW=1, RW=33, EL=ROW4, GEL=34, PK=False, HX=32)]
    for l_ in (1, 2):
        if not LP[l_]["PK"]:
            LP[l_].update(EL=ROW, GEL=ROW)

    nc = bacc.Bacc(num_swdge_queues=2)
    SIM1 = dims.get("sim1", False)

    # ---- params
    pr = {}
    for nm, shp, dt in [("SRC16", [P, C * 8], I16), ("DSTR", [P, C], F32),
                        ("EAT", [5, C * CHUNK], BF16), ("XT", [8, NW * P], F32),
                        ("BATCH", [P, NW], F32), ("W1", [8, 132], F32),
                        ("WL2", [128, 132], F32), ("WL3", [128, 132], F32),
                        ("WL4", [128, 33], F32), ("W5X10", [5, 10], F32),
                        ("BOUT", [4, 128], F32),
                        ("WD", [48, 32], F32), ("BD", [32, 1], F32),
                        ("WLIN", [64, 1], F32), ("DESCT", [48, Gn], F32),
                        ("CNTR", [1, Gn], F32), ("TG0", [N, 128], BF16)]:
        pr[nm] = nc.declare_dram_parameter(nm, shp, dt, isOutput=False)
    out_p = nc.declare_dram_parameter("out", [1, Gn], F32, isOutput=True)
    pr_TG0_ph = pr["TG0"]

    # ---- internal DRAM
    T_loc = [None] + [nc.dram_tensor(f"T_loc{l}", [NPC, LP[l]["EL"]], BF16)
                      for l in range(1, 4)]
    T_glob = [pr_TG0_ph] + [nc.dram_tensor(f"T_glob{l}", [N, LP[l]["EL"]], BF16,
                                           addr_space="Shared")
                            for l in range(1, 4)]
    ar_in = nc.dram_tensor("ar_in", [32, Gn], F32)
    ar_out = nc.dram_tensor("ar_out", [32, Gn], F32, addr_space="Shared")

    # bin/window bookkeeping (compile-time)
    bin_of_chunk = []
    for b in range(NBINS):
        bin_of_chunk += [b] * int(cpb[b])
    BPW = P // BIN  # bins per window
    win_of_bin = [b // BPW for b in range(NBINS)]
    last_chunk_of_bin = {}
    first_chunk_of_bin = {}
    for c_i, b in enumerate(bin_of_chunk):
        last_chunk_of_bin[b] = c_i
        first_chunk_of_bin.setdefault(b, c_i)

    with tile.TileContext(nc) as tc, ExitStack() as ctx:
        cp = ctx.enter_context(tc.tile_pool(name="const", bufs=1))
        wp = ctx.enter_context(tc.tile_pool(name="work", bufs=2))
        vp = ctx.enter_context(tc.tile_pool(name="win", bufs=dims.get("vpb", 2)))
        pp = ctx.enter_context(tc.tile_pool(name="psum", bufs=2, space="PSUM"))
        bp = ctx.enter_context(tc.tile_pool(name="binp", bufs=2, space="PSUM"))

        sync, gps, vec, act, pe = nc.sync, nc.gpsimd, nc.vector, nc.scalar, nc.tensor

        def dma_gather_short(out_ap, in_ap, idxs_ap, num_idxs, elem_size,
                             elem_step, queue_num):
            from concourse.bass import exact_div
            eng = gps
            _in_ap = eng.lower_ap_dma(in_ap, for_custom_bir_dma=True)
            _idxs_ap = eng.lower_ap(idxs_ap)
            _out_ap = eng.lower_ap(out_ap)
            stride_bytes_256 = exact_div(elem_step * 2, 256)
            return eng.add_instruction(
                mybir.InstDMAGatherAnt(
                    name=eng.bass.get_next_instruction_name(),
                    ins=[*_in_ap, _idxs_ap,
                         eng.lower_val_access(eng.to_reg(num_idxs))],
                    outs=[_out_ap],
                    transpose=False, num_idxs=num_idxs, elem_size=elem_size,
                    stride_bytes_256=stride_bytes_256, gen_mode=0,
                    single_packet=False, queue_num=queue_num,
                    sbuf_tokens_per_rank=0, sbuf_free_dim_per_rank=0,
                    sbuf_free_dim_pad_per_rank=0, sbuf_byte_offset=0))
        ZTPB = dims.get("ztpb", 1)

        # ---- resident tiles
        src16 = cp.tile([P, C * 8], I16)
        sync.dma_start(out=src16[:], in_=pr["SRC16"][:, :])
        dstr = cp.tile([P, C], BF16)
        gps.dma_start(out=dstr[:], in_=pr["DSTR"][:, :])   # f32 -> bf16 cast
        batcht = cp.tile([P, NW], F32)
        sync.dma_start(out=batcht[:], in_=pr["BATCH"][:, :])
        xT_sb = cp.tile([8, NW * P], BF16)
        gps.dma_start(out=xT_sb[:], in_=pr["XT"][:, :])

        iota_i = cp.tile([P, BIN], I32)
        gps.iota(iota_i[:], pattern=[[1, BIN]], base=0, channel_multiplier=0)
        iotab = cp.tile([P, BIN], BF16)
        vec.tensor_copy(iotab[:], iota_i[:])
        iotag_i = cp.tile([P, Gn], I32)
        gps.iota(iotag_i[:], pattern=[[1, Gn]], base=0, channel_multiplier=0)
        iotagf = cp.tile([P, Gn], F32)
        vec.tensor_copy(iotagf[:], iotag_i[:])
        identf = cp.tile([P, P], F32)
        make_identity(nc, identf[:])

        w1_sb = cp.tile([8, 132], BF16)
        gps.dma_start(out=w1_sb[:], in_=pr["W1"][:, :])
        wl_sb = [None,
                 cp.tile([128, 132], BF16, name="wl2", tag="wl2"),
                 cp.tile([128, 132], BF16, name="wl3", tag="wl3"),
                 cp.tile([128, 33], BF16, name="wl4", tag="wl4")]
        gps.dma_start(out=wl_sb[1][:], in_=pr["WL2"][:, :])   # gpsimd casts f32->bf16
        gps.dma_start(out=wl_sb[2][:], in_=pr["WL3"][:, :])
        gps.dma_start(out=wl_sb[3][:], in_=pr["WL4"][:, :])
        w5x10 = cp.tile([5, 10], BF16)
        gps.dma_start(out=w5x10[:], in_=pr["W5X10"][:, :])
        bout_t = []
        for l in range(4):
            t3 = cp.tile([P, 128], F32, tag=f"bout{l}")
            sync.dma_start(out=t3[:], in_=pr["BOUT"][l:l + 1, :].to_broadcast([P, 128]))
            bout_t.append(t3)

        etc = cp.tile([P, C, 10], BF16)      # eterm9 | cnt  per edge
        pt_all = cp.tile([P, C, BIN], BF16)  # one-hot dst rows per edge
        loop_sb = cp.tile([P, NW, 10], F32)
        gsp = ctx.enter_context(tc.tile_pool(name="gsp", bufs=1, space="PSUM"))
        eap = ctx.enter_context(tc.tile_pool(name="eap", bufs=1))
        gsum_ps = None
        n_pool_mm = [0]

        # ---- readout head start: descriptor branch is input-independent
        comb = cp.tile([64, Gn], F32)
        wd_sb = cp.tile([48, 32], F32)
        sync.dma_start(out=wd_sb[:], in_=pr["WD"][:, :])
        desct_sb = cp.tile([48, Gn], F32)
        sync.dma_start(out=desct_sb[:], in_=pr["DESCT"][:, :])
        bd_sb = cp.tile([32, 1], F32)
        sync.dma_start(out=bd_sb[:], in_=pr["BD"][:, :])
        dps = pp.tile([32, Gn], F32, tag="hps", bufs=dims.get("hpsb", 2))
        pe.matmul(out=dps[:], lhsT=wd_sb[:], rhs=desct_sb[:], start=True, stop=True)
        act.activation(out=comb[32:64, :], in_=dps[:], func=AF.Relu, bias=bd_sb[:])
        wlin_sb = cp.tile([64, 1], F32)
        sync.dma_start(out=wlin_sb[:], in_=pr["WLIN"][:, :])
        cntrb = cp.tile([32, Gn], F32)
        sync.dma_start(out=cntrb[:], in_=pr["CNTR"][0:1, :].to_broadcast([32, Gn]))

        WG = dims.get("wg", 5)  # max windows per epilogue group
        # non-uniform groups: taper toward the end so the serial layer-boundary
        # tail (last epilogue -> node phase -> AllGather) shrinks
        grp_bounds = []
        w0_ = 0
        while NW - w0_ > 10:
            grp_bounds.append((w0_, WG))
            w0_ += WG
        for t_ in dims.get("taper", (4, 3, 2, 1)):
            if NW - w0_ > t_:
                grp_bounds.append((w0_, t_))
                w0_ += t_
        if NW > w0_:
            grp_bounds.append((w0_, NW - w0_))
        NG = len(grp_bounds)
        grp_of_win = {}
        for gi, (gw0, gsz_) in enumerate(grp_bounds):
            for w_ in range(gw0, gw0 + gsz_):
                grp_of_win[w_] = gi
        last_chunk_of_grp = {}
        for b in range(NBINS):
            g_ = grp_of_win[win_of_bin[b]]
            last_chunk_of_grp[g_] = max(last_chunk_of_grp.get(g_, -1),
                                        last_chunk_of_bin[b])

        # T_sb tables: [P, NW, 132] (h | b); layer l+1's is built during
        # layer l's edge phase, group by group.
        def node_phase_group(l, g_, T_next, z_src):
            """Build T_next rows for group g_ of layer l (0-based), write T_loc."""
            w0, gsz = grp_bounds[g_]
            HWn = LP[l]["HW"]
            BW = HWn + LP[l]["AW"]  # table row width
            for w_ in range(w0, w0 + gsz):
                if l == 0:
                    hps = pp.tile([P, 132], F32, tag="hps", bufs=dims.get("hpsb", 2))
                    pe.matmul(out=hps[:, 0:BW], lhsT=xT_sb[:, w_ * P:(w_ + 1) * P],
                              rhs=w1_sb[:], start=True, stop=True)
                else:
                    ztp = pp.tile([P, P], F32, tag="ztp", bufs=ZTPB)
                    pe.transpose(out=ztp[:], in_=z_src[:, w_ - w0, 0:128],
                                 identity=identf[:])
                    zt_sb = wp.tile([P, P], BF16, tag="ztsb")
                    act.copy(out=zt_sb[:], in_=ztp[:])
                    hps = pp.tile([P, 132], F32, tag="hps", bufs=dims.get("hpsb", 2))
                    pe.matmul(out=hps[:, 0:BW], lhsT=zt_sb[:], rhs=wl_sb[l][:],
                              start=True, stop=True)
                act.copy(out=T_next[:, w_, 0:BW], in_=hps[:, 0:BW])
                if l > 0 and LP[l]["PK"]:
                    act.copy(out=Tpk[:, w_, 0:4], in_=hps[:, 128:132])
                    act.copy(out=Tpk[:, w_, 4:68].bitcast(FP8), in_=hps[:, 0:128])
            if l == 0:
                return  # layer-1 table ships as the TG0 param; SBUF copy only
            stage, SW = (Tpk, 68) if LP[l]["PK"] else (T_next, BW)
            # batched table write: full windows in one DMA, ragged tail apart
            wfull = gsz - (1 if (w0 + gsz) * P > NPC else 0)
            if wfull > 0:
                sync.dma_start(
                    out=T_loc[l][w0 * P:(w0 + wfull) * P, 0:SW]
                        .rearrange("(w p) e -> p w e", p=P),
                    in_=stage[:, w0:w0 + wfull, 0:SW])
            if wfull < gsz:
                w_ = w0 + wfull
                nr = NPC - w_ * P
                sync.dma_start(out=T_loc[l][w_ * P:w_ * P + nr, 0:SW],
                               in_=stage[0:nr, w_, 0:SW])

        PT_AHEAD = dims.get("pt_ahead", 64)
        # prebuild the one-hot cache for the first chunks while the layer-0
        # node phase occupies PE/Act
        for g in range(0, PT_AHEAD, 8):
            vec.tensor_tensor(
                out=pt_all[:, g:g + 8, :],
                in0=dstr[:, g:g + 8].unsqueeze(2).to_broadcast([P, 8, BIN]),
                in1=iotab[:].unsqueeze(1).to_broadcast([P, 8, BIN]),
                op=ALU.is_equal)

        # ---- layer 0 node phase (all groups up front)
        T_sb = wp.tile([P, NW, 132], BF16, tag="tsb")
        for g_ in range(NG):
            node_phase_group(0, g_, T_sb, None)

        for l in range(4):
            HW, AW, RW, EL, GEL, PK, HX = (
                LP[l][k] for k in ("HW", "AW", "RW", "EL", "GEL", "PK", "HX"))
            BW = HW + AW

            T_next = None
            if l < 3:
                T_next = wp.tile([P, NW, 132], BF16, name="tnext", tag="tsb")
                if LP[l + 1]["PK"]:
                    Tpk = wp.tile([P, NW, 68], BF16, name="tpk", tag="tpk", bufs=1)

            grp_tiles = {}
            grp_done = set()

            def open_group(g_):
                t = vp.tile([P, WG, 142], F32, name="wingrp", tag="wingrp")
                grp_tiles[g_] = t
                return t

            def epilogue_group(g_):
                w0, gsz = grp_bounds[g_]
                wg = grp_tiles[g_]
                scr = wp.tile([P, WG, 12], F32, name="scr", tag="scr")
                # self-loop alpha (= b_own [+ eterm means]) -> exp
                if l > 0:
                    sl = [None, (0, 4), (4, 8), (8, 9)][l]
                    vec.tensor_tensor(out=scr[:, 0:gsz, 0:AW],
                                      in0=T_sb[:, w0:w0 + gsz, HW:HW + AW],
                                      in1=loop_sb[:, w0:w0 + gsz, sl[0]:sl[1]],
                                      op=ALU.add)
                else:
                    act.copy(out=scr[:, 0:gsz, 0:AW],
                             in_=T_sb[:, w0:w0 + gsz, HW:HW + AW])
                vec.tensor_scalar_mul(out=scr[:, 0:gsz, 4:4 + AW],
                                      in0=scr[:, 0:gsz, 0:AW], scalar1=0.2)
                vec.tensor_tensor(out=scr[:, 0:gsz, 0:AW], in0=scr[:, 0:gsz, 0:AW],
                                  in1=scr[:, 0:gsz, 4:4 + AW], op=ALU.max)
                act.activation(out=scr[:, 0:gsz, 0:AW], in_=scr[:, 0:gsz, 0:AW],
                               func=AF.Exp)
                # num += h_own * ex_loop
                nt = wp.tile([P, WG, 128], BF16, name="nt", tag="nt")
                vec.tensor_tensor(
                    out=nt[:, 0:gsz, 0:HW].rearrange("p g (c a) -> p g c a", a=AW),
                    in0=T_sb[:, w0:w0 + gsz, 0:HW].rearrange("p g (c a) -> p g c a", a=AW),
                    in1=scr[:, 0:gsz, 0:AW].unsqueeze(2)
                        .to_broadcast([P, gsz, HW // AW, AW]),
                    op=ALU.mult)
                vec.tensor_tensor(out=wg[:, 0:gsz, 0:HW], in0=wg[:, 0:gsz, 0:HW],
                                  in1=nt[:, 0:gsz, 0:HW], op=ALU.add)
                # den -> reciprocal
                vec.tensor_tensor(out=scr[:, 0:gsz, 4:4 + AW],
                                  in0=wg[:, 0:gsz, HW:HW + AW],
                                  in1=scr[:, 0:gsz, 0:AW], op=ALU.add)
                vec.tensor_scalar_add(out=scr[:, 0:gsz, 4:4 + AW],
                                      in0=scr[:, 0:gsz, 4:4 + AW], scalar1=1e-16)
                vec.reciprocal(out=scr[:, 0:gsz, 4:4 + AW], in_=scr[:, 0:gsz, 4:4 + AW])
                if l == 0:
                    vec.tensor_scalar_max(out=scr[:, 0:gsz, 8:9],
                                          in0=wg[:, 0:gsz, 141:142], scalar1=1.0)
                    vec.reciprocal(out=scr[:, 0:gsz, 8:9], in_=scr[:, 0:gsz, 8:9])
                    vec.tensor_tensor(
                        out=loop_sb[:, w0:w0 + gsz, 0:9], in0=wg[:, 0:gsz, 132:141],
                        in1=scr[:, 0:gsz, 8:9].to_broadcast([P, gsz, 9]), op=ALU.mult)
                # z = num * recip(den) + bias [+ relu]
                vec.tensor_tensor(
                    out=wg[:, 0:gsz, 0:HW].rearrange("p g (c a) -> p g c a", a=AW),
                    in0=wg[:, 0:gsz, 0:HW].rearrange("p g (c a) -> p g c a", a=AW),
                    in1=scr[:, 0:gsz, 4:4 + AW].unsqueeze(2)
                        .to_broadcast([P, gsz, HW // AW, AW]),
                    op=ALU.mult)
                vec.tensor_tensor(
                    out=wg[:, 0:gsz, 0:HW], in0=wg[:, 0:gsz, 0:HW],
                    in1=bout_t[l][:, 0:HW].unsqueeze(1).to_broadcast([P, gsz, HW]),
                    op=ALU.add)
                if l < 3:
                    act.activation(out=wg[:, 0:gsz, 0:128], in_=wg[:, 0:gsz, 0:128],
                                   func=AF.Relu)
                    node_phase_group(l + 1, g_, T_next, wg)
                else:
                    nonlocal gsum_ps
                    pool_sb = wp.tile([P, WG, 32], BF16, name="pool_sb", tag="poolsb")
                    act.copy(out=pool_sb[:, 0:gsz, 0:32], in_=wg[:, 0:gsz, 0:32])
                    bt = wp.tile([P, WG, Gn], BF16, name="bt", tag="bt", bufs=1)
                    vec.tensor_tensor(
                        out=bt[:, 0:gsz, :],
                        in0=batcht[:, w0:w0 + gsz].unsqueeze(2).to_broadcast([P, gsz, Gn]),
                        in1=iotagf[:].unsqueeze(1).to_broadcast([P, gsz, Gn]),
                        op=ALU.is_equal)
                    if gsum_ps is None:
                        gsum_ps = gsp.tile([32, Gn], F32, name="gsum_ps")
                    for j_ in range(gsz):
                        n_pool_mm[0] += 1
                        pe.matmul(out=gsum_ps[:], lhsT=pool_sb[:, j_, :],
                                  rhs=bt[:, j_, :],
                                  start=(n_pool_mm[0] == 1),
                                  stop=(n_pool_mm[0] == NW))
                grp_done.add(g_)

            cur_bin_tile = {}
            SSL = dims.get("ss4", 96) if l == 3 else SS
            ss_plan = []
            rem_ = C
            for n_ in dims.get("ss_head", ()):
                if rem_ > n_ + SSL:
                    ss_plan.append(n_)
                    rem_ -= n_
            while rem_ > 0:
                n_ = min(SSL, rem_)
                ss_plan.append(n_)
                rem_ -= n_
            for t_ in dims.get("ss_tail", (12,)):
                if ss_plan[-1] > t_:
                    ss_plan[-1] -= t_
                    ss_plan.append(t_)
            s0 = 0
            GW = GEL if PK else max(GEL, RW)
            for ss, NCH in enumerate(ss_plan):
                Gt = wp.tile([P, SSL, GW], BF16, tag="gt" if GW > 68 else "gtp", bufs=4)
                if GEL == EL:
                    gps.dma_gather(
                        out_ap=Gt[:, 0:NCH, 0:GEL], in_ap=T_glob[l][:, :],
                        idxs_ap=src16[:, s0 * 8:(s0 + NCH) * 8],
                        num_idxs=NCH * CHUNK, num_idxs_reg=NCH * CHUNK,
                        elem_size=EL, single_packet=False, queue_num=ss % 2)
                else:
                    dma_gather_short(
                        out_ap=Gt[:, 0:NCH, 0:GEL], in_ap=T_glob[l][:, 0:GEL],
                        idxs_ap=src16[:, s0 * 8:(s0 + NCH) * 8],
                        num_idxs=NCH * CHUNK, elem_size=GEL, elem_step=EL,
                        queue_num=ss % 2)
                if l == 0:
                    # edge-term + mask precompute (feeds rhs cols 132:142 +
                    # later layers' alpha); mask folded into EAT row 5.
                    eaT_sl = eap.tile([5, SS * CHUNK], BF16, name="easl", tag="eat")
                    sync.dma_start(
                        out=eaT_sl[:, 0:NCH * CHUNK],
                        in_=pr["EAT"][:, s0 * CHUNK:(s0 + NCH) * CHUNK])
                    for q0 in range(0, NCH, 16):
                        qn = min(16, NCH - q0)
                        etp = pp.tile([P, 160], F32, tag="etp", bufs=dims.get("etpb", 2))
                        for j in range(qn):
                            ci = q0 + j
                            pe.matmul(out=etp[:, j * 10:(j + 1) * 10],
                                      lhsT=eaT_sl[:, ci * CHUNK:(ci + 1) * CHUNK],
                                      rhs=w5x10[:], start=True, stop=True)
                        act.copy(out=etc[:, s0 + q0:s0 + q0 + qn, :]
                                 .rearrange("p a b -> p (a b)"),
                                 in_=etp[:, 0:qn * 10])
                    # staircase one-hots built once, reused by all layers;
                    # built PT_AHEAD chunks ahead so the DVE cost sits in the
                    # pipeline's slack instead of its critical phase
                    pb0 = PT_AHEAD + s0
                    pb1 = min(pb0 + NCH, C)
                    for g in range(pb0, pb1, 8):
                        gn = min(8, pb1 - g)
                        vec.tensor_tensor(
                            out=pt_all[:, g:g + gn, :],
                            in0=dstr[:, g:g + gn].unsqueeze(2).to_broadcast([P, gn, BIN]),
                            in1=iotab[:].unsqueeze(1).to_broadcast([P, gn, BIN]),
                            op=ALU.is_equal)
                # alpha = b[src] (+ eterm) -> leaky relu -> exp
                AT = wp.tile([P, SSL, 8], BF16, tag="at", bufs=dims.get("atb", 2))
                if PK:
                    SCT = wp.tile([P, SS, 142], BF16, tag="rhs", bufs=dims.get("rhsb", 3))
                    BS = 0            # b slot in the packed gathered row
                else:
                    SCT = Gt
                    BS = HW
                if l > 0:
                    sl = [None, (0, 4), (4, 8), (8, 9)][l]
                    vec.tensor_tensor(out=AT[:, 0:NCH, 0:AW],
                                      in0=Gt[:, 0:NCH, BS:BS + AW],
                                      in1=etc[:, s0:s0 + NCH, sl[0]:sl[1]],
                                      op=ALU.add)
                    vec.tensor_scalar_mul(out=AT[:, 0:NCH, AW:2 * AW],
                                          in0=AT[:, 0:NCH, 0:AW], scalar1=0.2)
                    vec.tensor_tensor(out=AT[:, 0:NCH, 0:AW], in0=AT[:, 0:NCH, 0:AW],
                                      in1=AT[:, 0:NCH, AW:2 * AW], op=ALU.max)
                else:
                    vec.tensor_scalar_mul(out=AT[:, 0:NCH, AW:2 * AW],
                                          in0=Gt[:, 0:NCH, BS:BS + AW], scalar1=0.2)
                    vec.tensor_tensor(out=AT[:, 0:NCH, 0:AW],
                                      in0=Gt[:, 0:NCH, BS:BS + AW],
                                      in1=AT[:, 0:NCH, AW:2 * AW], op=ALU.max)
                act.activation(out=SCT[:, 0:NCH, HW:HW + AW], in_=AT[:, 0:NCH, 0:AW],
                               func=AF.Exp)
                if PK:
                    # h x ex: leading HX columns straight from fp8 on DVE (1x);
                    # the rest cast to bf16 on Act, then multiplied in 2x mode
                    vec.tensor_tensor(
                        out=SCT[:, 0:NCH, 0:HX].rearrange("p s (c a) -> p s c a", a=AW),
                        in0=Gt[:, 0:NCH, 4:4 + HX // 2].bitcast(FP8)
                            .rearrange("p s (c a) -> p s c a", a=AW),
                        in1=SCT[:, 0:NCH, HW:HW + AW].unsqueeze(2)
                            .to_broadcast([P, NCH, HX // AW, AW]),
                        op=ALU.mult)
                    if HX < HW:
                        act.copy(out=SCT[:, 0:NCH, HX:HW],
                                 in_=Gt[:, 0:NCH, 4 + HX // 2:4 + HW // 2].bitcast(FP8))
                        PHX = (dims.get("phx", 16) if l in (1, 2)
                               else (dims.get("phx0", 16) if l == 0 else 0))
                        DH = HW - PHX
                        vec.tensor_tensor(
                            out=SCT[:, 0:NCH, HX:DH].rearrange("p s (c a) -> p s c a", a=AW),
                            in0=SCT[:, 0:NCH, HX:DH].rearrange("p s (c a) -> p s c a", a=AW),
                            in1=SCT[:, 0:NCH, HW:HW + AW].unsqueeze(2)
                                .to_broadcast([P, NCH, (DH - HX) // AW, AW]),
                            op=ALU.mult)
                        if PHX:
                            gps.tensor_tensor(
                                out=SCT[:, 0:NCH, DH:HW].rearrange("p s (c a) -> p s c a", a=AW),
                                in0=SCT[:, 0:NCH, DH:HW].rearrange("p s (c a) -> p s c a", a=AW),
                                in1=SCT[:, 0:NCH, HW:HW + AW].unsqueeze(2)
                                    .to_broadcast([P, NCH, PHX // AW, AW]),
                                op=ALU.mult)
                else:
                    vec.tensor_tensor(
                        out=SCT[:, 0:NCH, 0:HW].rearrange("p s (c a) -> p s c a", a=AW),
                        in0=SCT[:, 0:NCH, 0:HW].rearrange("p s (c a) -> p s c a", a=AW),
                        in1=SCT[:, 0:NCH, HW:HW + AW].unsqueeze(2)
                            .to_broadcast([P, NCH, HW // AW, AW]),
                        op=ALU.mult)
                if l == 0:
                    # append eterm9|cnt as rhs cols 132:142
                    act.copy(out=SCT[:, 0:NCH, 132:142],
                             in_=etc[:, s0:s0 + NCH, :])
                # scatter matmuls
                for c_i in range(NCH):
                    gc = s0 + c_i
                    b = bin_of_chunk[gc]
                    w_ = win_of_bin[b]
                    g_ = grp_of_win[w_]
                    if g_ not in grp_tiles:
                        open_group(g_)
                    if gc == first_chunk_of_bin[b]:
                        cur_bin_tile[b] = bp.tile([BIN, 142], F32, name="binacc",
                                                  tag="binacc")
                    pe.matmul(out=cur_bin_tile[b][:, 0:RW],
                              lhsT=pt_all[:, gc, :], rhs=SCT[:, c_i, 0:RW],
                              start=(gc == first_chunk_of_bin[b]),
                              stop=(gc == last_chunk_of_bin[b]))
                    if gc == last_chunk_of_bin[b]:
                        j = b % BPW
                        wrel = w_ - grp_bounds[g_][0]
                        act.copy(out=grp_tiles[g_][BIN * j:BIN * (j + 1), wrel, 0:RW],
                                 in_=cur_bin_tile[b][:, 0:RW])
                        del cur_bin_tile[b]
                    if gc == last_chunk_of_grp.get(g_, None):
                        epilogue_group(g_)
                s0 += NCH
            # groups never triggered (e.g. all-empty windows)
            for g_ in range(NG):
                if g_ not in grp_done:
                    if g_ not in grp_tiles:
                        open_group(g_)
                    epilogue_group(g_)
            if l < 3:
                if SIM1:
                    sync.dma_start(out=T_glob[l + 1][0:NPC, :], in_=T_loc[l + 1][:, :])
                else:
                    gps.collective_compute(
                        "AllGather", ALU.bypass, replica_groups=[list(range(NCORES))],
                        ins=[T_loc[l + 1][:, :]], outs=[T_glob[l + 1][:, :]])
                T_sb = T_next

        # ============ readout
        gsum_sb = cp.tile([32, Gn], F32)
        act.copy(out=gsum_sb[:], in_=gsum_ps[:])
        sync.dma_start(out=ar_in[:], in_=gsum_sb[:])
        if SIM1:
            sync.dma_start(out=ar_out[:], in_=ar_in[:])
        else:
            gps.collective_compute("AllReduce", ALU.add,
                                   replica_groups=[list(range(NCORES))],
                                   ins=[ar_in[:]], outs=[ar_out[:]])
        gs = cp.tile([32, Gn], F32)
        sync.dma_start(out=gs[:], in_=ar_out[:])
        vec.tensor_tensor(out=comb[0:32, :], in0=gs[:, :], in1=cntrb[:],
                          op=ALU.mult)
        blt = cp.tile([1, 1], F32)
        vec.memset(blt[:], bl)
        fin = pp.tile([1, Gn], F32, tag="hps", bufs=dims.get("hpsb", 2))
        pe.matmul(out=fin[:], lhsT=wlin_sb[:], rhs=comb[:], start=True, stop=True)
        res_sb = cp.tile([1, Gn], F32)
        act.activation(out=res_sb[:], in_=fin[:], func=AF.Sigmoid, bias=blt[:])
        sync.dma_start(out=out_p[:, :], in_=res_sb[:])

    nc.finalize()
    return nc


# ------------------------------------------------------------------ entry
def _run(inputs, trace=False, debug=False):
    dims, shared, per_core = host_prep(inputs)
    nc = build_program(dims, shared)
    in_maps = [{**shared, **pc} for pc in per_core]
    from concourse.bass_utils import run_bass_kernel_spmd
    return run_bass_kernel_spmd(nc, in_maps, list(range(NCORES)), trace=trace)


def kernel(**inputs):
    res = _run(inputs)
    return res.results[0]["out"].reshape(-1).astype(np.float32)


# revision 46
# speedup vs baseline: 1.0456x; 1.0015x over previous
"""EnhancedGAT Trainium2 Bass kernel (8 NeuronCores, SPMD).

Strategy:
  - Edges are sorted by destination node on the host; core k owns dst nodes
    [k*N/8, (k+1)*N/8) and every edge targeting them. Per-core edge lists are
    bucketed into 64-node bins and padded to 128-edge chunks with a per-bin
    chunk count shared across cores (SPMD uniformity). Dummy (padding) edges
    carry dst-offset 64, which falls outside the 64-wide one-hot used by the
    scatter matmuls, so they contribute exactly nothing.
  - Each GAT layer:
      node phase: every core computes a table row [h(128) | b(4)] for its own
        nodes, where b = per-head <h, att_s + att_d> comes directly out of the
        h matmul via 4 extra weight columns W @ A. Rows live in a [NPC, 256]
        bf16 DRAM table (512B stride for the gather); an AllGather replicates
        it to every core.
      edge phase: per 4096-edge superstep one dma_gather pulls the rows for
        the edges' sources; attention coefficients alpha = b[src] (+ edge
        term) are leaky-relu'd and exp'd in place, messages h*ex are scattered
        into per-bin PSUM accumulators via one-hot matmuls. Softmax is
        unnormalized (max-subtraction skipped; alphas are O(0.3)); the divide
        happens per node at the group epilogue, where self-loop contributions
        are added. As soon as a window-group's epilogue finishes, the NEXT
        layer's node phase for those windows runs (transpose + matmul + table
        write), hiding the layer boundary behind the remaining gathers.
  - Layer 1 additionally computes, per edge, the folded edge-attention terms
    for layers 2-4 (eterm = ea @ V + be, with the padding mask folded in as a
    fifth all-ones/zeros EAT row) plus the per-edge mask into an [C,10] SBUF
    cache, and accumulates per-node mean edge-feature terms and in-degrees
    (extra scatter-matmul columns) used by the self-loops of layers 2-4.
  - Final graph mean-pool via one-hot matmuls into a [33, G] accumulator,
    AllReduce across cores, tiny dense readout replicated on every core.
"""
import sys
import numpy as np

sys.path.insert(0, "/opt/trn_rl_repo")

HID = 32
NCORES = 8
P = 128
BIN = 64
SS = 32          # chunks per superstep
CHUNK = 128
ROW = 256        # table row elements (bf16) for layers 1-3 (512B stride)
ROW4 = 128       # layer-4 table row elements


# ----------------------------------------------------------------- host prep
def host_prep(inputs):
    x = np.asarray(inputs["x"], np.float32)
    ei = np.asarray(inputs["edge_index"]).astype(np.int64)
    ea = np.asarray(inputs["edge_attr"], np.float32)
    batch = np.asarray(inputs["batch"]).astype(np.int64)
    desc = np.asarray(inputs["descriptors"], np.float32)

    N = x.shape[0]
    E = ei.shape[1]
    Gn = desc.shape[0]
    NPC = N // NCORES
    NW = -(-NPC // P)
    NBINS = -(-NPC // BIN)

    src_all, dst_all = ei[0], ei[1]
    order = np.argsort(dst_all, kind="stable")
    src_s, dst_s = src_all[order], dst_all[order]
    ea_s = ea[order]
    core_of = dst_s // NPC
    local = dst_s - core_of * NPC
    bin_of = local // BIN

    cnt = np.zeros((NCORES, NBINS), np.int64)
    np.add.at(cnt, (core_of, bin_of), 1)
    cpb = np.max(-(-cnt // CHUNK), axis=0)          # chunks per bin (shared)
    cpb = np.maximum(cpb, 1)                        # every bin gets a chunk
    C_total = int(cpb.sum())
    off = np.zeros(NBINS, np.int64)
    off[1:] = np.cumsum(cpb)[:-1]
    EP = C_total * CHUNK                            # padded edges per core

    per_core = []
    for k in range(NCORES):
        srck = np.zeros(EP, np.int64)
        dstrk = np.full(EP, float(BIN), np.float32)  # dummies -> dead one-hot
        maskk = np.zeros(EP, np.float32)
        eak = np.zeros((EP, 4), np.float32)
        sel = core_of == k
        bins_k = bin_of[sel]
        start = np.searchsorted(bins_k, np.arange(NBINS))
        pos = np.arange(bins_k.size) - start[bins_k]
        slot = off[bins_k] * CHUNK + pos
        srck[slot] = src_s[sel]
        dstrk[slot] = (local[sel] - bins_k * BIN).astype(np.float32)
        maskk[slot] = 1.0
        eak[slot] = ea_s[sel]

        # device layouts: edge e = c*128 + p
        src16 = np.tile(srck.reshape(-1, 16).T.astype(np.int16), (8, 1))
        dstr_d = dstrk.reshape(C_total, P).T.copy()
        import ml_dtypes
        ea5 = np.concatenate([eak.T, maskk[None, :]], axis=0).astype(ml_dtypes.bfloat16)

        xk = x[k * NPC:(k + 1) * NPC]
        xT = np.zeros((8, NW * P), np.float32)
        xT[:, :NPC] = xk.T
        bk = np.full(NW * P, Gn + 5, np.float32)
        bk[:NPC] = batch[k * NPC:(k + 1) * NPC].astype(np.float32)
        batch_d = bk.reshape(NW, P).T.copy()

        per_core.append(dict(SRC16=src16, DSTR=dstr_d, EAT=ea5,
                             XT=xT, BATCH=batch_d))

    # ---- weight folding
    w = {k: np.asarray(v, np.float32) for k, v in inputs.items()
         if k not in ("x", "edge_index", "edge_attr", "batch", "descriptors")}

    def vfold(We, ae, heads):
        Vp = (We.reshape(w["We_enc"].shape[1], heads, HID) * ae[None]).sum(-1)
        return w["We_enc"] @ Vp, w["be_enc"] @ Vp      # [4,heads],[heads]

    V2, bv2 = vfold(w["We2"], w["ae2"], 4)
    V3, bv3 = vfold(w["We3"], w["ae3"], 4)
    V4, bv4 = vfold(w["We4"], w["ae4"], 1)
    # [5,10]: rows = 4 edge-attr dims + mask; cols = 9 eterms + cnt
    W5x10 = np.zeros((5, 10), np.float32)
    W5x10[0:4, 0:9] = np.concatenate([V2, V3, V4], axis=1)
    W5x10[4, 0:9] = np.concatenate([bv2, bv3, bv4])
    W5x10[4, 9] = 1.0

    def padr(v, n):
        o = np.zeros(n, np.float32)
        o[: v.size] = v
        return o

    # channel-major reorder of the 128-wide (4 heads x 32 ch) dimension:
    # new position c*4+a holds old a*32+c. Keeps per-head broadcasts
    # innermost-packed on DVE (2x mode).
    cm = (np.arange(128) % 4) * 32 + np.arange(128) // 4

    def wext(W, att_s, att_d, heads):
        # append per-head b-columns: b_a = h . (att_s+att_d)_a
        att = (att_s + att_d).reshape(-1)  # [heads*HID] head-major
        if heads == 4:
            attc = att[cm]                 # channel-major to match W cols
            A = np.zeros((128, 4), np.float32)
            A[np.arange(128), np.arange(128) % 4] = attc
        else:
            A = att[:, None]               # [32,1]
        return np.concatenate([W, W @ A], axis=1)

    W1e = wext(w["W1"][:, cm], w["as1"], w["ad1"], 4)            # [8,132]
    W2e = wext(w["W2"][cm][:, cm], w["as2"], w["ad2"], 4)        # [128,132]
    W3e = wext(w["W3"][cm][:, cm], w["as3"], w["ad3"], 4)
    W4e = wext(w["W4"][cm], w["as4"], w["ad4"], 1)               # [128,33]

    bout = np.stack([padr(w["b1"][cm], 128), padr(w["b2"][cm], 128),
                     padr(w["b3"][cm], 128), padr(w["b4"], 128)])

    import ml_dtypes
    T0h = (x @ W1e).astype(np.float32)
    pk0 = np.zeros((N, 256), np.uint8)
    pk0[:, 0:8] = T0h[:, 128:132].astype(ml_dtypes.bfloat16).view(np.uint8)
    pk0[:, 8:136] = T0h[:, 0:128].astype(ml_dtypes.float8_e4m3).view(np.uint8)
    TG0 = pk0.view(ml_dtypes.bfloat16)

    gcnt = np.bincount(batch, minlength=Gn).astype(np.float32)
    cntr = (1.0 / np.maximum(gcnt, 1.0))[None, :]           # [1, Gn]
    shared = dict(
        W1=W1e, WL2=W2e, WL3=W3e, WL4=W4e, TG0=TG0,
        W5X10=W5x10, BOUT=bout, CNTR=cntr,
        WD=w["Wd"], BD=w["bd"][:, None], WLIN=w["Wl"], DESCT=desc.T.copy(),
    )
    bl = float(np.asarray(w["bl"]).reshape(-1)[0])

    dims = dict(N=N, E=E, Gn=Gn, NPC=NPC, NW=NW, NBINS=NBINS,
                C=C_total, cpb=cpb, off=off, bl=bl)
    return dims, shared, per_core


# ------------------------------------------------------------- program build
def build_program(dims, shared):
    import concourse.bass as bass
    import concourse.mybir as mybir
    import concourse.tile as tile
    import concourse.bacc as bacc
    from concourse.masks import make_identity
    from contextlib import ExitStack

    F32 = mybir.dt.float32
    FP8 = mybir.dt.float8e4
    BF16 = mybir.dt.bfloat16
    I32 = mybir.dt.int32
    I16 = mybir.dt.int16
    AF = mybir.ActivationFunctionType
    ALU = mybir.AluOpType
    AX = mybir.AxisListType

    N, Gn, NPC, NW, NBINS, C = (dims[k] for k in ("N", "Gn", "NPC", "NW", "NBINS", "C"))
    cpb, off, bl = dims["cpb"], dims["off"], dims["bl"]
    NSS = C // SS
    # layer params: h width, heads, rhs width, gather row elems
    # PK tables pack rows as [b bf16 x4 | h fp8 x128] (136B) in a 256B stride;
    # HX = leading h-columns multiplied on DVE straight from fp8 (1x mode), the
    # rest is cast to bf16 on Act first so the DVE part runs in 2x mode.
    HXD = dims.get("hx", {0: 64, 1: 48, 2: 48})
    PKL = dims.get("pk_layers", (0, 1, 2))
    LP = [dict(HW=128, AW=4, RW=142, EL=128 if 0 in PKL else ROW,
               GEL=68 if 0 in PKL else 132, PK=0 in PKL, HX=HXD[0]),
          dict(HW=128, AW=4, RW=132, EL=128, GEL=68, PK=1 in PKL, HX=HXD[1]),
          dict(HW=128, AW=4, RW=132, EL=128, GEL=68, PK=2 in PKL, HX=HXD[2]),
          dict(HW=32, AW=1, RW=33, EL=ROW4, GEL=34, PK=False, HX=32)]
    for l_ in (1, 2):
        if not LP[l_]["PK"]:
            LP[l_].update(EL=ROW, GEL=ROW)

    nc = bacc.Bacc(num_swdge_queues=2)
    SIM1 = dims.get("sim1", False)

    # ---- params
    pr = {}
    for nm, shp, dt in [("SRC16", [P, C * 8], I16), ("DSTR", [P, C], F32),
                        ("EAT", [5, C * CHUNK], BF16), ("XT", [8, NW * P], F32),
                        ("BATCH", [P, NW], F32), ("W1", [8, 132], F32),
                        ("WL2", [128, 132], F32), ("WL3", [128, 132], F32),
                        ("WL4", [128, 33], F32), ("W5X10", [5, 10], F32),
                        ("BOUT", [4, 128], F32),
                        ("WD", [48, 32], F32), ("BD", [32, 1], F32),
                        ("WLIN", [64, 1], F32), ("DESCT", [48, Gn], F32),
                        ("CNTR", [1, Gn], F32), ("TG0", [N, 128], BF16)]:
        pr[nm] = nc.declare_dram_parameter(nm, shp, dt, isOutput=False)
    out_p = nc.declare_dram_parameter("out", [1, Gn], F32, isOutput=True)
    pr_TG0_ph = pr["TG0"]

    # ---- internal DRAM
    T_loc = [None] + [nc.dram_tensor(f"T_loc{l}", [NPC, LP[l]["EL"]], BF16)
                      for l in range(1, 4)]
    T_glob = [pr_TG0_ph] + [nc.dram_tensor(f"T_glob{l}", [N, LP[l]["EL"]], BF16,
                                           addr_space="Shared")
                            for l in range(1, 4)]
    ar_in = nc.dram_tensor("ar_in", [32, Gn], F32)
    ar_out = nc.dram_tensor("ar_out", [32, Gn], F32, addr_space="Shared")

    # bin/window bookkeeping (compile-time)
    bin_of_chunk = []
    for b in range(NBINS):
        bin_of_chunk += [b] * int(cpb[b])
    BPW = P // BIN  # bins per window
    win_of_bin = [b // BPW for b in range(NBINS)]
    last_chunk_of_bin = {}
    first_chunk_of_bin = {}
    for c_i, b in enumerate(bin_of_chunk):
        last_chunk_of_bin[b] = c_i
        first_chunk_of_bin.setdefault(b, c_i)

    with tile.TileContext(nc) as tc, ExitStack() as ctx:
        cp = ctx.enter_context(tc.tile_pool(name="const", bufs=1))
        wp = ctx.enter_context(tc.tile_pool(name="work", bufs=2))
        vp = ctx.enter_context(tc.tile_pool(name="win", bufs=dims.get("vpb", 2)))
        pp = ctx.enter_context(tc.tile_pool(name="psum", bufs=2, space="PSUM"))
        bp = ctx.enter_context(tc.tile_pool(name="binp", bufs=2, space="PSUM"))

        sync, gps, vec, act, pe = nc.sync, nc.gpsimd, nc.vector, nc.scalar, nc.tensor

        def dma_gather_short(out_ap, in_ap, idxs_ap, num_idxs, elem_size,
                             elem_step, queue_num):
            from concourse.bass import exact_div
            eng = gps
            _in_ap = eng.lower_ap_dma(in_ap, for_custom_bir_dma=True)
            _idxs_ap = eng.lower_ap(idxs_ap)
            _out_ap = eng.lower_ap(out_ap)
            stride_bytes_256 = exact_div(elem_step * 2, 256)
            return eng.add_instruction(
                mybir.InstDMAGatherAnt(
                    name=eng.bass.get_next_instruction_name(),
                    ins=[*_in_ap, _idxs_ap,
                         eng.lower_val_access(eng.to_reg(num_idxs))],
                    outs=[_out_ap],
                    transpose=False, num_idxs=num_idxs, elem_size=elem_size,
                    stride_bytes_256=stride_bytes_256, gen_mode=0,
                    single_packet=False, queue_num=queue_num,
                    sbuf_tokens_per_rank=0, sbuf_free_dim_per_rank=0,
                    sbuf_free_dim_pad_per_rank=0, sbuf_byte_offset=0))
        ZTPB = dims.get("ztpb", 1)

        # ---- resident tiles
        src16 = cp.tile([P, C * 8], I16)
        sync.dma_start(out=src16[:], in_=pr["SRC16"][:, :])
        dstr = cp.tile([P, C], BF16)
        gps.dma_start(out=dstr[:], in_=pr["DSTR"][:, :])   # f32 -> bf16 cast
        batcht = cp.tile([P, NW], F32)
        sync.dma_start(out=batcht[:], in_=pr["BATCH"][:, :])
        xT_sb = cp.tile([8, NW * P], BF16)
        gps.dma_start(out=xT_sb[:], in_=pr["XT"][:, :])

        iota_i = cp.tile([P, BIN], I32)
        gps.iota(iota_i[:], pattern=[[1, BIN]], base=0, channel_multiplier=0)
        iotab = cp.tile([P, BIN], BF16)
        vec.tensor_copy(iotab[:], iota_i[:])
        iotag_i = cp.tile([P, Gn], I32)
        gps.iota(iotag_i[:], pattern=[[1, Gn]], base=0, channel_multiplier=0)
        iotagf = cp.tile([P, Gn], F32)
        vec.tensor_copy(iotagf[:], iotag_i[:])
        identf = cp.tile([P, P], F32)
        make_identity(nc, identf[:])

        w1_sb = cp.tile([8, 132], BF16)
        gps.dma_start(out=w1_sb[:], in_=pr["W1"][:, :])
        wl_sb = [None,
                 cp.tile([128, 132], BF16, name="wl2", tag="wl2"),
                 cp.tile([128, 132], BF16, name="wl3", tag="wl3"),
                 cp.tile([128, 33], BF16, name="wl4", tag="wl4")]
        gps.dma_start(out=wl_sb[1][:], in_=pr["WL2"][:, :])   # gpsimd casts f32->bf16
        gps.dma_start(out=wl_sb[2][:], in_=pr["WL3"][:, :])
        gps.dma_start(out=wl_sb[3][:], in_=pr["WL4"][:, :])
        w5x10 = cp.tile([5, 10], BF16)
        gps.dma_start(out=w5x10[:], in_=pr["W5X10"][:, :])
        bout_t = []
        for l in range(4):
            t3 = cp.tile([P, 128], F32, tag=f"bout{l}")
            sync.dma_start(out=t3[:], in_=pr["BOUT"][l:l + 1, :].to_broadcast([P, 128]))
            bout_t.append(t3)

        etc = cp.tile([P, C, 10], BF16)      # eterm9 | cnt  per edge
        pt_all = cp.tile([P, C, BIN], BF16)  # one-hot dst rows per edge
        loop_sb = cp.tile([P, NW, 10], F32)
        gsp = ctx.enter_context(tc.tile_pool(name="gsp", bufs=1, space="PSUM"))
        eap = ctx.enter_context(tc.tile_pool(name="eap", bufs=1))
        gsum_ps = None
        n_pool_mm = [0]

        # ---- readout head start: descriptor branch is input-independent
        comb = cp.tile([64, Gn], F32)
        wd_sb = cp.tile([48, 32], F32)
        sync.dma_start(out=wd_sb[:], in_=pr["WD"][:, :])
        desct_sb = cp.tile([48, Gn], F32)
        sync.dma_start(out=desct_sb[:], in_=pr["DESCT"][:, :])
        bd_sb = cp.tile([32, 1], F32)
        sync.dma_start(out=bd_sb[:], in_=pr["BD"][:, :])
        dps = pp.tile([32, Gn], F32, tag="hps", bufs=dims.get("hpsb", 2))
        pe.matmul(out=dps[:], lhsT=wd_sb[:], rhs=desct_sb[:], start=True, stop=True)
        act.activation(out=comb[32:64, :], in_=dps[:], func=AF.Relu, bias=bd_sb[:])
        wlin_sb = cp.tile([64, 1], F32)
        sync.dma_start(out=wlin_sb[:], in_=pr["WLIN"][:, :])
        cntrb = cp.tile([32, Gn], F32)
        sync.dma_start(out=cntrb[:], in_=pr["CNTR"][0:1, :].to_broadcast([32, Gn]))

        WG = dims.get("wg", 5)  # max windows per epilogue group
        # non-uniform groups: taper toward the end so the serial layer-boundary
        # tail (last epilogue -> node phase -> AllGather) shrinks
        grp_bounds = []
        w0_ = 0
        while NW - w0_ > 10:
            grp_bounds.append((w0_, WG))
            w0_ += WG
        for t_ in dims.get("taper", (4, 3, 2, 1)):
            if NW - w0_ > t_:
                grp_bounds.append((w0_, t_))
                w0_ += t_
        if NW > w0_:
            grp_bounds.append((w0_, NW - w0_))
        NG = len(grp_bounds)
        grp_of_win = {}
        for gi, (gw0, gsz_) in enumerate(grp_bounds):
            for w_ in range(gw0, gw0 + gsz_):
                grp_of_win[w_] = gi
        last_chunk_of_grp = {}
        for b in range(NBINS):
            g_ = grp_of_win[win_of_bin[b]]
            last_chunk_of_grp[g_] = max(last_chunk_of_grp.get(g_, -1),
                                        last_chunk_of_bin[b])

        # T_sb tables: [P, NW, 132] (h | b); layer l+1's is built during
        # layer l's edge phase, group by group.
        def node_phase_group(l, g_, T_next, z_src):
            """Build T_next rows for group g_ of layer l (0-based), write T_loc."""
            w0, gsz = grp_bounds[g_]
            HWn = LP[l]["HW"]
            BW = HWn + LP[l]["AW"]  # table row width
            for w_ in range(w0, w0 + gsz):
                if l == 0:
                    hps = pp.tile([P, 132], F32, tag="hps", bufs=dims.get("hpsb", 2))
                    pe.matmul(out=hps[:, 0:BW], lhsT=xT_sb[:, w_ * P:(w_ + 1) * P],
                              rhs=w1_sb[:], start=True, stop=True)
                else:
                    ztp = pp.tile([P, P], F32, tag="ztp", bufs=ZTPB)
                    pe.transpose(out=ztp[:], in_=z_src[:, w_ - w0, 0:128],
                                 identity=identf[:])
                    zt_sb = wp.tile([P, P], BF16, tag="ztsb")
                    act.copy(out=zt_sb[:], in_=ztp[:])
                    hps = pp.tile([P, 132], F32, tag="hps", bufs=dims.get("hpsb", 2))
                    pe.matmul(out=hps[:, 0:BW], lhsT=zt_sb[:], rhs=wl_sb[l][:],
                              start=True, stop=True)
                act.copy(out=T_next[:, w_, 0:BW], in_=hps[:, 0:BW])
                if l > 0 and LP[l]["PK"]:
                    act.copy(out=Tpk[:, w_, 0:4], in_=hps[:, 128:132])
                    act.copy(out=Tpk[:, w_, 4:68].bitcast(FP8), in_=hps[:, 0:128])
            if l == 0:
                return  # layer-1 table ships as the TG0 param; SBUF copy only
            stage, SW = (Tpk, 68) if LP[l]["PK"] else (T_next, BW)
            # batched table write: full windows in one DMA, ragged tail apart
            wfull = gsz - (1 if (w0 + gsz) * P > NPC else 0)
            if wfull > 0:
                sync.dma_start(
                    out=T_loc[l][w0 * P:(w0 + wfull) * P, 0:SW]
                        .rearrange("(w p) e -> p w e", p=P),
                    in_=stage[:, w0:w0 + wfull, 0:SW])
            if wfull < gsz:
                w_ = w0 + wfull
                nr = NPC - w_ * P
                sync.dma_start(out=T_loc[l][w_ * P:w_ * P + nr, 0:SW],
                               in_=stage[0:nr, w_, 0:SW])

        PT_AHEAD = dims.get("pt_ahead", 64)
        # prebuild the one-hot cache for the first chunks while the layer-0
        # node phase occupies PE/Act
        for g in range(0, PT_AHEAD, 8):
            vec.tensor_tensor(
                out=pt_all[:, g:g + 8, :],
                in0=dstr[:, g:g + 8].unsqueeze(2).to_broadcast([P, 8, BIN]),
                in1=iotab[:].unsqueeze(1).to_broadcast([P, 8, BIN]),
                op=ALU.is_equal)

        # ---- layer 0 node phase (all groups up front)
        T_sb = wp.tile([P, NW, 132], BF16, tag="tsb")
        for g_ in range(NG):
            node_phase_group(0, g_, T_sb, None)

        for l in range(4):
            HW, AW, RW, EL, GEL, PK, HX = (
                LP[l][k] for k in ("HW", "AW", "RW", "EL", "GEL", "PK", "HX"))
            BW = HW + AW

            T_next = None
            if l < 3:
                T_next = wp.tile([P, NW, 132], BF16, name="tnext", tag="tsb")
                if LP[l + 1]["PK"]:
                    Tpk = wp.tile([P, NW, 68], BF16, name="tpk", tag="tpk", bufs=1)

            grp_tiles = {}
            grp_done = set()

            def open_group(g_):
                t = vp.tile([P, WG, 142], F32, name="wingrp", tag="wingrp")
                grp_tiles[g_] = t
                return t

            def epilogue_group(g_):
                w0, gsz = grp_bounds[g_]
                wg = grp_tiles[g_]
                scr = wp.tile([P, WG, 12], F32, name="scr", tag="scr")
                # self-loop alpha (= b_own [+ eterm means]) -> exp
                if l > 0:
                    sl = [None, (0, 4), (4, 8), (8, 9)][l]
                    vec.tensor_tensor(out=scr[:, 0:gsz, 0:AW],
                                      in0=T_sb[:, w0:w0 + gsz, HW:HW + AW],
                                      in1=loop_sb[:, w0:w0 + gsz, sl[0]:sl[1]],
                                      op=ALU.add)
                else:
                    act.copy(out=scr[:, 0:gsz, 0:AW],
                             in_=T_sb[:, w0:w0 + gsz, HW:HW + AW])
                vec.tensor_scalar_mul(out=scr[:, 0:gsz, 4:4 + AW],
                                      in0=scr[:, 0:gsz, 0:AW], scalar1=0.2)
                vec.tensor_tensor(out=scr[:, 0:gsz, 0:AW], in0=scr[:, 0:gsz, 0:AW],
                                  in1=scr[:, 0:gsz, 4:4 + AW], op=ALU.max)
                act.activation(out=scr[:, 0:gsz, 0:AW], in_=scr[:, 0:gsz, 0:AW],
                               func=AF.Exp)
                # num += h_own * ex_loop
                nt = wp.tile([P, WG, 128], BF16, name="nt", tag="nt")
                vec.tensor_tensor(
                    out=nt[:, 0:gsz, 0:HW].rearrange("p g (c a) -> p g c a", a=AW),
                    in0=T_sb[:, w0:w0 + gsz, 0:HW].rearrange("p g (c a) -> p g c a", a=AW),
                    in1=scr[:, 0:gsz, 0:AW].unsqueeze(2)
                        .to_broadcast([P, gsz, HW // AW, AW]),
                    op=ALU.mult)
                vec.tensor_tensor(out=wg[:, 0:gsz, 0:HW], in0=wg[:, 0:gsz, 0:HW],
                                  in1=nt[:, 0:gsz, 0:HW], op=ALU.add)
                # den -> reciprocal
                vec.tensor_tensor(out=scr[:, 0:gsz, 4:4 + AW],
                                  in0=wg[:, 0:gsz, HW:HW + AW],
                                  in1=scr[:, 0:gsz, 0:AW], op=ALU.add)
                vec.tensor_scalar_add(out=scr[:, 0:gsz, 4:4 + AW],
                                      in0=scr[:, 0:gsz, 4:4 + AW], scalar1=1e-16)
                vec.reciprocal(out=scr[:, 0:gsz, 4:4 + AW], in_=scr[:, 0:gsz, 4:4 + AW])
                if l == 0:
                    vec.tensor_scalar_max(out=scr[:, 0:gsz, 8:9],
                                          in0=wg[:, 0:gsz, 141:142], scalar1=1.0)
                    vec.reciprocal(out=scr[:, 0:gsz, 8:9], in_=scr[:, 0:gsz, 8:9])
                    vec.tensor_tensor(
                        out=loop_sb[:, w0:w0 + gsz, 0:9], in0=wg[:, 0:gsz, 132:141],
                        in1=scr[:, 0:gsz, 8:9].to_broadcast([P, gsz, 9]), op=ALU.mult)
                # z = num * recip(den) + bias [+ relu]
                vec.tensor_tensor(
                    out=wg[:, 0:gsz, 0:HW].rearrange("p g (c a) -> p g c a", a=AW),
                    in0=wg[:, 0:gsz, 0:HW].rearrange("p g (c a) -> p g c a", a=AW),
                    in1=scr[:, 0:gsz, 4:4 + AW].unsqueeze(2)
                        .to_broadcast([P, gsz, HW // AW, AW]),
                    op=ALU.mult)
                vec.tensor_tensor(
                    out=wg[:, 0:gsz, 0:HW], in0=wg[:, 0:gsz, 0:HW],
                    in1=bout_t[l][:, 0:HW].unsqueeze(1).to_broadcast([P, gsz, HW]),
                    op=ALU.add)
                if l < 3:
                    act.activation(out=wg[:, 0:gsz, 0:128], in_=wg[:, 0:gsz, 0:128],
                                   func=AF.Relu)
                    node_phase_group(l + 1, g_, T_next, wg)
                else:
                    nonlocal gsum_ps
                    pool_sb = wp.tile([P, WG, 32], BF16, name="pool_sb", tag="poolsb")
                    act.copy(out=pool_sb[:, 0:gsz, 0:32], in_=wg[:, 0:gsz, 0:32])
                    bt = wp.tile([P, WG, Gn], BF16, name="bt", tag="bt", bufs=1)
                    vec.tensor_tensor(
                        out=bt[:, 0:gsz, :],
                        in0=batcht[:, w0:w0 + gsz].unsqueeze(2).to_broadcast([P, gsz, Gn]),
                        in1=iotagf[:].unsqueeze(1).to_broadcast([P, gsz, Gn]),
                        op=ALU.is_equal)
                    if gsum_ps is None:
                        gsum_ps = gsp.tile([32, Gn], F32, name="gsum_ps")
                    for j_ in range(gsz):
                        n_pool_mm[0] += 1
                        pe.matmul(out=gsum_ps[:], lhsT=pool_sb[:, j_, :],
                                  rhs=bt[:, j_, :],
                                  start=(n_pool_mm[0] == 1),
                                  stop=(n_pool_mm[0] == NW))
                grp_done.add(g_)

            cur_bin_tile = {}
            pending_epi = []
            SSL = dims.get("ss4", 96) if l == 3 else SS
            ss_plan = []
            rem_ = C
            for n_ in dims.get("ss_head", ()):
                if rem_ > n_ + SSL:
                    ss_plan.append(n_)
                    rem_ -= n_
            while rem_ > 0:
                n_ = min(SSL, rem_)
                ss_plan.append(n_)
                rem_ -= n_
            for t_ in dims.get("ss_tail", (12,)):
                if ss_plan[-1] > t_:
                    ss_plan[-1] -= t_
                    ss_plan.append(t_)
            s0 = 0
            GW = GEL if PK else max(GEL, RW)
            for ss, NCH in enumerate(ss_plan):
                Gt = wp.tile([P, SSL, GW], BF16, tag="gt" if GW > 68 else "gtp", bufs=4)
                if GEL == EL:
                    gps.dma_gather(
                        out_ap=Gt[:, 0:NCH, 0:GEL], in_ap=T_glob[l][:, :],
                        idxs_ap=src16[:, s0 * 8:(s0 + NCH) * 8],
                        num_idxs=NCH * CHUNK, num_idxs_reg=NCH * CHUNK,
                        elem_size=EL, single_packet=False, queue_num=ss % 2)
                else:
                    dma_gather_short(
                        out_ap=Gt[:, 0:NCH, 0:GEL], in_ap=T_glob[l][:, 0:GEL],
                        idxs_ap=src16[:, s0 * 8:(s0 + NCH) * 8],
                        num_idxs=NCH * CHUNK, elem_size=GEL, elem_step=EL,
                        queue_num=ss % 2)
                if l == 0:
                    # edge-term + mask precompute (feeds rhs cols 132:142 +
                    # later layers' alpha); mask folded into EAT row 5.
                    eaT_sl = eap.tile([5, SS * CHUNK], BF16, name="easl", tag="eat")
                    sync.dma_start(
                        out=eaT_sl[:, 0:NCH * CHUNK],
                        in_=pr["EAT"][:, s0 * CHUNK:(s0 + NCH) * CHUNK])
                    for q0 in range(0, NCH, 16):
                        qn = min(16, NCH - q0)
                        etp = pp.tile([P, 160], F32, tag="etp", bufs=dims.get("etpb", 2))
                        for j in range(qn):
                            ci = q0 + j
                            pe.matmul(out=etp[:, j * 10:(j + 1) * 10],
                                      lhsT=eaT_sl[:, ci * CHUNK:(ci + 1) * CHUNK],
                                      rhs=w5x10[:], start=True, stop=True)
                        act.copy(out=etc[:, s0 + q0:s0 + q0 + qn, :]
                                 .rearrange("p a b -> p (a b)"),
                                 in_=etp[:, 0:qn * 10])
                    # staircase one-hots built once, reused by all layers;
                    # built PT_AHEAD chunks ahead so the DVE cost sits in the
                    # pipeline's slack instead of its critical phase
                    pb0 = PT_AHEAD + s0
                    pb1 = min(pb0 + NCH, C)
                    for g in range(pb0, pb1, 8):
                        gn = min(8, pb1 - g)
                        vec.tensor_tensor(
                            out=pt_all[:, g:g + gn, :],
                            in0=dstr[:, g:g + gn].unsqueeze(2).to_broadcast([P, gn, BIN]),
                            in1=iotab[:].unsqueeze(1).to_broadcast([P, gn, BIN]),
                            op=ALU.is_equal)
                # alpha = b[src] (+ eterm) -> leaky relu -> exp
                AT = wp.tile([P, SSL, 8], BF16, tag="at", bufs=dims.get("atb", 2))
                if PK:
                    SCT = wp.tile([P, SS, 142], BF16, tag="rhs", bufs=dims.get("rhsb", 3))
                    BS = 0            # b slot in the packed gathered row
                else:
                    SCT = Gt
                    BS = HW
                if l > 0:
                    sl = [None, (0, 4), (4, 8), (8, 9)][l]
                    vec.tensor_tensor(out=AT[:, 0:NCH, 0:AW],
                                      in0=Gt[:, 0:NCH, BS:BS + AW],
                                      in1=etc[:, s0:s0 + NCH, sl[0]:sl[1]],
                                      op=ALU.add)
                    vec.tensor_scalar_mul(out=AT[:, 0:NCH, AW:2 * AW],
                                          in0=AT[:, 0:NCH, 0:AW], scalar1=0.2)
                    vec.tensor_tensor(out=AT[:, 0:NCH, 0:AW], in0=AT[:, 0:NCH, 0:AW],
                                      in1=AT[:, 0:NCH, AW:2 * AW], op=ALU.max)
                else:
                    vec.tensor_scalar_mul(out=AT[:, 0:NCH, AW:2 * AW],
                                          in0=Gt[:, 0:NCH, BS:BS + AW], scalar1=0.2)
                    vec.tensor_tensor(out=AT[:, 0:NCH, 0:AW],
                                      in0=Gt[:, 0:NCH, BS:BS + AW],
                                      in1=AT[:, 0:NCH, AW:2 * AW], op=ALU.max)
                act.activation(out=SCT[:, 0:NCH, HW:HW + AW], in_=AT[:, 0:NCH, 0:AW],
                               func=AF.Exp)
                if PK:
                    # h x ex: leading HX columns straight from fp8 on DVE (1x);
                    # the rest cast to bf16 on Act, then multiplied in 2x mode
                    vec.tensor_tensor(
                        out=SCT[:, 0:NCH, 0:HX].rearrange("p s (c a) -> p s c a", a=AW),
                        in0=Gt[:, 0:NCH, 4:4 + HX // 2].bitcast(FP8)
                            .rearrange("p s (c a) -> p s c a", a=AW),
                        in1=SCT[:, 0:NCH, HW:HW + AW].unsqueeze(2)
                            .to_broadcast([P, NCH, HX // AW, AW]),
                        op=ALU.mult)
                    if HX < HW:
                        act.copy(out=SCT[:, 0:NCH, HX:HW],
                                 in_=Gt[:, 0:NCH, 4 + HX // 2:4 + HW // 2].bitcast(FP8))
                        PHX = (dims.get("phx", 16) if l in (1, 2)
                               else (dims.get("phx0", 16) if l == 0 else 0))
                        DH = HW - PHX
                        vec.tensor_tensor(
                            out=SCT[:, 0:NCH, HX:DH].rearrange("p s (c a) -> p s c a", a=AW),
                            in0=SCT[:, 0:NCH, HX:DH].rearrange("p s (c a) -> p s c a", a=AW),
                            in1=SCT[:, 0:NCH, HW:HW + AW].unsqueeze(2)
                                .to_broadcast([P, NCH, (DH - HX) // AW, AW]),
                            op=ALU.mult)
                        if PHX:
                            gps.tensor_tensor(
                                out=SCT[:, 0:NCH, DH:HW].rearrange("p s (c a) -> p s c a", a=AW),
                                in0=SCT[:, 0:NCH, DH:HW].rearrange("p s (c a) -> p s c a", a=AW),
                                in1=SCT[:, 0:NCH, HW:HW + AW].unsqueeze(2)
                                    .to_broadcast([P, NCH, PHX // AW, AW]),
                                op=ALU.mult)
                else:
                    vec.tensor_tensor(
                        out=SCT[:, 0:NCH, 0:HW].rearrange("p s (c a) -> p s c a", a=AW),
                        in0=SCT[:, 0:NCH, 0:HW].rearrange("p s (c a) -> p s c a", a=AW),
                        in1=SCT[:, 0:NCH, HW:HW + AW].unsqueeze(2)
                            .to_broadcast([P, NCH, HW // AW, AW]),
                        op=ALU.mult)
                if l == 0:
                    # append eterm9|cnt as rhs cols 132:142
                    act.copy(out=SCT[:, 0:NCH, 132:142],
                             in_=etc[:, s0:s0 + NCH, :])
                # flush epilogues of groups completed last superstep: issuing
                # them here (after this superstep's alpha/exp/h*ex) keeps the
                # in-order DVE queue from blocking the critical per-edge ops
                if dims.get("defer_epi", 1):
                    for g_ in pending_epi:
                        epilogue_group(g_)
                    pending_epi.clear()
                # scatter matmuls
                for c_i in range(NCH):
                    gc = s0 + c_i
                    b = bin_of_chunk[gc]
                    w_ = win_of_bin[b]
                    g_ = grp_of_win[w_]
                    if g_ not in grp_tiles:
                        open_group(g_)
                    if gc == first_chunk_of_bin[b]:
                        cur_bin_tile[b] = bp.tile([BIN, 142], F32, name="binacc",
                                                  tag="binacc")
                    pe.matmul(out=cur_bin_tile[b][:, 0:RW],
                              lhsT=pt_all[:, gc, :], rhs=SCT[:, c_i, 0:RW],
                              start=(gc == first_chunk_of_bin[b]),
                              stop=(gc == last_chunk_of_bin[b]))
                    if gc == last_chunk_of_bin[b]:
                        j = b % BPW
                        wrel = w_ - grp_bounds[g_][0]
                        act.copy(out=grp_tiles[g_][BIN * j:BIN * (j + 1), wrel, 0:RW],
                                 in_=cur_bin_tile[b][:, 0:RW])
                        del cur_bin_tile[b]
                    if gc == last_chunk_of_grp.get(g_, None):
                        if dims.get("defer_epi", 1):
                            pending_epi.append(g_)
                        else:
                            epilogue_group(g_)
                s0 += NCH
            for g_ in pending_epi:
                epilogue_group(g_)
            pending_epi.clear()
            # groups never triggered (e.g. all-empty windows)
            for g_ in range(NG):
                if g_ not in grp_done:
                    if g_ not in grp_tiles:
                        open_group(g_)
                    epilogue_group(g_)
            if l < 3:
                if SIM1:
                    sync.dma_start(out=T_glob[l + 1][0:NPC, :], in_=T_loc[l + 1][:, :])
                else:
                    gps.collective_compute(
                        "AllGather", ALU.bypass, replica_groups=[list(range(NCORES))],
                        ins=[T_loc[l + 1][:, :]], outs=[T_glob[l + 1][:, :]])
                T_sb = T_next

        # ============ readout
        gsum_sb = cp.tile([32, Gn], F32)
        act.copy(out=gsum_sb[:], in_=gsum_ps[:])
        sync.dma_start(out=ar_in[:], in_=gsum_sb[:])
        if SIM1:
            sync.dma_start(out=ar_out[:], in_=ar_in[:])
        else:
            gps.collective_compute("AllReduce", ALU.add,
                                   replica_groups=[list(range(NCORES))],
                                   ins=[ar_in[:]], outs=[ar_out[:]])
        gs = cp.tile([32, Gn], F32)
        sync.dma_start(out=gs[:], in_=ar_out[:])
        vec.tensor_tensor(out=comb[0:32, :], in0=gs[:, :], in1=cntrb[:],
                          op=ALU.mult)
        blt = cp.tile([1, 1], F32)
        vec.memset(blt[:], bl)
        fin = pp.tile([1, Gn], F32, tag="hps", bufs=dims.get("hpsb", 2))
        pe.matmul(out=fin[:], lhsT=wlin_sb[:], rhs=comb[:], start=True, stop=True)
        res_sb = cp.tile([1, Gn], F32)
        act.activation(out=res_sb[:], in_=fin[:], func=AF.Sigmoid, bias=blt[:])
        sync.dma_start(out=out_p[:, :], in_=res_sb[:])

    nc.finalize()
    return nc


# ------------------------------------------------------------------ entry
def _run(inputs, trace=False, debug=False):
    dims, shared, per_core = host_prep(inputs)
    nc = build_program(dims, shared)
    in_maps = [{**shared, **pc} for pc in per_core]
    from concourse.bass_utils import run_bass_kernel_spmd
    return run_bass_kernel_spmd(nc, in_maps, list(range(NCORES)), trace=trace)


def kernel(**inputs):
    res = _run(inputs)
    return res.results[0]["out"].reshape(-1).astype(np.float32)
